# revision 36
# speedup vs baseline: 27.6662x; 1.1234x over previous
"""DBSS block as three SPMD bass launches on 8 NeuronCores."""
import os, sys
for _p in ('/opt/trn_rl_repo', os.path.expanduser('~/.axon_site/_ro/trn_rl_repo')):
    if os.path.isdir(_p) and _p not in sys.path:
        sys.path.insert(0, _p)

import numpy as np
from contextlib import ExitStack
import concourse.bass as bass
import concourse.mybir as mybir
from concourse import tile

# The walrus build in this container rejects TPB_CTRL instructions carrying
# more than one semaphore wait ("Too many sync wait commands" in codegen's
# setupSyncWait). Tile's kernel-tail drain aggregates one wait per live
# semaphore, so split that drain into a chain of single-wait drains.
_orig_drain_and_barrier = tile.TileContext._drain_and_barrier

def _split_drain_and_barrier(self, tick_clock, wait_clock):
    from concourse.vector_clock import ScopedClock
    import bass_rust as _br
    probe = self.nc.sync.drain()
    wait_clock.add_sem_waits(probe.ins, ScopedClock({None: tick_clock.global_clock}))
    waits = list(probe.ins.sync_info.on_wait) if probe.ins.sync_info else []
    if waits:
        probe.ins.sync_info = _br.SyncInfo(on_wait=[], on_update=[])
        scratch = self.nc.alloc_sbuf_tensor(f"tail_wait_scratch_{self.uid}", [1, 64],
                                            mybir.dt.float32)
        for wi, w in enumerate(waits):
            ins = self.nc.vector.memset(scratch.ap()[:, wi % 64:wi % 64 + 1], 0.0)
            ins.ins.sync_info = _br.SyncInfo(on_wait=[w], on_update=[])
    self.nc.all_engine_barrier()
    assert self.sems is not None
    popped = self.nc._tile_sem_poison_stack.pop()
    assert popped is self._sem_poison
    self.nc.clear_and_free_semaphores(list(self.sems.allocated().values()))
    self.nc.all_engine_barrier()

tile.TileContext._drain_and_barrier = _split_drain_and_barrier


def _new_bass():
    nc = bass.Bass()
    nc._mw_scratch = nc.alloc_sbuf_tensor("mw_scratch", [1, 512], mybir.dt.float32)
    nc._mw_sems = [nc.alloc_semaphore(f"mw_sem_{i}") for i in range(64)]
    return nc


def _fix_multiwaits(nc):
    """This walrus accepts at most one sem wait per instruction. Distribute
    extra waits over single-wait DVE memset carriers; same-engine targets are
    ordered behind their carriers by the engine stream, cross-engine targets
    wait on a helper semaphore bumped by the last carrier."""
    import bass_rust as _br
    scratch = nc._mw_scratch
    helper_sems = []
    use_count = {}    # sem num -> times used (wait threshold increases per reuse)
    scri = [0]
    rr = [0]
    for bbw in nc.main_func.blocks:
        insns = bbw.instructions
        out = []
        for ins in insns:
            si = ins.sync_info
            waits = list(si.on_wait) if si else []
            if len(waits) <= 1:
                out.append(ins)
                continue
            eng = str(ins.engine)
            mk = []
            def carrier(w, upd=None):
                si_ = scri[0] % 512
                scri[0] += 1
                c = mybir.InstMemset(name=nc.get_next_instruction_name(),
                                     mode="Const", constant=0, ins=[],
                                     outs=[nc.vector.lower_ap(scratch.ap()[:, si_:si_ + 1])])
                c.engine = ins.engine if eng in ("EngineType.DVE", "EngineType.Pool") else mybir.EngineType.DVE
                c.sync_info = _br.SyncInfo(on_wait=[w] if w else [],
                                           on_update=[upd] if upd else [])
                nc.register_instruction(c, overwrite=True)
                mk.append(c)
            if eng in ("EngineType.DVE", "EngineType.Pool"):
                for w in waits[:-1]:
                    carrier(w)
                ins.sync_info = _br.SyncInfo(on_wait=[waits[-1]],
                                             on_update=list(si.on_update) if si else [])
            else:
                assert nc._mw_sems, "helper semaphore pool exhausted"
                sem = nc._mw_sems[rr[0] % len(nc._mw_sems)]
                rr[0] += 1
                n = use_count.get(sem.num, 0) + 1
                use_count[sem.num] = n
                if n == 1:
                    helper_sems.append(sem)
                for w in waits[:-1]:
                    carrier(w)
                carrier(waits[-1],
                        _br.SyncUpdate(sync_type='semaphore', id=sem.num,
                                       ant_name=sem.name, update_mode='sem-inc',
                                       update_value=1, update_reg=None))
                ins.sync_info = _br.SyncInfo(
                    on_wait=[_br.SyncWait(sync_type='semaphore', id=sem.num,
                                          ant_name=sem.name, wait_mode='sem-ge-imm',
                                          wait_value=n, wait_reg=None)],
                    on_update=list(si.on_update) if si else [])
            out.extend(mk)
            out.append(ins)
        bbw.instructions = out
    if helper_sems:
        from concourse.bass import compact_to_ranges as _ctr
        nums = [s.num for s in helper_sems]
        first_bb = nc.main_func.blocks[0]
        home = nc.cur_bb.bb
        n0 = len(home.instructions)
        try:
            rngs = _ctr(sorted(nums))
        except Exception:
            rngs = [range(n, n + 1) for n in sorted(nums)]
        for r in rngs:
            nc.gpsimd.sem_clear(r)
        lst = home.instructions
        head_clears = lst[n0:]
        home.instructions = lst[:n0]
        first_bb.instructions = head_clears + first_bb.instructions
        for r in rngs:
            nc.gpsimd.sem_clear(r)

F32 = mybir.dt.float32
F32R = mybir.dt.float32r
AL = mybir.AluOpType
AF = mybir.ActivationFunctionType

B, C, H, W = 2, 64, 64, 64
D2, L, N, R = 32, H * W, 16, 2
HID = 256
EPS = 1e-5
NCORE = 8
LINEARIZE = True

# ---------------------------------------------------------------- host utils
def row_snake(H=64, W=64):
    idx = np.arange(H * W).reshape(H, W)
    idx[1::2] = idx[1::2, ::-1]
    return idx.reshape(-1)

def col_snake(H=64, W=64):
    idx = np.arange(H * W).reshape(H, W).T.copy()
    idx[1::2] = idx[1::2, ::-1]
    return idx.reshape(-1)

IDX_R = row_snake()
IDX_C = col_snake()
INV_R = np.argsort(IDX_R)
INV_C = np.argsort(IDX_C)

def _f(a):
    return np.ascontiguousarray(a, dtype=np.float32)


def _const(nc, name, arr):
    return nc.inline_tensor(_f(arr), name=name)


def ksel(nc, pool, name, allt_ap, P, F, K, oh_ap, out=None):
    """Select the oh-weighted sum of K stacked (P,F) blocks of allt_ap.
    Intermediate accumulators rotate through a shared scratch tag so
    concurrent ksel results only pay for their dedicated `out` tile."""
    if out is None:
        out = pool.tile([P, F], F32, name=f"sel_{name}")
    cur = None
    for k in range(K):
        dst = out if k == K - 1 else pool.tile([P, F], F32, name=f"kscr_{name}_{k}",
                                               tag=f"ksel_scr_{P}x{F}", bufs=2)
        if k == 0:
            nc.vector.tensor_scalar(dst[:], allt_ap[:, 0:F], oh_ap[0:P, 0:1],
                                    None, AL.mult)
        else:
            nc.vector.scalar_tensor_tensor(dst[:], allt_ap[:, k * F:(k + 1) * F],
                                           oh_ap[0:P, k:k + 1], cur[:], AL.mult, AL.add)
        cur = dst
    return out

# ================================================================ LAUNCH 1
# ln1 over channels + 3x3 dilated (=2) depthwise conv.
# core i: b=i//4, q=i%4. own rows [16q,16q+16); stored 20 rows [16q-2,16q+18)
# partitions p = 64*h + c, h in {0,1}: half h stored-local rows [8h, 8h+12)
# tile free layout (12, 68), cols 2:66 are real, pad cols zero.

def mm_chunks(nc, out_ap, lhsT_ap, rhs_ap, chunk=512, f32r=True, acc=False):
    """Chunked matmul along free dim (PSUM bank limit). out/rhs are 2D flat APs."""
    F = out_ap.shape[-1]
    o = 0
    while o < F:
        n = min(chunk, F - o)
        lh, rh = lhsT_ap, rhs_ap[:, o:o + n]
        if f32r:
            lh, rh = lh.bitcast(F32R), rh.bitcast(F32R)
        if acc:
            nc.tensor.matmul(out_ap[:, o:o + n], lh, rh, start=acc[0], stop=acc[1],
                             skip_group_check=True)
        else:
            nc.tensor.matmul(out_ap[:, o:o + n], lh, rh)
        o += n


def build_l1(cw):
    nc = _new_bass()
    xs = nc.dram_tensor("xs", [C, 20, W], F32, kind="ExternalInput")
    oh = nc.dram_tensor("oh", [128, 8], F32, kind="ExternalInput")
    selsum = _const(nc, "selsum", cw["selsum"])
    selg = _const(nc, "selg", cw["selg"])
    b2 = _const(nc, "b2", cw["b2"])
    w9 = _const(nc, "w9", cw["w9"])
    cb = _const(nc, "cb", cw["cb"])
    cst = _const(nc, "cst", cw["cst"])  # [eps]
    mask_all = _const(nc, "mask_all", cw["mask_all"])  # (128, 4*12)
    hs = nc.dram_tensor("hs", [C, 16, W], F32, kind="ExternalOutput")

    with tile.TileContext(nc, linearize=LINEARIZE) as tc, ExitStack() as ctx:
        pool = ctx.enter_context(tc.tile_pool(name="pool", bufs=1))
        psum = ctx.enter_context(tc.tile_pool(name="psum", bufs=1, space="PSUM"))

        xt = pool.tile([128, 12, 68], F32)
        nc.vector.memset(xt[:], 0.0)
        for h in (0, 1):
            nc.sync.dma_start(xt[64 * h:64 * h + 64, :, 2:66], xs[:, 8 * h:8 * h + 12, :])
        selsum_t = pool.tile([128, 2], F32)
        selg_t = pool.tile([2, 128], F32)
        b2_t = pool.tile([128, 1], F32)
        w9_t = pool.tile([128, 9], F32)
        cb_t = pool.tile([128, 1], F32)
        oh_t = pool.tile([128, 8], F32)
        mask_all_t = pool.tile([128, 48], F32)
        cst_t = pool.tile([2, 2], F32)
        nc.sync.dma_start(selsum_t[:], selsum[:])
        nc.sync.dma_start(selg_t[:], selg[:])
        nc.sync.dma_start(b2_t[:], b2[:])
        nc.sync.dma_start(w9_t[:], w9[:])
        nc.sync.dma_start(cb_t[:], cb[:])
        nc.sync.dma_start(oh_t[:], oh[:])
        nc.sync.dma_start(mask_all_t[:], mask_all[:])
        nc.sync.dma_start(cst_t[:], cst[:])
        mask_t = ksel(nc, pool, "mask", mask_all_t[:], 128, 12, 4, oh_t[:])

        XW = xt[:, :, 2:66]                      # (128, 12, 64)
        sq = pool.tile([128, 12, 64], F32)
        nc.scalar.activation(sq[:], XW, AF.Square)

        st_x = psum.tile([2, 768], F32)
        st_xx = psum.tile([2, 768], F32)
        for r0, r1 in ((0, 8), (8, 12)):
            nc.tensor.matmul(st_x[:, r0 * 64:r1 * 64], selsum_t[:], xt[:, r0:r1, 2:66])
            nc.tensor.matmul(st_xx[:, r0 * 64:r1 * 64], selsum_t[:], sq[:, r0:r1, :])

        # per-pixel stats on (2,768)
        sm = pool.tile([2, 768], F32)
        nc.vector.tensor_scalar(sm[:], st_x[:], 1.0 / 64, None, AL.mult)
        var = pool.tile([2, 768], F32)
        nc.vector.tensor_tensor(var[:], sm[:], sm[:], AL.mult)
        nc.vector.scalar_tensor_tensor(var[:], st_xx[:], 1.0 / 64, var[:], AL.mult, AL.subtract)
        inv = pool.tile([2, 768], F32)
        nc.scalar.activation(inv[:], var[:], AF.Ln, bias=cst_t[:, 0:1])
        nc.scalar.activation(inv[:], inv[:], AF.Exp, scale=-0.5)
        minv = pool.tile([2, 768], F32)
        nc.vector.tensor_tensor(minv[:], sm[:], inv[:], AL.mult)

        sgb = psum.tile([128, 12, 64], F32)
        msgb = psum.tile([128, 12, 64], F32)
        for r0, r1 in ((0, 8), (8, 12)):
            nc.tensor.matmul(sgb[:, r0:r1, :], selg_t[:], inv[:, r0 * 64:r1 * 64])
            nc.tensor.matmul(msgb[:, r0:r1, :], selg_t[:], minv[:, r0 * 64:r1 * 64])

        xn = pool.tile([128, 12, 64], F32)
        nc.vector.tensor_tensor(xn[:], XW, sgb[:], AL.mult)
        nc.vector.scalar_tensor_tensor(xn[:], xn[:], b2_t[:], msgb[:], AL.add, AL.subtract)
        # masked into padded buffer
        xmp = pool.tile([128, 12, 68], F32)
        nc.vector.memset(xmp[:], 0.0)
        nc.vector.tensor_tensor(xmp[:, :, 2:66], xn[:],
                                mask_t[:].unsqueeze(2).broadcast_to([128, 12, 64]), AL.mult)

        # conv: out rows = stored 2..10 per half
        acc0 = pool.tile([128, 8, 64], F32)
        acc1 = pool.tile([128, 8, 64], F32)
        acc = [acc0, acc1]
        taps = [(dy, dx) for dy in (-2, 0, 2) for dx in (-2, 0, 2)]
        cur = 0
        for ti, (dy, dx) in enumerate(taps):
            src = xmp[:, 2 + dy:10 + dy, 2 + dx:66 + dx]
            if ti == 0:
                nc.vector.tensor_scalar(acc[0][:], src, w9_t[:, 0:1], cb_t[:], AL.mult, AL.add)
            else:
                eng = nc.vector
                eng.scalar_tensor_tensor(acc[1 - cur][:], src, w9_t[:, ti:ti + 1], acc[cur][:], AL.mult, AL.add)
                cur = 1 - cur
        for h in (0, 1):
            nc.sync.dma_start(hs[:, 8 * h:8 * h + 8, :], acc[cur][64 * h:64 * h + 64])
    _fix_multiwaits(nc)
    return nc


def l1_consts(ln1_g, ln1_b, con1_w, con1_b):
    selsum = np.zeros((128, 2), np.float32)
    selg = np.zeros((2, 128), np.float32)
    b2 = np.zeros((128, 1), np.float32)
    w9 = np.zeros((128, 9), np.float32)
    cbv = np.zeros((128, 1), np.float32)
    for p in range(128):
        h, c = p // 64, p % 64
        selsum[p, h] = 1.0
        selg[h, p] = ln1_g[c]
        b2[p, 0] = ln1_b[c]
        w9[p] = con1_w[c, 0].reshape(-1)
        cbv[p, 0] = con1_b[c]
    cst = np.zeros((2, 2), np.float32); cst[:, 0] = EPS
    mask_all = np.zeros((128, 4, 12), np.float32)
    for q in range(4):
        r0 = 16 * q
        for p in range(128):
            h = p // 64
            for r in range(12):
                g = r0 + 8 * h + r - 2
                if 0 <= g < H:
                    mask_all[p, q, r] = 1.0
    return dict(selsum=selsum, selg=selg, b2=b2, w9=w9, cb=cbv, cst=cst,
                mask_all=mask_all.reshape(128, 48))


def core_oh(i):
    oh = np.zeros((128, 8), np.float32)
    oh[:, i % 4] = 1.0
    oh[:, 4 + i // 4] = 1.0
    return oh


def l1_inputs(x):
    maps = []
    for i in range(NCORE):
        b, q = i // 4, i % 4
        r0 = 16 * q
        xs = np.zeros((C, 20, W), np.float32)
        lo, hi = r0 - 2, r0 + 18
        slo, shi = max(lo, 0), min(hi, H)
        xs[:, slo - lo:shi - lo, :] = x[b, :, slo:shi, :]
        maps.append(dict(xs=_f(xs), oh=core_oh(i)))
    return maps


def l1_gather(results):
    h = np.zeros((B, C, H, W), np.float32)
    for i in range(NCORE):
        b, q = i // 4, i % 4
        h[b, :, 16 * q:16 * q + 16, :] = results[i]["hs"]
    return h

# ================================================================ LAUNCH 2
# Selective scan for one direction k on one batch b per core.
# partitions p = 32*j + d (j = n_local 0..3, n = 4g+j), free = t (4096).

F32R_PROJ = False   # delta projection in full fp32 (precision-sensitive)
F32R_BC = False      # B4/C4 expansion matmuls in f32r
F32R_Y = False       # y reduction matmuls in f32r

def build_l2(cw):
    nc = _new_bass()
    u = nc.dram_tensor("u", [D2, L], F32, kind="ExternalInput")
    oh = nc.dram_tensor("oh", [128, 8], F32, kind="ExternalInput")
    d_all = _const(nc, "d_all", cw["d_all"])          # (32, 4*128)
    bias_all = _const(nc, "bias_all", cw["bias_all"])  # (128, 4)
    B_all = _const(nc, "B_all", cw["B_all"])          # (32, 4*512)
    C_all = _const(nc, "C_all", cw["C_all"])          # (32, 4*512)
    u4_c = _const(nc, "u4_c", cw["u4"])               # (32, 128)
    A_all = _const(nc, "A_all", cw["A_all"])          # (128, 16)
    y_c = _const(nc, "y_c", cw["y_lhsT"])             # (128, 32)
    Ds_all = _const(nc, "Ds_all", cw["Ds_all"])       # (32, 4*32)
    y = nc.dram_tensor("y", [D2, L], F32, kind="ExternalOutput")

    NCH = 8           # 512-column chunks
    CH = L // NCH

    with tile.TileContext(nc, linearize=LINEARIZE) as tc, ExitStack() as ctx:
        pool = ctx.enter_context(tc.tile_pool(name="pool", bufs=1))
        psA = ctx.enter_context(tc.tile_pool(name="psA", bufs=3, space="PSUM"))
        psY = ctx.enter_context(tc.tile_pool(name="psY", bufs=1, space="PSUM"))

        ut = pool.tile([D2, L], F32)
        nc.sync.dma_start(ut[:], u[:])
        oh_t = pool.tile([128, 8], F32)
        nc.sync.dma_start(oh_t[:], oh[:])
        lhsT_u4_t = pool.tile([D2, 128], F32)
        lhsT_y_t = pool.tile([128, D2], F32)
        nc.sync.dma_start(lhsT_u4_t[:], u4_c[:])
        nc.sync.dma_start(lhsT_y_t[:], y_c[:])
        def ksel_dram(nm, hd, P_, F_, fc=None):
            """Chunked DMA of a (P_, 4*F_) stacked DRAM const + oh-select."""
            outt = pool.tile([P_, F_], F32, name=f"sel_{nm}")
            fc = fc or F_
            for o in range(0, F_, fc):
                st = pool.tile([P_, 4 * fc], F32, name=f"st_{nm}_{o}",
                               tag=f"kst_{P_}x{4 * fc}", bufs=2)
                for k in range(4):
                    nc.sync.dma_start(st[:, k * fc:(k + 1) * fc],
                                      hd[:, k * F_ + o:k * F_ + o + fc])
                cur = None
                for k in range(4):
                    if k == 3:
                        dst = outt[:, o:o + fc]
                    else:
                        dst = pool.tile([P_, fc], F32, name=f"ks_{nm}_{o}_{k}",
                                        tag=f"kscr_{P_}x{fc}", bufs=2)[:]
                    if k == 0:
                        nc.vector.tensor_scalar(dst, st[:, 0:fc], oh_t[0:P_, 0:1],
                                                None, AL.mult)
                    else:
                        nc.vector.scalar_tensor_tensor(dst, st[:, k * fc:(k + 1) * fc],
                                                       oh_t[0:P_, k:k + 1], cur,
                                                       AL.mult, AL.add)
                    cur = dst
            return outt

        stack_t = {}
        for nm, hd, P_, F_, fc in (("d", d_all, D2, 128, None),
                                   ("bias", bias_all, 128, 1, None),
                                   ("B", B_all, D2, 512, 128),
                                   ("C", C_all, D2, 512, 128),
                                   ("A", A_all, 128, 4, None),
                                   ("Ds", Ds_all, D2, D2, None)):
            stack_t[nm] = ksel_dram(nm, hd, P_, F_, fc)
        lhsT_d_t = stack_t["d"]
        bias4_t = stack_t["bias"]
        lhsT_B_t = stack_t["B"]
        lhsT_C_t = stack_t["C"]
        A4_t = stack_t["A"]
        lhsT_Ds_t = stack_t["Ds"]

        def mm(out_ap, lh, rh, f32r, **kw):
            if f32r:
                lh, rh = lh.bitcast(F32R), rh.bitcast(F32R)
            nc.tensor.matmul(out_ap, lh, rh, **kw)

        d4 = pool.tile([128, L], F32)
        w4 = pool.tile([128, L], F32)
        # --- delta, w
        for c in range(NCH):
            dp = psA.tile([128, CH], F32, name=f"dp{c}", tag="ps")
            mm(dp[:], lhsT_d_t[:], ut[:, c * CH:(c + 1) * CH], F32R_PROJ)
            # softplus(x+b) = ln(1 + exp(x+b)); keeps ACT in the ln/exp table
            nc.scalar.activation(d4[:, c * CH:(c + 1) * CH], dp[:], AF.Exp,
                                 bias=bias4_t[:])
            nc.scalar.activation(d4[:, c * CH:(c + 1) * CH],
                                 d4[:, c * CH:(c + 1) * CH], AF.Ln,
                                 bias=nc.const_aps.tensor(1.0, (128, 1)))
        for c in range(NCH):
            u4p = psA.tile([128, CH], F32, name=f"u4p{c}", tag="ps")
            mm(u4p[:], lhsT_u4_t[:], ut[:, c * CH:(c + 1) * CH], F32R_BC)
            nc.vector.tensor_tensor(w4[:, c * CH:(c + 1) * CH],
                                    d4[:, c * CH:(c + 1) * CH], u4p[:], AL.mult)

        hs = []
        for g in range(4):
            dBu = pool.tile([128, L], F32, name=f"dBu{g}", tag="dBu", bufs=1)
            for c in range(NCH):
                b4 = psA.tile([128, CH], F32, name=f"b4_{g}_{c}", tag="ps")
                mm(b4[:], lhsT_B_t[:, g * 128:(g + 1) * 128],
                   ut[:, c * CH:(c + 1) * CH], F32R_BC)
                nc.vector.tensor_tensor(dBu[:, c * CH:(c + 1) * CH],
                                        w4[:, c * CH:(c + 1) * CH], b4[:], AL.mult)
            dA = pool.tile([128, L], F32, name=f"dA{g}", tag="dA", bufs=2)
            nc.scalar.activation(dA[:], d4[:], AF.Exp, scale=A4_t[:, g:g + 1])
            hsg = pool.tile([128, L], F32, name=f"hs{g}")
            eng = nc.vector
            eng.tensor_tensor_scan(hsg[:], dA[:], dBu[:], 0.0, AL.mult, AL.add)
            hs.append(hsg)

        # --- phase B: y = sum_g lhsT_y.T @ (hs_g * C4_g) + diag(Ds) @ u
        for half in range(2):
            yps = psY.tile([D2, L // 2], F32, name=f"yps{half}", tag="yps")
            for g in range(4):
                fsb = pool.tile([128, L // 2], F32, name=f"f_{half}_{g}", tag="fsb", bufs=2)
                for cc in range(NCH // 2):
                    c = half * (NCH // 2) + cc
                    c4 = psA.tile([128, CH], F32, name=f"c4_{g}_{c}", tag="ps")
                    mm(c4[:], lhsT_C_t[:, g * 128:(g + 1) * 128],
                       ut[:, c * CH:(c + 1) * CH], F32R_BC)
                    if g % 2 == 0:
                        nc.vector.tensor_tensor(fsb[:, cc * CH:(cc + 1) * CH],
                                                hs[g][:, c * CH:(c + 1) * CH], c4[:], AL.mult)
                    else:
                        c4sb = pool.tile([128, CH], F32, name=f"c4sb_{g}_{c}", tag="c4sb", bufs=2)
                        nc.scalar.copy(c4sb[:], c4[:])
                        nc.vector.tensor_tensor(fsb[:, cc * CH:(cc + 1) * CH],
                                                hs[g][:, c * CH:(c + 1) * CH], c4sb[:], AL.mult)
                for cc in range(NCH // 2):
                    mm(yps[:, cc * CH:(cc + 1) * CH], lhsT_y_t[:],
                       fsb[:, cc * CH:(cc + 1) * CH], F32R_Y,
                       start=(g == 0), stop=False, skip_group_check=True)
            for cc in range(NCH // 2):
                c = half * (NCH // 2) + cc
                mm(yps[:, cc * CH:(cc + 1) * CH], lhsT_Ds_t[:],
                   ut[:, c * CH:(c + 1) * CH], F32R_Y,
                   start=False, stop=True, skip_group_check=True)
            ysb = pool.tile([D2, L // 2], F32, name=f"ysb{half}", tag="ysb", bufs=1)
            nc.scalar.copy(ysb[:], yps[:])
            nc.sync.dma_start(y[:, half * (L // 2):(half + 1) * (L // 2)], ysb[:])
    _fix_multiwaits(nc)
    return nc


def l2_consts(xproj_w, dtproj_w, dtproj_b, A_log, Ds):
    A = -np.exp(np.asarray(A_log))
    d_all = np.zeros((D2, 4, 128), np.float32)
    B_all = np.zeros((D2, 4, 512), np.float32)
    C_all = np.zeros((D2, 4, 512), np.float32)
    bias_all = np.zeros((128, 4), np.float32)
    A_all = np.zeros((128, 16), np.float32)
    Ds_all = np.zeros((D2, 4, D2), np.float32)
    u4 = np.zeros((D2, 128), np.float32)
    y_lhsT = np.zeros((128, D2), np.float32)
    for k in range(4):
        Wd = dtproj_w[k] @ xproj_w[k, :R]        # (32, 32)
        Ds_all[:, k, :] = np.diag(Ds[k])
        for j in range(4):
            for d in range(D2):
                p = 32 * j + d
                d_all[:, k, p] = Wd[d]
                u4[d, p] = 1.0
                bias_all[p, k] = dtproj_b[k, d]
                y_lhsT[p, d] = 1.0
                for g in range(4):
                    n = 4 * g + j
                    B_all[:, k, g * 128 + p] = xproj_w[k, R + n]
                    C_all[:, k, g * 128 + p] = xproj_w[k, R + N + n]
                    A_all[p, 4 * k + g] = A[k, d, n]
    return dict(d_all=d_all.reshape(D2, 512), bias_all=bias_all,
                B_all=B_all.reshape(D2, 2048), C_all=C_all.reshape(D2, 2048),
                u4=u4, A_all=A_all, y_lhsT=y_lhsT, Ds_all=Ds_all.reshape(D2, 128))


def l2_inputs(h):
    """h: (B, 64, H, W) conv output. returns 8 in_maps, core i -> (b=i//4, k=i%4)."""
    xf = h.reshape(B, C, L)
    maps = []
    for i in range(NCORE):
        b, k = i // 4, i % 4
        half = xf[b, :D2] if k < 2 else xf[b, D2:]
        perm = IDX_R if k < 2 else IDX_C
        uu = half[:, perm]
        if k % 2 == 1:
            uu = uu[:, ::-1]
        maps.append(dict(u=_f(uu), oh=core_oh(i)))
    return maps

# ================================================================ LAUNCH 3
# core i: b=i//4, q=i%4; own rows [16q,16q+16); stored 22 rows [16q-3,16q+19)
# partitions p = 64h + c ; half h stored-local rows [8h, 8h+14)
# free layout (14, 70), real cols 3:67; own window rows [3,11) cols [3,67)

NPIX = 8192.0   # B*H*W
NPB = 4096.0    # H*W

def build_l3(cw):
    nc = _new_bass()
    y4 = nc.dram_tensor("y4", [4, D2, 22, W], F32, kind="ExternalInput")
    xs = nc.dram_tensor("xs", [C, 22, W], F32, kind="ExternalInput")
    oh = nc.dram_tensor("oh", [128, 8], F32, kind="ExternalInput")
    inp = {"y4": y4, "xs": xs}
    cshapes = [("rowm_all", [128, 56]), ("bmask_all", [128, 16]),
               ("selsum", [128, 2]), ("selhp", [128, 128]),
               ("sg_ssm", [2, 128]), ("sb_ssm", [128, 1]),
               ("sg_ln2", [2, 128]), ("sb_ln2", [128, 1]),
               ("projT", [128, 128]), ("projb", [128, 1]),
               ("bnp_g", [128, 1]), ("bnp_b", [128, 1]),
               ("ecaT", [128, 128]),
               ("fc1T", [128, 512]),
               ("bn1_g", [128, 4]), ("bn1_b", [128, 4]),
               ("dw_w", [128, 4, 49]), ("dw_b", [128, 4]),
               ("bn2_g", [128, 4]), ("bn2_b", [128, 4]),
               ("fc2T", [128, 4, 128]),
               ("bn3_g", [128, 1]), ("bn3_b", [128, 1]),
               ("cst", [128, 2])]
    consts = {nm: _const(nc, nm, cw[nm].reshape(shp)) for nm, shp in cshapes}
    out = nc.dram_tensor("out", [C, 16, W], F32, kind="ExternalOutput")
    cc_in = [nc.dram_tensor(f"ccin{r}", [128, 16], F32) for r in range(4)]
    cc_out = [nc.dram_tensor(f"ccout{r}", [NCORE * 128, 16], F32) for r in range(4)]

    RW = 70           # row width incl pads
    FF = 14 * RW      # 980
    OWN = (slice(3, 11), slice(3, 67))

    with tile.TileContext(nc, linearize=LINEARIZE) as tc, ExitStack() as ctx:
        pool = ctx.enter_context(tc.tile_pool(name="pool", bufs=1))
        psT = ctx.enter_context(tc.tile_pool(name="psT", bufs=2, space="PSUM"))
        psS = ctx.enter_context(tc.tile_pool(name="psS", bufs=2, space="PSUM"))

        T = {}
        for nm, shp in cshapes:
            T[nm] = pool.tile(shp, F32, name=f"t_{nm}")
            nc.sync.dma_start(T[nm][:], consts[nm][:])
        eps_ap = T["cst"][:, 0:1]
        oh_t = pool.tile([128, 8], F32)
        nc.sync.dma_start(oh_t[:], oh[:])
        rowm = ksel(nc, pool, "rowm", T["rowm_all"][:], 128, 14, 4, oh_t[:])
        bmask_t = ksel(nc, pool, "bmask", T["bmask_all"][:], 128, 8, 2, oh_t[:, 4:6])

        def chunks2(tile3, rows=14):
            """two row-chunks of a (128,14,70) tile"""
            return [tile3[:, 0:7, :], tile3[:, 7:14, :]]

        def ln_ch(src, selg_key, b_key, nm):
            """channel LayerNorm of (128,14,70) tile -> new tile"""
            sq = pool.tile([128, 14, RW], F32, name=f"sq_{nm}", tag="lnsq")
            nc.scalar.activation(sq[:], src[:], AF.Square)
            inv = pool.tile([2, 14, RW], F32, name=f"inv_{nm}", tag="lninv")
            minv = pool.tile([2, 14, RW], F32, name=f"minv_{nm}", tag="lnminv")
            for ci, (s_ap, q_ap) in enumerate(zip(chunks2(src), chunks2(sq))):
                px = psS.tile([2, 7 * RW], F32, name=f"px_{nm}{ci}", tag="lnst")
                pq = psS.tile([2, 7 * RW], F32, name=f"pq_{nm}{ci}", tag="lnst")
                nc.tensor.matmul(px[:], T["selsum"][:], s_ap)
                nc.tensor.matmul(pq[:], T["selsum"][:], q_ap)
                ivc = inv[:, 7 * ci:7 * ci + 7, :]
                mvc = minv[:, 7 * ci:7 * ci + 7, :]
                sm = pool.tile([2, 7, RW], F32, name=f"sm_{nm}{ci}", tag="lnsm")
                nc.vector.tensor_scalar(sm[:], px[:], 1.0 / 64, None, AL.mult)
                nc.vector.tensor_tensor(ivc, sm[:], sm[:], AL.mult)
                nc.vector.scalar_tensor_tensor(ivc, pq[:], 1.0 / 64, ivc, AL.mult, AL.subtract)
                nc.scalar.activation(ivc, ivc, AF.Ln, bias=T["cst"][0:2, 0:1])
                nc.scalar.activation(ivc, ivc, AF.Exp, scale=-0.5)
                nc.vector.tensor_tensor(mvc, sm[:], ivc, AL.mult)
            dst = pool.tile([128, 14, RW], F32, name=f"ln_{nm}")
            for ci in range(2):
                rs = slice(7 * ci, 7 * ci + 7)
                sgb = psS.tile([128, 7 * RW], F32, name=f"sgb_{nm}{ci}", tag="lnbc")
                msgb = psS.tile([128, 7 * RW], F32, name=f"msgb_{nm}{ci}", tag="lnbc")
                nc.tensor.matmul(sgb[:], T[selg_key][:], inv[:, rs, :])
                nc.tensor.matmul(msgb[:], T[selg_key][:], minv[:, rs, :])
                nc.vector.tensor_tensor(dst[:, rs, :], src[:, rs, :],
                                        sgb[:].rearrange("p (a b) -> p a b", a=7), AL.mult)
                nc.vector.scalar_tensor_tensor(dst[:, rs, :], dst[:, rs, :], T[b_key][:],
                                               msgb[:].rearrange("p (a b) -> p a b", a=7),
                                               AL.add, AL.subtract)
            return dst

        def allgather(rnd, cols_src_ap, ncols):
            """partials (128, ncols) -> gathered sbuf tile (128, ncols, 8)"""
            ci = pool.tile([128, 16], F32, name=f"cci_{rnd}", tag="cci")
            nc.vector.memset(ci[:], 0.0)
            nc.vector.tensor_copy(ci[:, 0:ncols], cols_src_ap)
            nc.sync.dma_start(cc_in[rnd][:], ci[:])
            nc.gpsimd.collective_compute(
                "AllGather", AL.bypass, replica_groups=[list(range(NCORE))],
                ins=[cc_in[rnd][:]], outs=[cc_out[rnd][:]])
            gat = pool.tile([128, 16, NCORE], F32, name=f"gat_{rnd}", tag="gat")
            src = cc_out[rnd][:].rearrange("(n p) c -> p c n", p=128)
            nc.sync.dma_start(gat[:, 0:16, :], src)
            return gat

        # ---- merge y4 into ym
        ya = pool.tile([128, 14, RW], F32)
        yb = pool.tile([128, 14, RW], F32)
        nc.vector.memset(ya[:], 0.0)
        nc.vector.memset(yb[:], 0.0)
        for h in (0, 1):
            for m, dsttile in ((0, ya), (1, yb), (2, ya), (3, yb)):
                p0 = 64 * h + 32 * (m // 2)
                nc.sync.dma_start(dsttile[p0:p0 + 32, :, 3:67],
                                  inp["y4"][m, :, 8 * h:8 * h + 14, :])
        ym = pool.tile([128, 14, RW], F32)
        nc.vector.tensor_tensor(ym[:], ya[:], yb[:], AL.add)

        xt = pool.tile([128, 14, RW], F32)
        nc.vector.memset(xt[:], 0.0)
        for h in (0, 1):
            nc.sync.dma_start(xt[64 * h:64 * h + 64, :, 3:67], inp["xs"][:, 8 * h:8 * h + 14, :])

        # ---- ssm_ln, ln2, proj, relu
        z1 = ln_ch(ym, "sg_ssm", "sb_ssm", "ssm")
        z2 = ln_ch(z1, "sg_ln2", "sb_ln2", "ln2a")
        pr = pool.tile([128, 14, RW], F32)
        for ci, z_ap in enumerate(chunks2(z2)):
            pp = psT.tile([128, 7 * RW], F32, name=f"pp{ci}", tag="ps1")
            nc.tensor.matmul(pp[:], T["projT"][:], z_ap)
            nc.scalar.activation(pr[:, 7 * ci:7 * ci + 7, :],
                                 pp[:].rearrange("p (a b) -> p a b", a=7),
                                 AF.Relu, bias=T["projb"][:])

        # ---- bn-proj + pool partials, round 0
        prow = pr[:, OWN[0], OWN[1]]
        sqs = pool.tile([128, 8, 64], F32, name="sqs", tag="sqscratch")
        part0 = pool.tile([128, 3], F32)
        nc.vector.tensor_reduce(part0[:, 0:1], prow, mybir.AxisListType.XY, AL.add)
        nc.scalar.activation(sqs[:], prow, AF.Square, accum_out=part0[:, 1:2])
        nc.vector.tensor_copy(part0[:, 2:3], part0[:, 0:1])
        gat0 = allgather(0, part0[:], 3)
        # bn sums over all 8; pool sums over own-b cores
        red0 = pool.tile([128, 4], F32)
        nc.vector.tensor_reduce(red0[:, 0:2], gat0[:, 0:2, :], mybir.AxisListType.X, AL.add)
        pm = pool.tile([128, 16, NCORE], F32, name="pm", tag="pmx")
        nc.vector.tensor_tensor(pm[:, 2:3, :], gat0[:, 2:3, :],
                                bmask_t[:].unsqueeze(1), AL.mult)
        nc.vector.tensor_reduce(red0[:, 2:3], pm[:, 2:3, :], mybir.AxisListType.X, AL.add)
        stat0 = psS.tile([128, 4], F32, name="stat0", tag="lnst")
        nc.tensor.matmul(stat0[:, 0:3], T["selhp"][:], red0[:, 0:3])
        # S = g*rsqrt(v+eps), TT = b - m*S ; pool_bn = poolmean*S + TT
        mS = pool.tile([128, 6], F32)
        nc.vector.tensor_scalar(mS[:, 0:1], stat0[:, 0:1], 1.0 / NPIX, None, AL.mult)
        nc.vector.tensor_tensor(mS[:, 1:2], mS[:, 0:1], mS[:, 0:1], AL.mult)
        nc.vector.scalar_tensor_tensor(mS[:, 1:2], stat0[:, 1:2], 1.0 / NPIX, mS[:, 1:2],
                                       AL.mult, AL.subtract)
        nc.scalar.activation(mS[:, 1:2], mS[:, 1:2], AF.Ln, bias=eps_ap)
        nc.scalar.activation(mS[:, 1:2], mS[:, 1:2], AF.Exp, scale=-0.5)
        nc.vector.tensor_tensor(mS[:, 1:2], mS[:, 1:2], T["bnp_g"][:], AL.mult)  # S
        nc.vector.tensor_tensor(mS[:, 2:3], mS[:, 0:1], mS[:, 1:2], AL.mult)
        nc.vector.scalar_tensor_tensor(mS[:, 2:3], T["bnp_b"][:], 1.0, mS[:, 2:3],
                                       AL.mult, AL.subtract)                      # TT
        nc.vector.tensor_scalar(mS[:, 3:4], stat0[:, 2:3], 1.0 / NPB, None, AL.mult)
        nc.vector.tensor_tensor(mS[:, 3:4], mS[:, 3:4], mS[:, 1:2], AL.mult)
        nc.vector.tensor_tensor(mS[:, 3:4], mS[:, 3:4], mS[:, 2:3], AL.add)       # pool_bn
        # ---- eca
        ecp = psS.tile([128, 1], F32, name="ecp", tag="lnst")
        nc.tensor.matmul(ecp[:], T["ecaT"][:], mS[:, 3:4])
        sg = pool.tile([128, 2], F32)
        nc.scalar.activation(sg[:, 0:1], ecp[:], AF.Exp, scale=-1.0)
        nc.vector.tensor_scalar(sg[:, 0:1], sg[:, 0:1], 1.0, None, AL.add)
        nc.vector.reciprocal(sg[:, 1:2], sg[:, 0:1])
        # ---- x1 = sg * (pr*S + TT) + xt
        x1 = pool.tile([128, 14, RW], F32)
        nc.vector.tensor_scalar(x1[:], pr[:], mS[:, 1:2], mS[:, 2:3], AL.mult, AL.add)
        nc.vector.scalar_tensor_tensor(x1[:], x1[:], sg[:, 1:2], xt[:], AL.mult, AL.add)

        # ---- mlp
        m2 = ln_ch(x1, "sg_ln2", "sb_ln2", "ln2b")
        hm = []
        for t_i in range(4):
            hmt = pool.tile([128, 14, RW], F32, name=f"hm{t_i}")
            for ci, m_ap in enumerate(chunks2(m2)):
                fp = psT.tile([128, 7 * RW], F32, name=f"fp{t_i}{ci}", tag="ps1")
                nc.tensor.matmul(fp[:], T["fc1T"][:, 128 * t_i:128 * t_i + 128], m_ap)
                nc.scalar.activation(hmt[:, 7 * ci:7 * ci + 7, :],
                                     fp[:].rearrange("p (a b) -> p a b", a=7), AF.Relu)
            hm.append(hmt)
        # bn1 partials
        part1 = pool.tile([128, 8], F32)
        for t_i in range(4):
            nc.vector.tensor_reduce(part1[:, 2 * t_i:2 * t_i + 1], hm[t_i][:, OWN[0], OWN[1]],
                                    mybir.AxisListType.XY, AL.add)
            nc.scalar.activation(sqs[:], hm[t_i][:, OWN[0], OWN[1]], AF.Square,
                                 accum_out=part1[:, 2 * t_i + 1:2 * t_i + 2])
        gat1 = allgather(1, part1[:], 8)
        red1 = pool.tile([128, 8], F32)
        nc.vector.tensor_reduce(red1[:], gat1[:, 0:8, :], mybir.AxisListType.X, AL.add)
        stat1 = psS.tile([128, 8], F32, name="stat1", tag="lnst")
        nc.tensor.matmul(stat1[:], T["selhp"][:], red1[:])
        S1 = pool.tile([128, 4], F32)
        T1 = pool.tile([128, 4], F32)
        for t_i in range(4):
            a, bcol = stat1[:, 2 * t_i:2 * t_i + 1], stat1[:, 2 * t_i + 1:2 * t_i + 2]
            mcol = pool.tile([128, 2], F32, name=f"mcol{t_i}", tag="mcol")
            nc.vector.tensor_scalar(mcol[:, 0:1], a, 1.0 / NPIX, None, AL.mult)
            nc.vector.tensor_tensor(mcol[:, 1:2], mcol[:, 0:1], mcol[:, 0:1], AL.mult)
            nc.vector.scalar_tensor_tensor(mcol[:, 1:2], bcol, 1.0 / NPIX, mcol[:, 1:2],
                                           AL.mult, AL.subtract)
            nc.scalar.activation(mcol[:, 1:2], mcol[:, 1:2], AF.Ln, bias=eps_ap)
            nc.scalar.activation(mcol[:, 1:2], mcol[:, 1:2], AF.Exp, scale=-0.5)
            nc.vector.tensor_tensor(S1[:, t_i:t_i + 1], mcol[:, 1:2],
                                    T["bn1_g"][:, t_i:t_i + 1], AL.mult)
            nc.vector.tensor_tensor(mcol[:, 0:1], mcol[:, 0:1], S1[:, t_i:t_i + 1], AL.mult)
            nc.vector.scalar_tensor_tensor(T1[:, t_i:t_i + 1], T["bn1_b"][:, t_i:t_i + 1],
                                           1.0, mcol[:, 0:1], AL.mult, AL.subtract)
        # apply bn1 + mask (valid rows via rowm broadcast, zero the pad cols)
        for t_i in range(4):
            nc.vector.tensor_scalar(hm[t_i][:], hm[t_i][:], S1[:, t_i:t_i + 1],
                                    T1[:, t_i:t_i + 1], AL.mult, AL.add)
            nc.vector.tensor_tensor(hm[t_i][:, :, 3:67], hm[t_i][:, :, 3:67],
                                    rowm[:].unsqueeze(2).broadcast_to([128, 14, 64]),
                                    AL.mult)
            nc.vector.memset(hm[t_i][:, :, 0:3], 0.0)
            nc.vector.memset(hm[t_i][:, :, 67:70], 0.0)

        # ---- depthwise convs + residual (+bias), relu, bn2 partials
        KS = [1, 3, 5, 7]
        part2 = pool.tile([128, 8], F32)
        r2 = []
        for t_i, ks in enumerate(KS):
            pad = ks // 2
            taps = [(dy, dx) for dy in range(-pad, pad + 1) for dx in range(-pad, pad + 1)]
            acc0 = pool.tile([128, 8, 64], F32, name=f"dacc0_{t_i}", tag="dacc0")
            acc1 = pool.tile([128, 8, 64], F32, name=f"dacc1_{t_i}", tag="dacc1")
            accs = [acc0, acc1]
            cur = 0
            for ti2, (dy, dx) in enumerate(taps):
                src = hm[t_i][:, 3 + dy:11 + dy, 3 + dx:67 + dx]
                wap = T["dw_w"][:, t_i, ti2:ti2 + 1]
                if ti2 == 0:
                    nc.vector.scalar_tensor_tensor(accs[0][:], src, wap,
                                                   hm[t_i][:, OWN[0], OWN[1]], AL.mult, AL.add)
                else:
                    nc.vector.scalar_tensor_tensor(accs[1 - cur][:], src, wap, accs[cur][:],
                                             AL.mult, AL.add)
                    cur = 1 - cur
            r2t = pool.tile([128, 8, 64], F32, name=f"r2_{t_i}")
            nc.scalar.activation(r2t[:], accs[cur][:], AF.Relu,
                                 bias=T["dw_b"][:, t_i:t_i + 1],
                                 accum_out=part2[:, 2 * t_i:2 * t_i + 1])
            nc.scalar.activation(sqs[:], r2t[:], AF.Square,
                                 accum_out=part2[:, 2 * t_i + 1:2 * t_i + 2])
            r2.append(r2t)
        gat2 = allgather(2, part2[:], 8)
        red2 = pool.tile([128, 8], F32)
        nc.vector.tensor_reduce(red2[:], gat2[:, 0:8, :], mybir.AxisListType.X, AL.add)
        stat2 = psS.tile([128, 8], F32, name="stat2", tag="lnst")
        nc.tensor.matmul(stat2[:], T["selhp"][:], red2[:])
        S2 = pool.tile([128, 4], F32)
        T2 = pool.tile([128, 4], F32)
        for t_i in range(4):
            a, bcol = stat2[:, 2 * t_i:2 * t_i + 1], stat2[:, 2 * t_i + 1:2 * t_i + 2]
            mcol = pool.tile([128, 2], F32, name=f"m2col{t_i}", tag="mcol")
            nc.vector.tensor_scalar(mcol[:, 0:1], a, 1.0 / NPIX, None, AL.mult)
            nc.vector.tensor_tensor(mcol[:, 1:2], mcol[:, 0:1], mcol[:, 0:1], AL.mult)
            nc.vector.scalar_tensor_tensor(mcol[:, 1:2], bcol, 1.0 / NPIX, mcol[:, 1:2],
                                           AL.mult, AL.subtract)
            nc.scalar.activation(mcol[:, 1:2], mcol[:, 1:2], AF.Ln, bias=eps_ap)
            nc.scalar.activation(mcol[:, 1:2], mcol[:, 1:2], AF.Exp, scale=-0.5)
            nc.vector.tensor_tensor(S2[:, t_i:t_i + 1], mcol[:, 1:2],
                                    T["bn2_g"][:, t_i:t_i + 1], AL.mult)
            nc.vector.tensor_tensor(mcol[:, 0:1], mcol[:, 0:1], S2[:, t_i:t_i + 1], AL.mult)
            nc.vector.scalar_tensor_tensor(T2[:, t_i:t_i + 1], T["bn2_b"][:, t_i:t_i + 1],
                                           1.0, mcol[:, 0:1], AL.mult, AL.subtract)
        # ---- fc2 (accumulate over 4 input tiles) + bn3 + x1
        fo = psT.tile([128, 8, 64], F32, name="fo", tag="dwps")
        for t_i in range(4):
            zt = pool.tile([128, 8, 64], F32, name=f"zt{t_i}", tag="zt", bufs=2)
            nc.vector.tensor_scalar(zt[:], r2[t_i][:], S2[:, t_i:t_i + 1],
                                    T2[:, t_i:t_i + 1], AL.mult, AL.add)
            nc.tensor.matmul(fo[:], T["fc2T"][:, t_i, :], zt[:],
                             start=(t_i == 0), stop=(t_i == 3), skip_group_check=True)
        fo_sb = pool.tile([128, 8, 64], F32)
        part3 = pool.tile([128, 8], F32)
        nc.scalar.activation(fo_sb[:], fo[:], AF.Copy, accum_out=part3[:, 0:1])
        nc.scalar.activation(sqs[:], fo_sb[:], AF.Square, accum_out=part3[:, 1:2])
        gat3 = allgather(3, part3[:, 0:2], 2)
        red3 = pool.tile([128, 2], F32)
        nc.vector.tensor_reduce(red3[:], gat3[:, 0:2, :], mybir.AxisListType.X, AL.add)
        stat3 = psS.tile([128, 2], F32, name="stat3", tag="lnst")
        nc.tensor.matmul(stat3[:], T["selhp"][:], red3[:])
        mS3 = pool.tile([128, 3], F32)
        nc.vector.tensor_scalar(mS3[:, 0:1], stat3[:, 0:1], 1.0 / NPIX, None, AL.mult)
        nc.vector.tensor_tensor(mS3[:, 1:2], mS3[:, 0:1], mS3[:, 0:1], AL.mult)
        nc.vector.scalar_tensor_tensor(mS3[:, 1:2], stat3[:, 1:2], 1.0 / NPIX, mS3[:, 1:2],
                                       AL.mult, AL.subtract)
        nc.scalar.activation(mS3[:, 1:2], mS3[:, 1:2], AF.Ln, bias=eps_ap)
        nc.scalar.activation(mS3[:, 1:2], mS3[:, 1:2], AF.Exp, scale=-0.5)
        nc.vector.tensor_tensor(mS3[:, 1:2], mS3[:, 1:2], T["bn3_g"][:], AL.mult)
        nc.vector.tensor_tensor(mS3[:, 2:3], mS3[:, 0:1], mS3[:, 1:2], AL.mult)
        nc.vector.scalar_tensor_tensor(mS3[:, 2:3], T["bn3_b"][:], 1.0, mS3[:, 2:3],
                                       AL.mult, AL.subtract)
        fin = pool.tile([128, 8, 64], F32)
        nc.vector.tensor_scalar(fin[:], fo_sb[:], mS3[:, 1:2], mS3[:, 2:3], AL.mult, AL.add)
        nc.vector.tensor_tensor(fin[:], fin[:], x1[:, OWN[0], OWN[1]], AL.add)
        for h in (0, 1):
            nc.sync.dma_start(out[:, 8 * h:8 * h + 8, :], fin[64 * h:64 * h + 64])
    _fix_multiwaits(nc)
    return nc

def l3_consts(W_):
    selsum = np.zeros((128, 2), np.float32)
    selhp = np.zeros((128, 128), np.float32)
    sg_ssm = np.zeros((2, 128), np.float32); sb_ssm = np.zeros((128, 1), np.float32)
    sg_ln2 = np.zeros((2, 128), np.float32); sb_ln2 = np.zeros((128, 1), np.float32)
    projT = np.zeros((128, 128), np.float32); projb = np.zeros((128, 1), np.float32)
    bnp_g = np.zeros((128, 1), np.float32); bnp_b = np.zeros((128, 1), np.float32)
    ecaT = np.zeros((128, 128), np.float32)
    fc1T = np.zeros((128, 512), np.float32)
    bn1_g = np.zeros((128, 4), np.float32); bn1_b = np.zeros((128, 4), np.float32)
    dw_w = np.zeros((128, 4, 49), np.float32); dw_b = np.zeros((128, 4), np.float32)
    bn2_g = np.zeros((128, 4), np.float32); bn2_b = np.zeros((128, 4), np.float32)
    fc2T = np.zeros((128, 4, 128), np.float32)
    bn3_g = np.zeros((128, 1), np.float32); bn3_b = np.zeros((128, 1), np.float32)
    cst = np.zeros((128, 2), np.float32); cst[:, 0] = EPS
    dwk = [W_["dw_w1"], W_["dw_w3"], W_["dw_w5"], W_["dw_w7"]]
    dwb = [W_["dw_b1"], W_["dw_b3"], W_["dw_b5"], W_["dw_b7"]]
    for p in range(128):
        h, c = p // 64, p % 64
        selsum[p, h] = 1.0
        sg_ssm[h, p] = W_["ssm_ln_g"][c]; sb_ssm[p, 0] = W_["ssm_ln_b"][c]
        sg_ln2[h, p] = W_["ln2_g"][c]; sb_ln2[p, 0] = W_["ln2_b"][c]
        projb[p, 0] = W_["proj_b"][c]
        bnp_g[p, 0] = W_["proj_bn_g"][c]; bnp_b[p, 0] = W_["proj_bn_b"][c]
        bn3_g[p, 0] = W_["bn3_g"][c]; bn3_b[p, 0] = W_["bn3_b"][c]
        for t in range(4):
            bn1_g[p, t] = W_["bn1_g"][64 * t + c]; bn1_b[p, t] = W_["bn1_b"][64 * t + c]
            bn2_g[p, t] = W_["bn2_g"][64 * t + c]; bn2_b[p, t] = W_["bn2_b"][64 * t + c]
            ks = 2 * t + 1
            kern = dwk[t][c, 0]
            for ti2, (dy, dx) in enumerate([(a, bb) for a in range(-(ks // 2), ks // 2 + 1)
                                            for bb in range(-(ks // 2), ks // 2 + 1)]):
                dw_w[p, t, ti2] = kern[dy + ks // 2, dx + ks // 2]
            dw_b[p, t] = dwb[t][c]
        for p2 in range(128):
            h2, c2 = p2 // 64, p2 % 64
            if c2 == c:
                selhp[p, p2] = 1.0
            if h2 == h:
                projT[p, p2] = W_["proj_w"][c2, c, 0, 0]
                fc2T[p, :, p2] = [W_["fc2_w"][c2, 64 * t + c, 0, 0] for t in range(4)]
                for t in range(4):
                    fc1T[p, 128 * t + p2] = W_["fc1_w"][64 * t + c2, c, 0, 0]
            if h == 0 and abs(c2 - c) <= 1:
                ecaT[p, p2] = W_["eca_w"][c - c2 + 1]
    rowm_all = np.zeros((128, 4, 14), np.float32)
    for q in range(4):
        for p in range(128):
            h = p // 64
            for r in range(14):
                g = 16 * q - 3 + 8 * h + r
                if 0 <= g < H:
                    rowm_all[p, q, r] = 1.0
    bmask_all = np.zeros((128, 2, 8), np.float32)
    for b in range(2):
        bmask_all[:, b, 4 * b:4 * b + 4] = 1.0
    return dict(selsum=selsum, selhp=selhp, sg_ssm=sg_ssm, sb_ssm=sb_ssm,
                sg_ln2=sg_ln2, sb_ln2=sb_ln2, projT=projT, projb=projb,
                bnp_g=bnp_g, bnp_b=bnp_b, ecaT=ecaT, fc1T=fc1T,
                bn1_g=bn1_g, bn1_b=bn1_b, dw_w=dw_w, dw_b=dw_b,
                bn2_g=bn2_g, bn2_b=bn2_b, fc2T=fc2T, bn3_g=bn3_g, bn3_b=bn3_b,
                cst=cst, rowm_all=rowm_all.reshape(128, 56),
                bmask_all=bmask_all.reshape(128, 16))


def l3_inputs(y_by_core, x):
    """y_by_core: list of 8 arrays (32, 4096) from L2 (core -> (b=i//4, k=i%4))."""
    yimg = {}
    for b in range(B):
        for k in range(4):
            yk = y_by_core[4 * b + k]
            if k % 2 == 1:
                yk = yk[:, ::-1]
            inv = INV_R if k < 2 else INV_C
            yimg[(b, k)] = yk[:, inv].reshape(D2, H, W)
    maps = []
    for i in range(NCORE):
        b, q = i // 4, i % 4
        r0 = 16 * q
        lo, hi = r0 - 3, r0 + 19
        slo, shi = max(lo, 0), min(hi, H)
        y4 = np.zeros((4, D2, 22, W), np.float32)
        for k in range(4):
            y4[k, :, slo - lo:shi - lo, :] = yimg[(b, k)][:, slo:shi, :]
        xs = np.zeros((C, 22, W), np.float32)
        xs[:, slo - lo:shi - lo, :] = x[b, :, slo:shi, :]
        maps.append(dict(y4=y4, xs=xs, oh=core_oh(i)))
    return maps


def l3_gather(results):
    o = np.zeros((B, C, H, W), np.float32)
    for i in range(NCORE):
        b, q = i // 4, i % 4
        o[b, :, 16 * q:16 * q + 16, :] = results[i]["out"]
    return o

# ================================================================ FUSED
# One launch: L1 -> AllGather(h, batch group) -> snake -> L2 -> unsnake
# -> AllGather(y, batch group) -> L3. Inputs per core: xs (22-row halo
# window of x) + oh. All weights are NEFF consts.

L3_CSHAPES = [("rowm_all", [128, 56]), ("bmask_all", [128, 16]),
              ("selsum", [128, 2]), ("selhp", [128, 128]),
              ("sg_ssm", [2, 128]), ("sb_ssm", [128, 1]),
              ("sg_ln2", [2, 128]), ("sb_ln2", [128, 1]),
              ("projT", [128, 128]), ("projb", [128, 1]),
              ("bnp_g", [128, 1]), ("bnp_b", [128, 1]),
              ("ecaT", [128, 128]),
              ("fc1T", [128, 512]),
              ("bn1_g", [128, 4]), ("bn1_b", [128, 4]),
              ("dw_w", [128, 4, 49]), ("dw_b", [128, 4]),
              ("bn2_g", [128, 4]), ("bn2_b", [128, 4]),
              ("fc2T", [128, 4, 128]),
              ("bn3_g", [128, 1]), ("bn3_b", [128, 1]),
              ("cst", [128, 2])]


BF16 = mybir.dt.bfloat16


def build_fused(cw1, cw2, cw3):
    nc = _new_bass()
    xs = nc.dram_tensor("xs", [C, 22, W], BF16, kind="ExternalInput")
    oh = nc.dram_tensor("oh", [128, 8], F32, kind="ExternalInput")
    out = nc.dram_tensor("out", [C, 16, W], BF16, kind="ExternalOutput")
    C1 = {nm: _const(nc, f"c1_{nm}", v) for nm, v in cw1.items()}
    C2 = {nm: _const(nc, f"c2_{nm}", v) for nm, v in cw2.items()}
    C3 = {nm: _const(nc, f"c3_{nm}", cw3[nm].reshape(shp)) for nm, shp in L3_CSHAPES}
    hsd = nc.dram_tensor("hsd", [C, 16, W], F32)
    hgat = nc.dram_tensor("hgat", [4 * C, 16, W], F32)
    yod = nc.dram_tensor("yod", [D2, L], F32)
    ygat = nc.dram_tensor("ygat", [4 * D2, L], F32)
    cc_in = [nc.dram_tensor(f"ccin{r}", [128, 16], F32) for r in range(4)]
    cc_out = [nc.dram_tensor(f"ccout{r}", [NCORE * 128, 16], F32) for r in range(4)]
    GRP4 = [[0, 1, 2, 3], [4, 5, 6, 7]]

    with tile.TileContext(nc, linearize=LINEARIZE) as tc, ExitStack() as ctx:
        pp = ctx.enter_context(tc.tile_pool(name="pp", bufs=1))
        oh_t = pp.tile([128, 8], F32)
        nc.sync.dma_start(oh_t[:], oh[:])
        xsb = pp.tile([128, 22, W], BF16, name="xsb")
        for h in (0, 1):
            nc.sync.dma_start(xsb[64 * h:64 * h + 64, :, :], xs[:])
        ut = pp.tile([D2, L], F32, name="ut")

        # ---------------- P1: ln1 + depthwise 3x3 dil-2 conv ----------------
        with tc.tile_pool(name="p1", bufs=1) as pool, \
             tc.tile_pool(name="ps1", bufs=1, space="PSUM") as psum:
            xt = pool.tile([128, 12, 68], F32)
            nc.vector.memset(xt[:], 0.0)
            for h in (0, 1):
                nc.vector.tensor_copy(xt[64 * h:64 * h + 64, :, 2:66],
                                      xsb[64 * h:64 * h + 64, 1 + 8 * h:13 + 8 * h, :])
            selsum_t = pool.tile([128, 2], F32)
            selg_t = pool.tile([2, 128], F32)
            b2_t = pool.tile([128, 1], F32)
            w9_t = pool.tile([128, 9], F32)
            cb_t = pool.tile([128, 1], F32)
            mask_all_t = pool.tile([128, 48], F32)
            cst_t = pool.tile([2, 2], F32)
            nc.sync.dma_start(selsum_t[:], C1["selsum"][:])
            nc.sync.dma_start(selg_t[:], C1["selg"][:])
            nc.sync.dma_start(b2_t[:], C1["b2"][:])
            nc.sync.dma_start(w9_t[:], C1["w9"][:])
            nc.sync.dma_start(cb_t[:], C1["cb"][:])
            nc.sync.dma_start(mask_all_t[:], C1["mask_all"][:])
            nc.sync.dma_start(cst_t[:], C1["cst"][:])
            mask_t = ksel(nc, pool, "mask", mask_all_t[:], 128, 12, 4, oh_t[:])

            XW = xt[:, :, 2:66]
            sq = pool.tile([128, 12, 64], F32)
            nc.scalar.activation(sq[:], XW, AF.Square)
            st_x = psum.tile([2, 768], F32)
            st_xx = psum.tile([2, 768], F32)
            for r0, r1 in ((0, 8), (8, 12)):
                nc.tensor.matmul(st_x[:, r0 * 64:r1 * 64], selsum_t[:], xt[:, r0:r1, 2:66])
                nc.tensor.matmul(st_xx[:, r0 * 64:r1 * 64], selsum_t[:], sq[:, r0:r1, :])
            sm = pool.tile([2, 768], F32)
            nc.vector.tensor_scalar(sm[:], st_x[:], 1.0 / 64, None, AL.mult)
            var = pool.tile([2, 768], F32)
            nc.vector.tensor_tensor(var[:], sm[:], sm[:], AL.mult)
            nc.vector.scalar_tensor_tensor(var[:], st_xx[:], 1.0 / 64, var[:], AL.mult, AL.subtract)
            inv = pool.tile([2, 768], F32)
            nc.scalar.activation(inv[:], var[:], AF.Ln, bias=cst_t[:, 0:1])
            nc.scalar.activation(inv[:], inv[:], AF.Exp, scale=-0.5)
            minv = pool.tile([2, 768], F32)
            nc.vector.tensor_tensor(minv[:], sm[:], inv[:], AL.mult)
            sgb = psum.tile([128, 12, 64], F32)
            msgb = psum.tile([128, 12, 64], F32)
            for r0, r1 in ((0, 8), (8, 12)):
                nc.tensor.matmul(sgb[:, r0:r1, :], selg_t[:], inv[:, r0 * 64:r1 * 64])
                nc.tensor.matmul(msgb[:, r0:r1, :], selg_t[:], minv[:, r0 * 64:r1 * 64])
            xn = pool.tile([128, 12, 64], F32)
            nc.vector.tensor_tensor(xn[:], XW, sgb[:], AL.mult)
            nc.vector.scalar_tensor_tensor(xn[:], xn[:], b2_t[:], msgb[:], AL.add, AL.subtract)
            xmp = pool.tile([128, 12, 68], F32)
            nc.vector.memset(xmp[:], 0.0)
            nc.vector.tensor_tensor(xmp[:, :, 2:66], xn[:],
                                    mask_t[:].unsqueeze(2).broadcast_to([128, 12, 64]), AL.mult)
            acc0 = pool.tile([128, 8, 64], F32)
            acc1 = pool.tile([128, 8, 64], F32)
            acc = [acc0, acc1]
            taps = [(dy, dx) for dy in (-2, 0, 2) for dx in (-2, 0, 2)]
            cur = 0
            for ti, (dy, dx) in enumerate(taps):
                src = xmp[:, 2 + dy:10 + dy, 2 + dx:66 + dx]
                if ti == 0:
                    nc.vector.tensor_scalar(acc[0][:], src, w9_t[:, 0:1], cb_t[:], AL.mult, AL.add)
                else:
                    nc.vector.scalar_tensor_tensor(acc[1 - cur][:], src, w9_t[:, ti:ti + 1],
                                                   acc[cur][:], AL.mult, AL.add)
                    cur = 1 - cur
            for h in (0, 1):
                nc.sync.dma_start(hsd[:, 8 * h:8 * h + 8, :], acc[cur][64 * h:64 * h + 64])

        # ---------------- G1: gather h within batch group ----------------
        nc.gpsimd.collective_compute("AllGather", AL.bypass, replica_groups=GRP4,
                                     ins=[hsd[:]], outs=[hgat[:]])

        # ---------------- P2a: build snake-ordered u for own direction ----
        with tc.tile_pool(name="p2a", bufs=1) as pool:
            himg = pool.tile([C, H, W], F32)
            for j in range(4):
                nc.sync.dma_start(himg[:, 16 * j:16 * j + 16, :],
                                  hgat[64 * j:64 * j + 64, :, :])
            hB = pool.tile([D2, H, W], F32)      # second channel half -> parts 0:32
            nc.sync.dma_start(hB[:], himg[32:64, :, :])
            u0 = pool.tile([D2, H, W], F32)
            nc.vector.tensor_copy(u0[:, 0::2, :], himg[0:32, 0::2, :])
            nc.vector.tensor_copy(u0[:, 1::2, :], himg[0:32, 1::2, ::-1])
            hBT = pool.tile([D2, W, H], F32)
            nc.vector.tensor_copy(hBT[:], hB[:].transpose([0, 2, 1]))
            u2 = pool.tile([D2, W, H], F32)
            nc.vector.tensor_copy(u2[:, 0::2, :], hBT[:, 0::2, :])
            nc.vector.tensor_copy(u2[:, 1::2, :], hBT[:, 1::2, ::-1])
            sa = pool.tile([D2, H, W], F32)
            sb = pool.tile([D2, H, W], F32)
            nc.vector.tensor_scalar(sa[:], u0[:], oh_t[0:D2, 0:1], None, AL.mult)
            nc.vector.scalar_tensor_tensor(sb[:], u0[:, ::-1, ::-1], oh_t[0:D2, 1:2],
                                           sa[:], AL.mult, AL.add)
            nc.vector.scalar_tensor_tensor(sa[:], u2[:], oh_t[0:D2, 2:3],
                                           sb[:], AL.mult, AL.add)
            nc.vector.scalar_tensor_tensor(ut[:].rearrange("p (a b) -> p a b", a=H),
                                           u2[:, ::-1, ::-1], oh_t[0:D2, 3:4],
                                           sa[:], AL.mult, AL.add)

        # ---------------- P2b: selective scan (direction-selected weights) -
        NCH = 8
        CH = L // NCH
        with tc.tile_pool(name="p2b", bufs=1) as pool, \
             tc.tile_pool(name="psA", bufs=3, space="PSUM") as psA, \
             tc.tile_pool(name="psY", bufs=1, space="PSUM") as psY:
            lhsT_u4_t = pool.tile([D2, 128], F32)
            lhsT_y_t = pool.tile([128, D2], F32)
            nc.sync.dma_start(lhsT_u4_t[:], C2["u4"][:])
            nc.sync.dma_start(lhsT_y_t[:], C2["y_lhsT"][:])

            def ksel_dram(nm, hd, P_, F_, fc=None):
                outt = pool.tile([P_, F_], F32, name=f"sel_{nm}")
                fc = fc or F_
                for o in range(0, F_, fc):
                    st = pool.tile([P_, 4 * fc], F32, name=f"st_{nm}_{o}",
                                   tag=f"kst_{P_}x{4 * fc}", bufs=2)
                    for k in range(4):
                        nc.sync.dma_start(st[:, k * fc:(k + 1) * fc],
                                          hd[:, k * F_ + o:k * F_ + o + fc])
                    cur2 = None
                    for k in range(4):
                        if k == 3:
                            dst = outt[:, o:o + fc]
                        else:
                            dst = pool.tile([P_, fc], F32, name=f"ks_{nm}_{o}_{k}",
                                            tag=f"kscr_{P_}x{fc}", bufs=2)[:]
                        if k == 0:
                            nc.vector.tensor_scalar(dst, st[:, 0:fc], oh_t[0:P_, 0:1],
                                                    None, AL.mult)
                        else:
                            nc.vector.scalar_tensor_tensor(dst, st[:, k * fc:(k + 1) * fc],
                                                           oh_t[0:P_, k:k + 1], cur2,
                                                           AL.mult, AL.add)
                        cur2 = dst
                return outt

            lhsT_d_t = ksel_dram("d", C2["d_all"], D2, 128)
            bias4_t = ksel_dram("bias", C2["bias_all"], 128, 1)
            lhsT_B_t = ksel_dram("B", C2["B_all"], D2, 512, 128)
            lhsT_C_t = ksel_dram("C", C2["C_all"], D2, 512, 128)
            A4_t = ksel_dram("A", C2["A_all"], 128, 4)
            lhsT_Ds_t = ksel_dram("Ds", C2["Ds_all"], D2, D2)

            def mm(out_ap, lh, rh, f32r, **kw):
                if f32r:
                    lh, rh = lh.bitcast(F32R), rh.bitcast(F32R)
                nc.tensor.matmul(out_ap, lh, rh, **kw)

            d4 = pool.tile([128, L], F32)
            w4 = pool.tile([128, L], F32)
            for c in range(NCH):
                dp = psA.tile([128, CH], F32, name=f"dp{c}", tag="ps")
                mm(dp[:], lhsT_d_t[:], ut[:, c * CH:(c + 1) * CH], F32R_PROJ)
                nc.scalar.activation(d4[:, c * CH:(c + 1) * CH], dp[:], AF.Exp,
                                     bias=bias4_t[:])
                nc.scalar.activation(d4[:, c * CH:(c + 1) * CH],
                                     d4[:, c * CH:(c + 1) * CH], AF.Ln,
                                     bias=nc.const_aps.tensor(1.0, (128, 1)))
            for c in range(NCH):
                u4p = psA.tile([128, CH], F32, name=f"u4p{c}", tag="ps")
                mm(u4p[:], lhsT_u4_t[:], ut[:, c * CH:(c + 1) * CH], F32R_BC)
                nc.vector.tensor_tensor(w4[:, c * CH:(c + 1) * CH],
                                        d4[:, c * CH:(c + 1) * CH], u4p[:], AL.mult)
            hs = []
            for g in range(4):
                dBu = pool.tile([128, L], F32, name=f"dBu{g}", tag="dBu", bufs=1)
                for c in range(NCH):
                    b4 = psA.tile([128, CH], F32, name=f"b4_{g}_{c}", tag="ps")
                    mm(b4[:], lhsT_B_t[:, g * 128:(g + 1) * 128],
                       ut[:, c * CH:(c + 1) * CH], F32R_BC)
                    nc.vector.tensor_tensor(dBu[:, c * CH:(c + 1) * CH],
                                            w4[:, c * CH:(c + 1) * CH], b4[:], AL.mult)
                dA = pool.tile([128, L], F32, name=f"dA{g}", tag="dA", bufs=2)
                nc.scalar.activation(dA[:], d4[:], AF.Exp, scale=A4_t[:, g:g + 1])
                hsg = pool.tile([128, L], F32, name=f"hs{g}")
                nc.vector.tensor_tensor_scan(hsg[:], dA[:], dBu[:], 0.0, AL.mult, AL.add)
                hs.append(hsg)
            for half in range(2):
                yps = psY.tile([D2, L // 2], F32, name=f"yps{half}", tag="yps")
                for g in range(4):
                    fsb = pool.tile([128, L // 2], F32, name=f"f_{half}_{g}", tag="fsb", bufs=2)
                    for cc in range(NCH // 2):
                        c = half * (NCH // 2) + cc
                        c4 = psA.tile([128, CH], F32, name=f"c4_{g}_{c}", tag="ps")
                        mm(c4[:], lhsT_C_t[:, g * 128:(g + 1) * 128],
                           ut[:, c * CH:(c + 1) * CH], F32R_BC)
                        if g % 2 == 0:
                            nc.vector.tensor_tensor(fsb[:, cc * CH:(cc + 1) * CH],
                                                    hs[g][:, c * CH:(c + 1) * CH], c4[:], AL.mult)
                        else:
                            c4sb = pool.tile([128, CH], F32, name=f"c4sb_{g}_{c}", tag="c4sb", bufs=2)
                            nc.scalar.copy(c4sb[:], c4[:])
                            nc.vector.tensor_tensor(fsb[:, cc * CH:(cc + 1) * CH],
                                                    hs[g][:, c * CH:(c + 1) * CH], c4sb[:], AL.mult)
                    for cc in range(NCH // 2):
                        mm(yps[:, cc * CH:(cc + 1) * CH], lhsT_y_t[:],
                           fsb[:, cc * CH:(cc + 1) * CH], F32R_Y,
                           start=(g == 0), stop=False, skip_group_check=True)
                for cc in range(NCH // 2):
                    c = half * (NCH // 2) + cc
                    mm(yps[:, cc * CH:(cc + 1) * CH], lhsT_Ds_t[:],
                       ut[:, c * CH:(c + 1) * CH], F32R_Y,
                       start=False, stop=True, skip_group_check=True)
                ysb = pool.tile([D2, L // 2], F32, name=f"ysb{half}", tag="ysb", bufs=1)
                nc.scalar.copy(ysb[:], yps[:])
                nc.sync.dma_start(yod[:, half * (L // 2):(half + 1) * (L // 2)], ysb[:])

        # ---------------- G2: gather y within batch group ----------------
        nc.gpsimd.collective_compute("AllGather", AL.bypass, replica_groups=GRP4,
                                     ins=[yod[:]], outs=[ygat[:]])

        # ---------------- P3: merge + LN + proj + eca + MLP ---------------
        RW = 70
        OWN = (slice(3, 11), slice(3, 67))
        with tc.tile_pool(name="p3", bufs=1) as pool, \
             tc.tile_pool(name="psT", bufs=2, space="PSUM") as psT, \
             tc.tile_pool(name="psS", bufs=2, space="PSUM") as psS:
            T = {}
            for nm, shp in L3_CSHAPES:
                T[nm] = pool.tile(shp, F32, name=f"t_{nm}")
                nc.sync.dma_start(T[nm][:], C3[nm][:])
            eps_ap = T["cst"][:, 0:1]
            rowm = ksel(nc, pool, "rowm", T["rowm_all"][:], 128, 14, 4, oh_t[:])
            bmask_t = ksel(nc, pool, "bmask", T["bmask_all"][:], 128, 8, 2, oh_t[:, 4:6])

            # unsnake the 4 gathered direction outputs into padded images
            ygi = pool.tile([128, 70, W], F32)
            nc.vector.memset(ygi[:], 0.0)
            with tc.tile_pool(name="p3u", bufs=1) as pu:
                ysn = pu.tile([128, H, W], F32)
                nc.sync.dma_start(ysn[:], ygat[:])
                wsc = pu.tile([128, W, H], F32)
                # k=0 rows even/odd
                nc.vector.tensor_copy(ygi[0:32, 3:66:2, :], ysn[0:32, 0::2, :])
                nc.vector.tensor_copy(ygi[0:32, 4:67:2, :], ysn[0:32, 1::2, ::-1])
                # k=1: reverse whole seq then row-unsnake
                y1v = ysn[32:64, ::-1, ::-1]
                nc.vector.tensor_copy(ygi[32:64, 3:66:2, :], y1v[:, 0::2, :])
                nc.vector.tensor_copy(ygi[32:64, 4:67:2, :], y1v[:, 1::2, ::-1])
                # k=2: col-grid row-unsnake then transpose
                nc.vector.tensor_copy(wsc[64:96, 0::2, :], ysn[64:96, 0::2, :])
                nc.vector.tensor_copy(wsc[64:96, 1::2, :], ysn[64:96, 1::2, ::-1])
                nc.vector.tensor_copy(ygi[64:96, 3:67, :], wsc[64:96].transpose([0, 2, 1]))
                # k=3: reversed col-grid
                y3v = ysn[96:128, ::-1, ::-1]
                nc.vector.tensor_copy(wsc[96:128, 0::2, :], y3v[:, 0::2, :])
                nc.vector.tensor_copy(wsc[96:128, 1::2, :], y3v[:, 1::2, ::-1])
                nc.vector.tensor_copy(ygi[96:128, 3:67, :], wsc[96:128].transpose([0, 2, 1]))

            # q-variant window select -> ym
            ym = pool.tile([128, 14, RW], F32, name="ym")
            with tc.tile_pool(name="p3v", bufs=1) as pv:
                scur = None
                for qv in range(4):
                    ya = pv.tile([128, 14, RW], F32, name=f"ya{qv}", tag="yaq", bufs=2)
                    yb = pv.tile([128, 14, RW], F32, name=f"yb{qv}", tag="ybq", bufs=2)
                    nc.vector.memset(ya[:], 0.0)
                    nc.vector.memset(yb[:], 0.0)
                    for h in (0, 1):
                        for m, dsttile in ((0, ya), (1, yb), (2, ya), (3, yb)):
                            p0 = 64 * h + 32 * (m // 2)
                            r0 = 16 * qv + 8 * h
                            nc.sync.dma_start(dsttile[p0:p0 + 32, :, 3:67],
                                              ygi[32 * m:32 * m + 32, r0:r0 + 14, :])
                    ymq = pv.tile([128, 14, RW], F32, name=f"ymq{qv}", tag="ymq", bufs=2)
                    nc.vector.tensor_tensor(ymq[:], ya[:], yb[:], AL.add)
                    if qv == 0:
                        s0 = pv.tile([128, 14, RW], F32, name="ysel0", tag="ysel", bufs=2)
                        nc.vector.tensor_scalar(s0[:], ymq[:], oh_t[:, 0:1], None, AL.mult)
                        scur = s0
                    else:
                        dst = ym if qv == 3 else pv.tile([128, 14, RW], F32,
                                                         name=f"ysel{qv}", tag="ysel", bufs=2)
                        nc.vector.scalar_tensor_tensor(dst[:], ymq[:], oh_t[:, qv:qv + 1],
                                                       scur[:], AL.mult, AL.add)
                        scur = dst

            xt = pool.tile([128, 14, RW], F32)
            nc.vector.memset(xt[:], 0.0)
            for h in (0, 1):
                nc.vector.tensor_copy(xt[64 * h:64 * h + 64, :, 3:67],
                                      xsb[64 * h:64 * h + 64, 8 * h:8 * h + 14, :])

            def chunks2(tile3, rows=14):
                return [tile3[:, 0:7, :], tile3[:, 7:14, :]]

            def ln_ch(src, selg_key, b_key, nm):
                sq = pool.tile([128, 14, RW], F32, name=f"sq_{nm}", tag="lnsq")
                nc.scalar.activation(sq[:], src[:], AF.Square)
                inv = pool.tile([2, 14, RW], F32, name=f"inv_{nm}", tag="lninv")
                minv = pool.tile([2, 14, RW], F32, name=f"minv_{nm}", tag="lnminv")
                for ci, (s_ap, q_ap) in enumerate(zip(chunks2(src), chunks2(sq))):
                    px = psS.tile([2, 7 * RW], F32, name=f"px_{nm}{ci}", tag="lnst")
                    pq = psS.tile([2, 7 * RW], F32, name=f"pq_{nm}{ci}", tag="lnst")
                    nc.tensor.matmul(px[:], T["selsum"][:], s_ap)
                    nc.tensor.matmul(pq[:], T["selsum"][:], q_ap)
                    ivc = inv[:, 7 * ci:7 * ci + 7, :]
                    mvc = minv[:, 7 * ci:7 * ci + 7, :]
                    sm = pool.tile([2, 7, RW], F32, name=f"sm_{nm}{ci}", tag="lnsm")
                    nc.vector.tensor_scalar(sm[:], px[:], 1.0 / 64, None, AL.mult)
                    nc.vector.tensor_tensor(ivc, sm[:], sm[:], AL.mult)
                    nc.vector.scalar_tensor_tensor(ivc, pq[:], 1.0 / 64, ivc, AL.mult, AL.subtract)
                    nc.scalar.activation(ivc, ivc, AF.Ln, bias=T["cst"][0:2, 0:1])
                    nc.scalar.activation(ivc, ivc, AF.Exp, scale=-0.5)
                    nc.vector.tensor_tensor(mvc, sm[:], ivc, AL.mult)
                dst = pool.tile([128, 14, RW], F32, name=f"ln_{nm}")
                for ci in range(2):
                    rs = slice(7 * ci, 7 * ci + 7)
                    sgb = psS.tile([128, 7 * RW], F32, name=f"sgb_{nm}{ci}", tag="lnbc")
                    msgb = psS.tile([128, 7 * RW], F32, name=f"msgb_{nm}{ci}", tag="lnbc")
                    nc.tensor.matmul(sgb[:], T[selg_key][:], inv[:, rs, :])
                    nc.tensor.matmul(msgb[:], T[selg_key][:], minv[:, rs, :])
                    nc.vector.tensor_tensor(dst[:, rs, :], src[:, rs, :],
                                            sgb[:].rearrange("p (a b) -> p a b", a=7), AL.mult)
                    nc.vector.scalar_tensor_tensor(dst[:, rs, :], dst[:, rs, :], T[b_key][:],
                                                   msgb[:].rearrange("p (a b) -> p a b", a=7),
                                                   AL.add, AL.subtract)
                return dst

            def allgather(rnd, cols_src_ap, ncols):
                ci = pool.tile([128, 16], F32, name=f"cci_{rnd}", tag="cci")
                nc.vector.memset(ci[:], 0.0)
                nc.vector.tensor_copy(ci[:, 0:ncols], cols_src_ap)
                nc.sync.dma_start(cc_in[rnd][:], ci[:])
                nc.gpsimd.collective_compute(
                    "AllGather", AL.bypass, replica_groups=[list(range(NCORE))],
                    ins=[cc_in[rnd][:]], outs=[cc_out[rnd][:]])
                gat = pool.tile([128, 16, NCORE], F32, name=f"gat_{rnd}", tag="gat")
                src = cc_out[rnd][:].rearrange("(n p) c -> p c n", p=128)
                nc.sync.dma_start(gat[:, 0:16, :], src)
                return gat

            z1 = ln_ch(ym, "sg_ssm", "sb_ssm", "ssm")
            z2 = ln_ch(z1, "sg_ln2", "sb_ln2", "ln2a")
            pr = pool.tile([128, 14, RW], F32)
            for ci, z_ap in enumerate(chunks2(z2)):
                pp2 = psT.tile([128, 7 * RW], F32, name=f"pp{ci}", tag="ps1")
                nc.tensor.matmul(pp2[:], T["projT"][:], z_ap)
                nc.scalar.activation(pr[:, 7 * ci:7 * ci + 7, :],
                                     pp2[:].rearrange("p (a b) -> p a b", a=7),
                                     AF.Relu, bias=T["projb"][:])
            prow = pr[:, OWN[0], OWN[1]]
            sqs = pool.tile([128, 8, 64], F32, name="sqs", tag="sqscratch")
            part0 = pool.tile([128, 3], F32)
            nc.vector.tensor_reduce(part0[:, 0:1], prow, mybir.AxisListType.XY, AL.add)
            nc.scalar.activation(sqs[:], prow, AF.Square, accum_out=part0[:, 1:2])
            nc.vector.tensor_copy(part0[:, 2:3], part0[:, 0:1])
            gat0 = allgather(0, part0[:], 3)
            red0 = pool.tile([128, 4], F32)
            nc.vector.tensor_reduce(red0[:, 0:2], gat0[:, 0:2, :], mybir.AxisListType.X, AL.add)
            pm = pool.tile([128, 16, NCORE], F32, name="pm", tag="pmx")
            nc.vector.tensor_tensor(pm[:, 2:3, :], gat0[:, 2:3, :],
                                    bmask_t[:].unsqueeze(1), AL.mult)
            nc.vector.tensor_reduce(red0[:, 2:3], pm[:, 2:3, :], mybir.AxisListType.X, AL.add)
            stat0 = psS.tile([128, 4], F32, name="stat0", tag="lnst")
            nc.tensor.matmul(stat0[:, 0:3], T["selhp"][:], red0[:, 0:3])
            mS = pool.tile([128, 6], F32)
            nc.vector.tensor_scalar(mS[:, 0:1], stat0[:, 0:1], 1.0 / NPIX, None, AL.mult)
            nc.vector.tensor_tensor(mS[:, 1:2], mS[:, 0:1], mS[:, 0:1], AL.mult)
            nc.vector.scalar_tensor_tensor(mS[:, 1:2], stat0[:, 1:2], 1.0 / NPIX, mS[:, 1:2],
                                           AL.mult, AL.subtract)
            nc.scalar.activation(mS[:, 1:2], mS[:, 1:2], AF.Ln, bias=eps_ap)
            nc.scalar.activation(mS[:, 1:2], mS[:, 1:2], AF.Exp, scale=-0.5)
            nc.vector.tensor_tensor(mS[:, 1:2], mS[:, 1:2], T["bnp_g"][:], AL.mult)
            nc.vector.tensor_tensor(mS[:, 2:3], mS[:, 0:1], mS[:, 1:2], AL.mult)
            nc.vector.scalar_tensor_tensor(mS[:, 2:3], T["bnp_b"][:], 1.0, mS[:, 2:3],
                                           AL.mult, AL.subtract)
            nc.vector.tensor_scalar(mS[:, 3:4], stat0[:, 2:3], 1.0 / NPB, None, AL.mult)
            nc.vector.tensor_tensor(mS[:, 3:4], mS[:, 3:4], mS[:, 1:2], AL.mult)
            nc.vector.tensor_tensor(mS[:, 3:4], mS[:, 3:4], mS[:, 2:3], AL.add)
            ecp = psS.tile([128, 1], F32, name="ecp", tag="lnst")
            nc.tensor.matmul(ecp[:], T["ecaT"][:], mS[:, 3:4])
            sg = pool.tile([128, 2], F32)
            nc.scalar.activation(sg[:, 0:1], ecp[:], AF.Exp, scale=-1.0)
            nc.vector.tensor_scalar(sg[:, 0:1], sg[:, 0:1], 1.0, None, AL.add)
            nc.vector.reciprocal(sg[:, 1:2], sg[:, 0:1])
            x1 = pool.tile([128, 14, RW], F32)
            nc.vector.tensor_scalar(x1[:], pr[:], mS[:, 1:2], mS[:, 2:3], AL.mult, AL.add)
            nc.vector.scalar_tensor_tensor(x1[:], x1[:], sg[:, 1:2], xt[:], AL.mult, AL.add)

            m2 = ln_ch(x1, "sg_ln2", "sb_ln2", "ln2b")
            hm = []
            for t_i in range(4):
                hmt = pool.tile([128, 14, RW], F32, name=f"hm{t_i}")
                for ci, m_ap in enumerate(chunks2(m2)):
                    fp = psT.tile([128, 7 * RW], F32, name=f"fp{t_i}{ci}", tag="ps1")
                    nc.tensor.matmul(fp[:], T["fc1T"][:, 128 * t_i:128 * t_i + 128], m_ap)
                    nc.scalar.activation(hmt[:, 7 * ci:7 * ci + 7, :],
                                         fp[:].rearrange("p (a b) -> p a b", a=7), AF.Relu)
                hm.append(hmt)
            part1 = pool.tile([128, 8], F32)
            for t_i in range(4):
                nc.vector.tensor_reduce(part1[:, 2 * t_i:2 * t_i + 1], hm[t_i][:, OWN[0], OWN[1]],
                                        mybir.AxisListType.XY, AL.add)
                nc.scalar.activation(sqs[:], hm[t_i][:, OWN[0], OWN[1]], AF.Square,
                                     accum_out=part1[:, 2 * t_i + 1:2 * t_i + 2])
            gat1 = allgather(1, part1[:], 8)
            red1 = pool.tile([128, 8], F32)
            nc.vector.tensor_reduce(red1[:], gat1[:, 0:8, :], mybir.AxisListType.X, AL.add)
            stat1 = psS.tile([128, 8], F32, name="stat1", tag="lnst")
            nc.tensor.matmul(stat1[:], T["selhp"][:], red1[:])
            S1 = pool.tile([128, 4], F32)
            T1 = pool.tile([128, 4], F32)
            for t_i in range(4):
                a, bcol = stat1[:, 2 * t_i:2 * t_i + 1], stat1[:, 2 * t_i + 1:2 * t_i + 2]
                mcol = pool.tile([128, 2], F32, name=f"mcol{t_i}", tag="mcol")
                nc.vector.tensor_scalar(mcol[:, 0:1], a, 1.0 / NPIX, None, AL.mult)
                nc.vector.tensor_tensor(mcol[:, 1:2], mcol[:, 0:1], mcol[:, 0:1], AL.mult)
                nc.vector.scalar_tensor_tensor(mcol[:, 1:2], bcol, 1.0 / NPIX, mcol[:, 1:2],
                                               AL.mult, AL.subtract)
                nc.scalar.activation(mcol[:, 1:2], mcol[:, 1:2], AF.Ln, bias=eps_ap)
                nc.scalar.activation(mcol[:, 1:2], mcol[:, 1:2], AF.Exp, scale=-0.5)
                nc.vector.tensor_tensor(S1[:, t_i:t_i + 1], mcol[:, 1:2],
                                        T["bn1_g"][:, t_i:t_i + 1], AL.mult)
                nc.vector.tensor_tensor(mcol[:, 0:1], mcol[:, 0:1], S1[:, t_i:t_i + 1], AL.mult)
                nc.vector.scalar_tensor_tensor(T1[:, t_i:t_i + 1], T["bn1_b"][:, t_i:t_i + 1],
                                               1.0, mcol[:, 0:1], AL.mult, AL.subtract)
            for t_i in range(4):
                nc.vector.tensor_scalar(hm[t_i][:], hm[t_i][:], S1[:, t_i:t_i + 1],
                                        T1[:, t_i:t_i + 1], AL.mult, AL.add)
                nc.vector.tensor_tensor(hm[t_i][:, :, 3:67], hm[t_i][:, :, 3:67],
                                        rowm[:].unsqueeze(2).broadcast_to([128, 14, 64]),
                                        AL.mult)
                nc.vector.memset(hm[t_i][:, :, 0:3], 0.0)
                nc.vector.memset(hm[t_i][:, :, 67:70], 0.0)

            KS = [1, 3, 5, 7]
            part2 = pool.tile([128, 8], F32)
            r2 = []
            for t_i, ks in enumerate(KS):
                pad = ks // 2
                taps = [(dy, dx) for dy in range(-pad, pad + 1) for dx in range(-pad, pad + 1)]
                acc0 = pool.tile([128, 8, 64], F32, name=f"dacc0_{t_i}", tag="dacc0")
                acc1 = pool.tile([128, 8, 64], F32, name=f"dacc1_{t_i}", tag="dacc1")
                accs = [acc0, acc1]
                cur = 0
                for ti2, (dy, dx) in enumerate(taps):
                    src = hm[t_i][:, 3 + dy:11 + dy, 3 + dx:67 + dx]
                    wap = T["dw_w"][:, t_i, ti2:ti2 + 1]
                    if ti2 == 0:
                        nc.vector.scalar_tensor_tensor(accs[0][:], src, wap,
                                                       hm[t_i][:, OWN[0], OWN[1]], AL.mult, AL.add)
                    else:
                        nc.vector.scalar_tensor_tensor(accs[1 - cur][:], src, wap, accs[cur][:],
                                                       AL.mult, AL.add)
                        cur = 1 - cur
                r2t = pool.tile([128, 8, 64], F32, name=f"r2_{t_i}")
                nc.scalar.activation(r2t[:], accs[cur][:], AF.Relu,
                                     bias=T["dw_b"][:, t_i:t_i + 1],
                                     accum_out=part2[:, 2 * t_i:2 * t_i + 1])
                nc.scalar.activation(sqs[:], r2t[:], AF.Square,
                                     accum_out=part2[:, 2 * t_i + 1:2 * t_i + 2])
                r2.append(r2t)
            gat2 = allgather(2, part2[:], 8)
            red2 = pool.tile([128, 8], F32)
            nc.vector.tensor_reduce(red2[:], gat2[:, 0:8, :], mybir.AxisListType.X, AL.add)
            stat2 = psS.tile([128, 8], F32, name="stat2", tag="lnst")
            nc.tensor.matmul(stat2[:], T["selhp"][:], red2[:])
            S2 = pool.tile([128, 4], F32)
            T2 = pool.tile([128, 4], F32)
            for t_i in range(4):
                a, bcol = stat2[:, 2 * t_i:2 * t_i + 1], stat2[:, 2 * t_i + 1:2 * t_i + 2]
                mcol = pool.tile([128, 2], F32, name=f"m2col{t_i}", tag="mcol")
                nc.vector.tensor_scalar(mcol[:, 0:1], a, 1.0 / NPIX, None, AL.mult)
                nc.vector.tensor_tensor(mcol[:, 1:2], mcol[:, 0:1], mcol[:, 0:1], AL.mult)
                nc.vector.scalar_tensor_tensor(mcol[:, 1:2], bcol, 1.0 / NPIX, mcol[:, 1:2],
                                               AL.mult, AL.subtract)
                nc.scalar.activation(mcol[:, 1:2], mcol[:, 1:2], AF.Ln, bias=eps_ap)
                nc.scalar.activation(mcol[:, 1:2], mcol[:, 1:2], AF.Exp, scale=-0.5)
                nc.vector.tensor_tensor(S2[:, t_i:t_i + 1], mcol[:, 1:2],
                                        T["bn2_g"][:, t_i:t_i + 1], AL.mult)
                nc.vector.tensor_tensor(mcol[:, 0:1], mcol[:, 0:1], S2[:, t_i:t_i + 1], AL.mult)
                nc.vector.scalar_tensor_tensor(T2[:, t_i:t_i + 1], T["bn2_b"][:, t_i:t_i + 1],
                                               1.0, mcol[:, 0:1], AL.mult, AL.subtract)
            fo = psT.tile([128, 8, 64], F32, name="fo", tag="dwps")
            for t_i in range(4):
                zt = pool.tile([128, 8, 64], F32, name=f"zt{t_i}", tag="zt", bufs=2)
                nc.vector.tensor_scalar(zt[:], r2[t_i][:], S2[:, t_i:t_i + 1],
                                        T2[:, t_i:t_i + 1], AL.mult, AL.add)
                nc.tensor.matmul(fo[:], T["fc2T"][:, t_i, :], zt[:],
                                 start=(t_i == 0), stop=(t_i == 3), skip_group_check=True)
            fo_sb = pool.tile([128, 8, 64], F32)
            part3 = pool.tile([128, 8], F32)
            nc.scalar.activation(fo_sb[:], fo[:], AF.Copy, accum_out=part3[:, 0:1])
            nc.scalar.activation(sqs[:], fo_sb[:], AF.Square, accum_out=part3[:, 1:2])
            gat3 = allgather(3, part3[:, 0:2], 2)
            red3 = pool.tile([128, 2], F32)
            nc.vector.tensor_reduce(red3[:], gat3[:, 0:2, :], mybir.AxisListType.X, AL.add)
            stat3 = psS.tile([128, 2], F32, name="stat3", tag="lnst")
            nc.tensor.matmul(stat3[:], T["selhp"][:], red3[:])
            mS3 = pool.tile([128, 3], F32)
            nc.vector.tensor_scalar(mS3[:, 0:1], stat3[:, 0:1], 1.0 / NPIX, None, AL.mult)
            nc.vector.tensor_tensor(mS3[:, 1:2], mS3[:, 0:1], mS3[:, 0:1], AL.mult)
            nc.vector.scalar_tensor_tensor(mS3[:, 1:2], stat3[:, 1:2], 1.0 / NPIX, mS3[:, 1:2],
                                           AL.mult, AL.subtract)
            nc.scalar.activation(mS3[:, 1:2], mS3[:, 1:2], AF.Ln, bias=eps_ap)
            nc.scalar.activation(mS3[:, 1:2], mS3[:, 1:2], AF.Exp, scale=-0.5)
            nc.vector.tensor_tensor(mS3[:, 1:2], mS3[:, 1:2], T["bn3_g"][:], AL.mult)
            nc.vector.tensor_tensor(mS3[:, 2:3], mS3[:, 0:1], mS3[:, 1:2], AL.mult)
            nc.vector.scalar_tensor_tensor(mS3[:, 2:3], T["bn3_b"][:], 1.0, mS3[:, 2:3],
                                           AL.mult, AL.subtract)
            fin = pool.tile([128, 8, 64], F32)
            nc.vector.tensor_scalar(fin[:], fo_sb[:], mS3[:, 1:2], mS3[:, 2:3], AL.mult, AL.add)
            nc.vector.tensor_tensor(fin[:], fin[:], x1[:, OWN[0], OWN[1]], AL.add)
            fin16 = pool.tile([128, 8, 64], BF16)
            nc.vector.tensor_copy(fin16[:], fin[:])
            for h in (0, 1):
                nc.sync.dma_start(out[:, 8 * h:8 * h + 8, :], fin16[64 * h:64 * h + 64])
    _fix_multiwaits(nc)
    return nc


def fused_inputs(x):
    import ml_dtypes
    maps = []
    for i in range(NCORE):
        b, q = i // 4, i % 4
        r0 = 16 * q
        lo, hi = r0 - 3, r0 + 19
        slo, shi = max(lo, 0), min(hi, H)
        xarr = np.zeros((C, 22, W), np.float32)
        xarr[:, slo - lo:shi - lo, :] = x[b, :, slo:shi, :]
        maps.append(dict(xs=xarr.astype(ml_dtypes.bfloat16), oh=core_oh(i)))
    return maps


# ================================================================ kernel()
_PROGS = {}
TRACE = False            # set True to collect per-launch NTFF exec times
LAST_EXEC_NS = []        # filled per kernel() call when TRACE

WEIGHT_NAMES = [
    "ln1_g", "ln1_b", "ln2_g", "ln2_b", "con1_w", "con1_b",
    "xproj_w", "dtproj_w", "dtproj_b", "A_log", "Ds", "ssm_ln_g", "ssm_ln_b",
    "proj_w", "proj_b", "proj_bn_g", "proj_bn_b", "eca_w",
    "fc1_w", "bn1_g", "bn1_b",
    "dw_w1", "dw_b1", "dw_w3", "dw_b3", "dw_w5", "dw_b5", "dw_w7", "dw_b7",
    "bn2_g", "bn2_b", "fc2_w", "bn3_g", "bn3_b"]


FUSED = not os.environ.get("KERNEL_3L")


def _programs(W_):
    import hashlib
    hsh = hashlib.sha1()
    for nm in WEIGHT_NAMES:
        hsh.update(_f(W_[nm]).tobytes())
    key = hsh.hexdigest()
    if key not in _PROGS:
        _PROGS.clear()
        cw1 = l1_consts(W_["ln1_g"], W_["ln1_b"], W_["con1_w"], W_["con1_b"])
        cw2 = l2_consts(W_["xproj_w"], W_["dtproj_w"], W_["dtproj_b"],
                        W_["A_log"], W_["Ds"])
        cw3 = l3_consts(W_)
        if FUSED:
            _PROGS[key] = dict(fused=build_fused(cw1, cw2, cw3))
        else:
            _PROGS[key] = dict(l1=build_l1(cw1), l2=build_l2(cw2), l3=build_l3(cw3))
    return _PROGS[key]

_RUNNERS = {}


def _fast_runner(nc, n_cores=NCORE):
    """Cached jit + device-resident output operand buffers for one program.
    run_bass_via_pjrt rebuilds its jit closure (full retrace) and re-ships
    donated zero output buffers on every call; this does both once."""
    rs = _RUNNERS.get(id(nc))
    if rs is not None:
        return rs
    import jax
    import concourse.mybir as _mb
    from jax.experimental.shard_map import shard_map
    from jax.sharding import Mesh, PartitionSpec
    from concourse import bass2jax
    bass2jax.install_neuronx_cc_hook()
    assert nc.dbg_addr is None
    part_name = nc.partition_id_tensor.name if nc.partition_id_tensor else None
    in_names, out_names, out_avals = [], [], []
    for alloc in nc.m.functions[0].allocations:
        if not isinstance(alloc, _mb.MemoryLocationSet):
            continue
        name = alloc.memorylocations[0].name if alloc.memorylocations else None
        if alloc.kind == "ExternalInput":
            if name != part_name:
                in_names.append(name)
        elif alloc.kind == "ExternalOutput":
            out_names.append(name)
            out_avals.append(jax.core.ShapedArray(tuple(alloc.tensor_shape),
                                                  _mb.dt.np(alloc.dtype)))
    n_params = len(in_names)
    all_names = list(in_names) + list(out_names)
    if part_name is not None:
        all_names.append(part_name)
    all_names = tuple(all_names)

    def _body(*args):
        operands = list(args)
        if part_name is not None:
            operands.append(bass2jax.partition_id_tensor())
        outs = bass2jax._bass_exec_p.bind(
            *operands, out_avals=tuple(out_avals), in_names=all_names,
            out_names=tuple(out_names), lowering_input_output_aliases=(),
            sim_require_finite=True, sim_require_nnan=True, nc=nc)
        return tuple(outs)

    devices = jax.devices()[:n_cores]
    mesh = Mesh(np.asarray(devices), ("core",))
    spec = (PartitionSpec("core"),)
    fn = jax.jit(shard_map(_body, mesh=mesh,
                           in_specs=spec * (n_params + len(out_names)),
                           out_specs=spec * len(out_names), check_rep=False),
                 keep_unused=True)
    # outputs are fully written by the kernels, so skip donation and keep the
    # operand buffers device-resident across calls (no re-transfer).
    out_bufs = [
        jax.device_put(
            np.zeros((n_cores * av.shape[0], *av.shape[1:]), av.dtype),
            jax.sharding.NamedSharding(mesh, PartitionSpec("core")))
        for av in out_avals]
    rs = dict(fn=fn, in_names=in_names, out_names=out_names,
              out_avals=out_avals, out_bufs=out_bufs, n_cores=n_cores)
    _RUNNERS[id(nc)] = rs
    return rs


def _run_fast(nc, maps):
    rs = _fast_runner(nc)
    n = rs["n_cores"]
    concat_in = [np.concatenate([np.asarray(m[nm]) for m in maps], axis=0)
                 for nm in rs["in_names"]]
    outs = rs["fn"](*concat_in, *rs["out_bufs"])
    res = []
    for c in range(n):
        res.append({nm: np.asarray(outs[i]).reshape(n, *rs["out_avals"][i].shape)[c]
                    for i, nm in enumerate(rs["out_names"])})
    return res


def _run(nc, maps, cores):
    import time as _t
    from concourse.bass_utils import run_bass_kernel_spmd
    t0 = _t.perf_counter()
    try:
        res = _run_fast(nc, maps)
        if TRACE:
            LAST_EXEC_NS.append(int((_t.perf_counter() - t0) * 1e9))
        return res
    except Exception:
        import traceback
        traceback.print_exc()
        _RUNNERS.pop(id(nc), None)
    t0 = _t.perf_counter()
    r = run_bass_kernel_spmd(nc, maps, cores)
    if TRACE:
        LAST_EXEC_NS.append(int((_t.perf_counter() - t0) * 1e9))
    return r.results


def kernel(**inputs):
    W_ = {k: np.asarray(v) for k, v in inputs.items()}
    x = np.asarray(W_["x"], np.float32)
    P = _programs(W_)
    cores = list(range(NCORE))
    LAST_EXEC_NS.clear()

    if FUSED:
        res = _run(P["fused"], fused_inputs(x), cores)
        return l3_gather(res)

    m1 = l1_inputs(x)
    h = l1_gather(_run(P["l1"], m1, cores))

    m2 = l2_inputs(h)
    r2 = _run(P["l2"], m2, cores)
    y_by_core = [r2[i]["y"] for i in range(NCORE)]

    m3 = l3_inputs(y_by_core, x)
    return l3_gather(_run(P["l3"], m3, cores))



# revision 38
# speedup vs baseline: 37.6948x; 1.3625x over previous
"""DBSS block as three SPMD bass launches on 8 NeuronCores."""
import os, sys
for _p in ('/opt/trn_rl_repo', os.path.expanduser('~/.axon_site/_ro/trn_rl_repo')):
    if os.path.isdir(_p) and _p not in sys.path:
        sys.path.insert(0, _p)

import numpy as np
from contextlib import ExitStack
import concourse.bass as bass
import concourse.mybir as mybir
from concourse import tile

# The walrus build in this container rejects TPB_CTRL instructions carrying
# more than one semaphore wait ("Too many sync wait commands" in codegen's
# setupSyncWait). Tile's kernel-tail drain aggregates one wait per live
# semaphore, so split that drain into a chain of single-wait drains.
_orig_drain_and_barrier = tile.TileContext._drain_and_barrier

def _split_drain_and_barrier(self, tick_clock, wait_clock):
    from concourse.vector_clock import ScopedClock
    import bass_rust as _br
    probe = self.nc.sync.drain()
    wait_clock.add_sem_waits(probe.ins, ScopedClock({None: tick_clock.global_clock}))
    waits = list(probe.ins.sync_info.on_wait) if probe.ins.sync_info else []
    if waits:
        probe.ins.sync_info = _br.SyncInfo(on_wait=[], on_update=[])
        scratch = self.nc.alloc_sbuf_tensor(f"tail_wait_scratch_{self.uid}", [1, 64],
                                            mybir.dt.float32)
        for wi, w in enumerate(waits):
            ins = self.nc.vector.memset(scratch.ap()[:, wi % 64:wi % 64 + 1], 0.0)
            ins.ins.sync_info = _br.SyncInfo(on_wait=[w], on_update=[])
    self.nc.all_engine_barrier()
    assert self.sems is not None
    popped = self.nc._tile_sem_poison_stack.pop()
    assert popped is self._sem_poison
    self.nc.clear_and_free_semaphores(list(self.sems.allocated().values()))
    self.nc.all_engine_barrier()

tile.TileContext._drain_and_barrier = _split_drain_and_barrier


def _new_bass():
    nc = bass.Bass()
    nc._mw_scratch = nc.alloc_sbuf_tensor("mw_scratch", [1, 512], mybir.dt.float32)
    nc._mw_sems = [nc.alloc_semaphore(f"mw_sem_{i}") for i in range(64)]
    return nc


def _fix_multiwaits(nc):
    """This walrus accepts at most one sem wait per instruction. Distribute
    extra waits over single-wait DVE memset carriers; same-engine targets are
    ordered behind their carriers by the engine stream, cross-engine targets
    wait on a helper semaphore bumped by the last carrier."""
    import bass_rust as _br
    scratch = nc._mw_scratch
    helper_sems = []
    use_count = {}    # sem num -> times used (wait threshold increases per reuse)
    scri = [0]
    rr = [0]
    for bbw in nc.main_func.blocks:
        insns = bbw.instructions
        out = []
        for ins in insns:
            si = ins.sync_info
            waits = list(si.on_wait) if si else []
            if len(waits) <= 1:
                out.append(ins)
                continue
            eng = str(ins.engine)
            mk = []
            def carrier(w, upd=None):
                si_ = scri[0] % 512
                scri[0] += 1
                c = mybir.InstMemset(name=nc.get_next_instruction_name(),
                                     mode="Const", constant=0, ins=[],
                                     outs=[nc.vector.lower_ap(scratch.ap()[:, si_:si_ + 1])])
                c.engine = ins.engine if eng in ("EngineType.DVE", "EngineType.Pool") else mybir.EngineType.DVE
                c.sync_info = _br.SyncInfo(on_wait=[w] if w else [],
                                           on_update=[upd] if upd else [])
                nc.register_instruction(c, overwrite=True)
                mk.append(c)
            if eng in ("EngineType.DVE", "EngineType.Pool"):
                for w in waits[:-1]:
                    carrier(w)
                ins.sync_info = _br.SyncInfo(on_wait=[waits[-1]],
                                             on_update=list(si.on_update) if si else [])
            else:
                assert nc._mw_sems, "helper semaphore pool exhausted"
                sem = nc._mw_sems[rr[0] % len(nc._mw_sems)]
                rr[0] += 1
                n = use_count.get(sem.num, 0) + 1
                use_count[sem.num] = n
                if n == 1:
                    helper_sems.append(sem)
                for w in waits[:-1]:
                    carrier(w)
                carrier(waits[-1],
                        _br.SyncUpdate(sync_type='semaphore', id=sem.num,
                                       ant_name=sem.name, update_mode='sem-inc',
                                       update_value=1, update_reg=None))
                ins.sync_info = _br.SyncInfo(
                    on_wait=[_br.SyncWait(sync_type='semaphore', id=sem.num,
                                          ant_name=sem.name, wait_mode='sem-ge-imm',
                                          wait_value=n, wait_reg=None)],
                    on_update=list(si.on_update) if si else [])
            out.extend(mk)
            out.append(ins)
        bbw.instructions = out
    if helper_sems:
        from concourse.bass import compact_to_ranges as _ctr
        nums = [s.num for s in helper_sems]
        first_bb = nc.main_func.blocks[0]
        home = nc.cur_bb.bb
        n0 = len(home.instructions)
        try:
            rngs = _ctr(sorted(nums))
        except Exception:
            rngs = [range(n, n + 1) for n in sorted(nums)]
        for r in rngs:
            nc.gpsimd.sem_clear(r)
        lst = home.instructions
        head_clears = lst[n0:]
        home.instructions = lst[:n0]
        first_bb.instructions = head_clears + first_bb.instructions
        for r in rngs:
            nc.gpsimd.sem_clear(r)

F32 = mybir.dt.float32
F32R = mybir.dt.float32r
AL = mybir.AluOpType
AF = mybir.ActivationFunctionType

B, C, H, W = 2, 64, 64, 64
D2, L, N, R = 32, H * W, 16, 2
HID = 256
EPS = 1e-5
NCORE = 8
LINEARIZE = True

# ---------------------------------------------------------------- host utils
def row_snake(H=64, W=64):
    idx = np.arange(H * W).reshape(H, W)
    idx[1::2] = idx[1::2, ::-1]
    return idx.reshape(-1)

def col_snake(H=64, W=64):
    idx = np.arange(H * W).reshape(H, W).T.copy()
    idx[1::2] = idx[1::2, ::-1]
    return idx.reshape(-1)

IDX_R = row_snake()
IDX_C = col_snake()
INV_R = np.argsort(IDX_R)
INV_C = np.argsort(IDX_C)

def _f(a):
    return np.ascontiguousarray(a, dtype=np.float32)


def _const(nc, name, arr):
    return nc.inline_tensor(_f(arr), name=name)


def ksel(nc, pool, name, allt_ap, P, F, K, oh_ap, out=None):
    """Select the oh-weighted sum of K stacked (P,F) blocks of allt_ap.
    Intermediate accumulators rotate through a shared scratch tag so
    concurrent ksel results only pay for their dedicated `out` tile."""
    if out is None:
        out = pool.tile([P, F], F32, name=f"sel_{name}")
    cur = None
    for k in range(K):
        dst = out if k == K - 1 else pool.tile([P, F], F32, name=f"kscr_{name}_{k}",
                                               tag=f"ksel_scr_{P}x{F}", bufs=2)
        if k == 0:
            nc.vector.tensor_scalar(dst[:], allt_ap[:, 0:F], oh_ap[0:P, 0:1],
                                    None, AL.mult)
        else:
            nc.vector.scalar_tensor_tensor(dst[:], allt_ap[:, k * F:(k + 1) * F],
                                           oh_ap[0:P, k:k + 1], cur[:], AL.mult, AL.add)
        cur = dst
    return out

# ================================================================ LAUNCH 1
# ln1 over channels + 3x3 dilated (=2) depthwise conv.
# core i: b=i//4, q=i%4. own rows [16q,16q+16); stored 20 rows [16q-2,16q+18)
# partitions p = 64*h + c, h in {0,1}: half h stored-local rows [8h, 8h+12)
# tile free layout (12, 68), cols 2:66 are real, pad cols zero.

def mm_chunks(nc, out_ap, lhsT_ap, rhs_ap, chunk=512, f32r=True, acc=False):
    """Chunked matmul along free dim (PSUM bank limit). out/rhs are 2D flat APs."""
    F = out_ap.shape[-1]
    o = 0
    while o < F:
        n = min(chunk, F - o)
        lh, rh = lhsT_ap, rhs_ap[:, o:o + n]
        if f32r:
            lh, rh = lh.bitcast(F32R), rh.bitcast(F32R)
        if acc:
            nc.tensor.matmul(out_ap[:, o:o + n], lh, rh, start=acc[0], stop=acc[1],
                             skip_group_check=True)
        else:
            nc.tensor.matmul(out_ap[:, o:o + n], lh, rh)
        o += n


def build_l1(cw):
    nc = _new_bass()
    xs = nc.dram_tensor("xs", [C, 20, W], F32, kind="ExternalInput")
    oh = nc.dram_tensor("oh", [128, 8], F32, kind="ExternalInput")
    selsum = _const(nc, "selsum", cw["selsum"])
    selg = _const(nc, "selg", cw["selg"])
    b2 = _const(nc, "b2", cw["b2"])
    w9 = _const(nc, "w9", cw["w9"])
    cb = _const(nc, "cb", cw["cb"])
    cst = _const(nc, "cst", cw["cst"])  # [eps]
    mask_all = _const(nc, "mask_all", cw["mask_all"])  # (128, 4*12)
    hs = nc.dram_tensor("hs", [C, 16, W], F32, kind="ExternalOutput")

    with tile.TileContext(nc, linearize=LINEARIZE) as tc, ExitStack() as ctx:
        pool = ctx.enter_context(tc.tile_pool(name="pool", bufs=1))
        psum = ctx.enter_context(tc.tile_pool(name="psum", bufs=1, space="PSUM"))

        xt = pool.tile([128, 12, 68], F32)
        nc.vector.memset(xt[:], 0.0)
        for h in (0, 1):
            nc.sync.dma_start(xt[64 * h:64 * h + 64, :, 2:66], xs[:, 8 * h:8 * h + 12, :])
        selsum_t = pool.tile([128, 2], F32)
        selg_t = pool.tile([2, 128], F32)
        b2_t = pool.tile([128, 1], F32)
        w9_t = pool.tile([128, 9], F32)
        cb_t = pool.tile([128, 1], F32)
        oh_t = pool.tile([128, 8], F32)
        mask_all_t = pool.tile([128, 48], F32)
        cst_t = pool.tile([2, 2], F32)
        nc.sync.dma_start(selsum_t[:], selsum[:])
        nc.sync.dma_start(selg_t[:], selg[:])
        nc.sync.dma_start(b2_t[:], b2[:])
        nc.sync.dma_start(w9_t[:], w9[:])
        nc.sync.dma_start(cb_t[:], cb[:])
        nc.sync.dma_start(oh_t[:], oh[:])
        nc.sync.dma_start(mask_all_t[:], mask_all[:])
        nc.sync.dma_start(cst_t[:], cst[:])
        mask_t = ksel(nc, pool, "mask", mask_all_t[:], 128, 12, 4, oh_t[:])

        XW = xt[:, :, 2:66]                      # (128, 12, 64)
        sq = pool.tile([128, 12, 64], F32)
        nc.scalar.activation(sq[:], XW, AF.Square)

        st_x = psum.tile([2, 768], F32)
        st_xx = psum.tile([2, 768], F32)
        for r0, r1 in ((0, 8), (8, 12)):
            nc.tensor.matmul(st_x[:, r0 * 64:r1 * 64], selsum_t[:], xt[:, r0:r1, 2:66])
            nc.tensor.matmul(st_xx[:, r0 * 64:r1 * 64], selsum_t[:], sq[:, r0:r1, :])

        # per-pixel stats on (2,768)
        sm = pool.tile([2, 768], F32)
        nc.vector.tensor_scalar(sm[:], st_x[:], 1.0 / 64, None, AL.mult)
        var = pool.tile([2, 768], F32)
        nc.vector.tensor_tensor(var[:], sm[:], sm[:], AL.mult)
        nc.vector.scalar_tensor_tensor(var[:], st_xx[:], 1.0 / 64, var[:], AL.mult, AL.subtract)
        inv = pool.tile([2, 768], F32)
        nc.scalar.activation(inv[:], var[:], AF.Ln, bias=cst_t[:, 0:1])
        nc.scalar.activation(inv[:], inv[:], AF.Exp, scale=-0.5)
        minv = pool.tile([2, 768], F32)
        nc.vector.tensor_tensor(minv[:], sm[:], inv[:], AL.mult)

        sgb = psum.tile([128, 12, 64], F32)
        msgb = psum.tile([128, 12, 64], F32)
        for r0, r1 in ((0, 8), (8, 12)):
            nc.tensor.matmul(sgb[:, r0:r1, :], selg_t[:], inv[:, r0 * 64:r1 * 64])
            nc.tensor.matmul(msgb[:, r0:r1, :], selg_t[:], minv[:, r0 * 64:r1 * 64])

        xn = pool.tile([128, 12, 64], F32)
        nc.vector.tensor_tensor(xn[:], XW, sgb[:], AL.mult)
        nc.vector.scalar_tensor_tensor(xn[:], xn[:], b2_t[:], msgb[:], AL.add, AL.subtract)
        # masked into padded buffer
        xmp = pool.tile([128, 12, 68], F32)
        nc.vector.memset(xmp[:], 0.0)
        nc.vector.tensor_tensor(xmp[:, :, 2:66], xn[:],
                                mask_t[:].unsqueeze(2).broadcast_to([128, 12, 64]), AL.mult)

        # conv: out rows = stored 2..10 per half
        acc0 = pool.tile([128, 8, 64], F32)
        acc1 = pool.tile([128, 8, 64], F32)
        acc = [acc0, acc1]
        taps = [(dy, dx) for dy in (-2, 0, 2) for dx in (-2, 0, 2)]
        cur = 0
        for ti, (dy, dx) in enumerate(taps):
            src = xmp[:, 2 + dy:10 + dy, 2 + dx:66 + dx]
            if ti == 0:
                nc.vector.tensor_scalar(acc[0][:], src, w9_t[:, 0:1], cb_t[:], AL.mult, AL.add)
            else:
                eng = nc.vector
                eng.scalar_tensor_tensor(acc[1 - cur][:], src, w9_t[:, ti:ti + 1], acc[cur][:], AL.mult, AL.add)
                cur = 1 - cur
        for h in (0, 1):
            nc.sync.dma_start(hs[:, 8 * h:8 * h + 8, :], acc[cur][64 * h:64 * h + 64])
    _fix_multiwaits(nc)
    return nc


def l1_consts(ln1_g, ln1_b, con1_w, con1_b):
    selsum = np.zeros((128, 2), np.float32)
    selg = np.zeros((2, 128), np.float32)
    b2 = np.zeros((128, 1), np.float32)
    w9 = np.zeros((128, 9), np.float32)
    cbv = np.zeros((128, 1), np.float32)
    for p in range(128):
        h, c = p // 64, p % 64
        selsum[p, h] = 1.0
        selg[h, p] = ln1_g[c]
        b2[p, 0] = ln1_b[c]
        w9[p] = con1_w[c, 0].reshape(-1)
        cbv[p, 0] = con1_b[c]
    cst = np.zeros((2, 2), np.float32); cst[:, 0] = EPS
    mask_all = np.zeros((128, 4, 12), np.float32)
    for q in range(4):
        r0 = 16 * q
        for p in range(128):
            h = p // 64
            for r in range(12):
                g = r0 + 8 * h + r - 2
                if 0 <= g < H:
                    mask_all[p, q, r] = 1.0
    return dict(selsum=selsum, selg=selg, b2=b2, w9=w9, cb=cbv, cst=cst,
                mask_all=mask_all.reshape(128, 48))


def core_oh(i):
    oh = np.zeros((128, 8), np.float32)
    oh[:, i % 4] = 1.0
    oh[:, 4 + i // 4] = 1.0
    return oh


def l1_inputs(x):
    maps = []
    for i in range(NCORE):
        b, q = i // 4, i % 4
        r0 = 16 * q
        xs = np.zeros((C, 20, W), np.float32)
        lo, hi = r0 - 2, r0 + 18
        slo, shi = max(lo, 0), min(hi, H)
        xs[:, slo - lo:shi - lo, :] = x[b, :, slo:shi, :]
        maps.append(dict(xs=_f(xs), oh=core_oh(i)))
    return maps


def l1_gather(results):
    h = np.zeros((B, C, H, W), np.float32)
    for i in range(NCORE):
        b, q = i // 4, i % 4
        h[b, :, 16 * q:16 * q + 16, :] = results[i]["hs"]
    return h

# ================================================================ LAUNCH 2
# Selective scan for one direction k on one batch b per core.
# partitions p = 32*j + d (j = n_local 0..3, n = 4g+j), free = t (4096).

F32R_PROJ = False   # delta projection in full fp32 (precision-sensitive)
F32R_BC = False      # B4/C4 expansion matmuls in f32r
F32R_Y = False       # y reduction matmuls in f32r

def build_l2(cw):
    nc = _new_bass()
    u = nc.dram_tensor("u", [D2, L], F32, kind="ExternalInput")
    oh = nc.dram_tensor("oh", [128, 8], F32, kind="ExternalInput")
    d_all = _const(nc, "d_all", cw["d_all"])          # (32, 4*128)
    bias_all = _const(nc, "bias_all", cw["bias_all"])  # (128, 4)
    B_all = _const(nc, "B_all", cw["B_all"])          # (32, 4*512)
    C_all = _const(nc, "C_all", cw["C_all"])          # (32, 4*512)
    u4_c = _const(nc, "u4_c", cw["u4"])               # (32, 128)
    A_all = _const(nc, "A_all", cw["A_all"])          # (128, 16)
    y_c = _const(nc, "y_c", cw["y_lhsT"])             # (128, 32)
    Ds_all = _const(nc, "Ds_all", cw["Ds_all"])       # (32, 4*32)
    y = nc.dram_tensor("y", [D2, L], F32, kind="ExternalOutput")

    NCH = 8           # 512-column chunks
    CH = L // NCH

    with tile.TileContext(nc, linearize=LINEARIZE) as tc, ExitStack() as ctx:
        pool = ctx.enter_context(tc.tile_pool(name="pool", bufs=1))
        psA = ctx.enter_context(tc.tile_pool(name="psA", bufs=3, space="PSUM"))
        psY = ctx.enter_context(tc.tile_pool(name="psY", bufs=1, space="PSUM"))

        ut = pool.tile([D2, L], F32)
        nc.sync.dma_start(ut[:], u[:])
        oh_t = pool.tile([128, 8], F32)
        nc.sync.dma_start(oh_t[:], oh[:])
        lhsT_u4_t = pool.tile([D2, 128], F32)
        lhsT_y_t = pool.tile([128, D2], F32)
        nc.sync.dma_start(lhsT_u4_t[:], u4_c[:])
        nc.sync.dma_start(lhsT_y_t[:], y_c[:])
        def ksel_dram(nm, hd, P_, F_, fc=None):
            """Chunked DMA of a (P_, 4*F_) stacked DRAM const + oh-select."""
            outt = pool.tile([P_, F_], F32, name=f"sel_{nm}")
            fc = fc or F_
            for o in range(0, F_, fc):
                st = pool.tile([P_, 4 * fc], F32, name=f"st_{nm}_{o}",
                               tag=f"kst_{P_}x{4 * fc}", bufs=2)
                for k in range(4):
                    nc.sync.dma_start(st[:, k * fc:(k + 1) * fc],
                                      hd[:, k * F_ + o:k * F_ + o + fc])
                cur = None
                for k in range(4):
                    if k == 3:
                        dst = outt[:, o:o + fc]
                    else:
                        dst = pool.tile([P_, fc], F32, name=f"ks_{nm}_{o}_{k}",
                                        tag=f"kscr_{P_}x{fc}", bufs=2)[:]
                    if k == 0:
                        nc.vector.tensor_scalar(dst, st[:, 0:fc], oh_t[0:P_, 0:1],
                                                None, AL.mult)
                    else:
                        nc.vector.scalar_tensor_tensor(dst, st[:, k * fc:(k + 1) * fc],
                                                       oh_t[0:P_, k:k + 1], cur,
                                                       AL.mult, AL.add)
                    cur = dst
            return outt

        stack_t = {}
        for nm, hd, P_, F_, fc in (("d", d_all, D2, 128, None),
                                   ("bias", bias_all, 128, 1, None),
                                   ("B", B_all, D2, 512, 128),
                                   ("C", C_all, D2, 512, 128),
                                   ("A", A_all, 128, 4, None),
                                   ("Ds", Ds_all, D2, D2, None)):
            stack_t[nm] = ksel_dram(nm, hd, P_, F_, fc)
        lhsT_d_t = stack_t["d"]
        bias4_t = stack_t["bias"]
        lhsT_B_t = stack_t["B"]
        lhsT_C_t = stack_t["C"]
        A4_t = stack_t["A"]
        lhsT_Ds_t = stack_t["Ds"]

        def mm(out_ap, lh, rh, f32r, **kw):
            if f32r:
                lh, rh = lh.bitcast(F32R), rh.bitcast(F32R)
            nc.tensor.matmul(out_ap, lh, rh, **kw)

        d4 = pool.tile([128, L], F32)
        w4 = pool.tile([128, L], F32)
        # --- delta, w
        for c in range(NCH):
            dp = psA.tile([128, CH], F32, name=f"dp{c}", tag="ps")
            mm(dp[:], lhsT_d_t[:], ut[:, c * CH:(c + 1) * CH], F32R_PROJ)
            # softplus(x+b) = ln(1 + exp(x+b)); keeps ACT in the ln/exp table
            nc.scalar.activation(d4[:, c * CH:(c + 1) * CH], dp[:], AF.Exp,
                                 bias=bias4_t[:])
            nc.scalar.activation(d4[:, c * CH:(c + 1) * CH],
                                 d4[:, c * CH:(c + 1) * CH], AF.Ln,
                                 bias=nc.const_aps.tensor(1.0, (128, 1)))
        for c in range(NCH):
            u4p = psA.tile([128, CH], F32, name=f"u4p{c}", tag="ps")
            mm(u4p[:], lhsT_u4_t[:], ut[:, c * CH:(c + 1) * CH], F32R_BC)
            nc.vector.tensor_tensor(w4[:, c * CH:(c + 1) * CH],
                                    d4[:, c * CH:(c + 1) * CH], u4p[:], AL.mult)

        hs = []
        for g in range(4):
            dBu = pool.tile([128, L], F32, name=f"dBu{g}", tag="dBu", bufs=1)
            for c in range(NCH):
                b4 = psA.tile([128, CH], F32, name=f"b4_{g}_{c}", tag="ps")
                mm(b4[:], lhsT_B_t[:, g * 128:(g + 1) * 128],
                   ut[:, c * CH:(c + 1) * CH], F32R_BC)
                nc.vector.tensor_tensor(dBu[:, c * CH:(c + 1) * CH],
                                        w4[:, c * CH:(c + 1) * CH], b4[:], AL.mult)
            dA = pool.tile([128, L], F32, name=f"dA{g}", tag="dA", bufs=2)
            nc.scalar.activation(dA[:], d4[:], AF.Exp, scale=A4_t[:, g:g + 1])
            hsg = pool.tile([128, L], F32, name=f"hs{g}")
            eng = nc.vector
            eng.tensor_tensor_scan(hsg[:], dA[:], dBu[:], 0.0, AL.mult, AL.add)
            hs.append(hsg)

        # --- phase B: y = sum_g lhsT_y.T @ (hs_g * C4_g) + diag(Ds) @ u
        for half in range(2):
            yps = psY.tile([D2, L // 2], F32, name=f"yps{half}", tag="yps")
            for g in range(4):
                fsb = pool.tile([128, L // 2], F32, name=f"f_{half}_{g}", tag="fsb", bufs=2)
                for cc in range(NCH // 2):
                    c = half * (NCH // 2) + cc
                    c4 = psA.tile([128, CH], F32, name=f"c4_{g}_{c}", tag="ps")
                    mm(c4[:], lhsT_C_t[:, g * 128:(g + 1) * 128],
                       ut[:, c * CH:(c + 1) * CH], F32R_BC)
                    if g % 2 == 0:
                        nc.vector.tensor_tensor(fsb[:, cc * CH:(cc + 1) * CH],
                                                hs[g][:, c * CH:(c + 1) * CH], c4[:], AL.mult)
                    else:
                        c4sb = pool.tile([128, CH], F32, name=f"c4sb_{g}_{c}", tag="c4sb", bufs=2)
                        nc.scalar.copy(c4sb[:], c4[:])
                        nc.vector.tensor_tensor(fsb[:, cc * CH:(cc + 1) * CH],
                                                hs[g][:, c * CH:(c + 1) * CH], c4sb[:], AL.mult)
                for cc in range(NCH // 2):
                    mm(yps[:, cc * CH:(cc + 1) * CH], lhsT_y_t[:],
                       fsb[:, cc * CH:(cc + 1) * CH], F32R_Y,
                       start=(g == 0), stop=False, skip_group_check=True)
            for cc in range(NCH // 2):
                c = half * (NCH // 2) + cc
                mm(yps[:, cc * CH:(cc + 1) * CH], lhsT_Ds_t[:],
                   ut[:, c * CH:(c + 1) * CH], F32R_Y,
                   start=False, stop=True, skip_group_check=True)
            ysb = pool.tile([D2, L // 2], F32, name=f"ysb{half}", tag="ysb", bufs=1)
            nc.scalar.copy(ysb[:], yps[:])
            nc.sync.dma_start(y[:, half * (L // 2):(half + 1) * (L // 2)], ysb[:])
    _fix_multiwaits(nc)
    return nc


def l2_consts(xproj_w, dtproj_w, dtproj_b, A_log, Ds):
    A = -np.exp(np.asarray(A_log))
    d_all = np.zeros((D2, 4, 128), np.float32)
    B_all = np.zeros((D2, 4, 512), np.float32)
    C_all = np.zeros((D2, 4, 512), np.float32)
    bias_all = np.zeros((128, 4), np.float32)
    A_all = np.zeros((128, 16), np.float32)
    Ds_all = np.zeros((D2, 4, D2), np.float32)
    u4 = np.zeros((D2, 128), np.float32)
    y_lhsT = np.zeros((128, D2), np.float32)
    for k in range(4):
        Wd = dtproj_w[k] @ xproj_w[k, :R]        # (32, 32)
        Ds_all[:, k, :] = np.diag(Ds[k])
        for j in range(4):
            for d in range(D2):
                p = 32 * j + d
                d_all[:, k, p] = Wd[d]
                u4[d, p] = 1.0
                bias_all[p, k] = dtproj_b[k, d]
                y_lhsT[p, d] = 1.0
                for g in range(4):
                    n = 4 * g + j
                    B_all[:, k, g * 128 + p] = xproj_w[k, R + n]
                    C_all[:, k, g * 128 + p] = xproj_w[k, R + N + n]
                    A_all[p, 4 * k + g] = A[k, d, n]
    return dict(d_all=d_all.reshape(D2, 512), bias_all=bias_all,
                B_all=B_all.reshape(D2, 2048), C_all=C_all.reshape(D2, 2048),
                u4=u4, A_all=A_all, y_lhsT=y_lhsT, Ds_all=Ds_all.reshape(D2, 128))


def l2_inputs(h):
    """h: (B, 64, H, W) conv output. returns 8 in_maps, core i -> (b=i//4, k=i%4)."""
    xf = h.reshape(B, C, L)
    maps = []
    for i in range(NCORE):
        b, k = i // 4, i % 4
        half = xf[b, :D2] if k < 2 else xf[b, D2:]
        perm = IDX_R if k < 2 else IDX_C
        uu = half[:, perm]
        if k % 2 == 1:
            uu = uu[:, ::-1]
        maps.append(dict(u=_f(uu), oh=core_oh(i)))
    return maps

# ================================================================ LAUNCH 3
# core i: b=i//4, q=i%4; own rows [16q,16q+16); stored 22 rows [16q-3,16q+19)
# partitions p = 64h + c ; half h stored-local rows [8h, 8h+14)
# free layout (14, 70), real cols 3:67; own window rows [3,11) cols [3,67)

NPIX = 8192.0   # B*H*W
NPB = 4096.0    # H*W

def build_l3(cw):
    nc = _new_bass()
    y4 = nc.dram_tensor("y4", [4, D2, 22, W], F32, kind="ExternalInput")
    xs = nc.dram_tensor("xs", [C, 22, W], F32, kind="ExternalInput")
    oh = nc.dram_tensor("oh", [128, 8], F32, kind="ExternalInput")
    inp = {"y4": y4, "xs": xs}
    cshapes = [("rowm_all", [128, 56]), ("bmask_all", [128, 16]),
               ("selsum", [128, 2]), ("selhp", [128, 128]),
               ("sg_ssm", [2, 128]), ("sb_ssm", [128, 1]),
               ("sg_ln2", [2, 128]), ("sb_ln2", [128, 1]),
               ("projT", [128, 128]), ("projb", [128, 1]),
               ("bnp_g", [128, 1]), ("bnp_b", [128, 1]),
               ("ecaT", [128, 128]),
               ("fc1T", [128, 512]),
               ("bn1_g", [128, 4]), ("bn1_b", [128, 4]),
               ("dw_w", [128, 4, 49]), ("dw_b", [128, 4]),
               ("bn2_g", [128, 4]), ("bn2_b", [128, 4]),
               ("fc2T", [128, 4, 128]),
               ("bn3_g", [128, 1]), ("bn3_b", [128, 1]),
               ("cst", [128, 2])]
    consts = {nm: _const(nc, nm, cw[nm].reshape(shp)) for nm, shp in cshapes}
    out = nc.dram_tensor("out", [C, 16, W], F32, kind="ExternalOutput")
    cc_in = [nc.dram_tensor(f"ccin{r}", [128, 16], F32) for r in range(4)]
    cc_out = [nc.dram_tensor(f"ccout{r}", [NCORE * 128, 16], F32) for r in range(4)]

    RW = 70           # row width incl pads
    FF = 14 * RW      # 980
    OWN = (slice(3, 11), slice(3, 67))

    with tile.TileContext(nc, linearize=LINEARIZE) as tc, ExitStack() as ctx:
        pool = ctx.enter_context(tc.tile_pool(name="pool", bufs=1))
        psT = ctx.enter_context(tc.tile_pool(name="psT", bufs=2, space="PSUM"))
        psS = ctx.enter_context(tc.tile_pool(name="psS", bufs=2, space="PSUM"))

        T = {}
        for nm, shp in cshapes:
            T[nm] = pool.tile(shp, F32, name=f"t_{nm}")
            nc.sync.dma_start(T[nm][:], consts[nm][:])
        eps_ap = T["cst"][:, 0:1]
        oh_t = pool.tile([128, 8], F32)
        nc.sync.dma_start(oh_t[:], oh[:])
        rowm = ksel(nc, pool, "rowm", T["rowm_all"][:], 128, 14, 4, oh_t[:])
        bmask_t = ksel(nc, pool, "bmask", T["bmask_all"][:], 128, 8, 2, oh_t[:, 4:6])

        def chunks2(tile3, rows=14):
            """two row-chunks of a (128,14,70) tile"""
            return [tile3[:, 0:7, :], tile3[:, 7:14, :]]

        def ln_ch(src, selg_key, b_key, nm):
            """channel LayerNorm of (128,14,70) tile -> new tile"""
            sq = pool.tile([128, 14, RW], F32, name=f"sq_{nm}", tag="lnsq")
            nc.scalar.activation(sq[:], src[:], AF.Square)
            inv = pool.tile([2, 14, RW], F32, name=f"inv_{nm}", tag="lninv")
            minv = pool.tile([2, 14, RW], F32, name=f"minv_{nm}", tag="lnminv")
            for ci, (s_ap, q_ap) in enumerate(zip(chunks2(src), chunks2(sq))):
                px = psS.tile([2, 7 * RW], F32, name=f"px_{nm}{ci}", tag="lnst")
                pq = psS.tile([2, 7 * RW], F32, name=f"pq_{nm}{ci}", tag="lnst")
                nc.tensor.matmul(px[:], T["selsum"][:], s_ap)
                nc.tensor.matmul(pq[:], T["selsum"][:], q_ap)
                ivc = inv[:, 7 * ci:7 * ci + 7, :]
                mvc = minv[:, 7 * ci:7 * ci + 7, :]
                sm = pool.tile([2, 7, RW], F32, name=f"sm_{nm}{ci}", tag="lnsm")
                nc.vector.tensor_scalar(sm[:], px[:], 1.0 / 64, None, AL.mult)
                nc.vector.tensor_tensor(ivc, sm[:], sm[:], AL.mult)
                nc.vector.scalar_tensor_tensor(ivc, pq[:], 1.0 / 64, ivc, AL.mult, AL.subtract)
                nc.scalar.activation(ivc, ivc, AF.Ln, bias=T["cst"][0:2, 0:1])
                nc.scalar.activation(ivc, ivc, AF.Exp, scale=-0.5)
                nc.vector.tensor_tensor(mvc, sm[:], ivc, AL.mult)
            dst = pool.tile([128, 14, RW], F32, name=f"ln_{nm}")
            for ci in range(2):
                rs = slice(7 * ci, 7 * ci + 7)
                sgb = psS.tile([128, 7 * RW], F32, name=f"sgb_{nm}{ci}", tag="lnbc")
                msgb = psS.tile([128, 7 * RW], F32, name=f"msgb_{nm}{ci}", tag="lnbc")
                nc.tensor.matmul(sgb[:], T[selg_key][:], inv[:, rs, :])
                nc.tensor.matmul(msgb[:], T[selg_key][:], minv[:, rs, :])
                nc.vector.tensor_tensor(dst[:, rs, :], src[:, rs, :],
                                        sgb[:].rearrange("p (a b) -> p a b", a=7), AL.mult)
                nc.vector.scalar_tensor_tensor(dst[:, rs, :], dst[:, rs, :], T[b_key][:],
                                               msgb[:].rearrange("p (a b) -> p a b", a=7),
                                               AL.add, AL.subtract)
            return dst

        def allgather(rnd, cols_src_ap, ncols):
            """partials (128, ncols) -> gathered sbuf tile (128, ncols, 8)"""
            ci = pool.tile([128, 16], F32, name=f"cci_{rnd}", tag="cci")
            nc.vector.memset(ci[:], 0.0)
            nc.vector.tensor_copy(ci[:, 0:ncols], cols_src_ap)
            nc.sync.dma_start(cc_in[rnd][:], ci[:])
            nc.gpsimd.collective_compute(
                "AllGather", AL.bypass, replica_groups=[list(range(NCORE))],
                ins=[cc_in[rnd][:]], outs=[cc_out[rnd][:]])
            gat = pool.tile([128, 16, NCORE], F32, name=f"gat_{rnd}", tag="gat")
            src = cc_out[rnd][:].rearrange("(n p) c -> p c n", p=128)
            nc.sync.dma_start(gat[:, 0:16, :], src)
            return gat

        # ---- merge y4 into ym
        ya = pool.tile([128, 14, RW], F32)
        yb = pool.tile([128, 14, RW], F32)
        nc.vector.memset(ya[:], 0.0)
        nc.vector.memset(yb[:], 0.0)
        for h in (0, 1):
            for m, dsttile in ((0, ya), (1, yb), (2, ya), (3, yb)):
                p0 = 64 * h + 32 * (m // 2)
                nc.sync.dma_start(dsttile[p0:p0 + 32, :, 3:67],
                                  inp["y4"][m, :, 8 * h:8 * h + 14, :])
        ym = pool.tile([128, 14, RW], F32)
        nc.vector.tensor_tensor(ym[:], ya[:], yb[:], AL.add)

        xt = pool.tile([128, 14, RW], F32)
        nc.vector.memset(xt[:], 0.0)
        for h in (0, 1):
            nc.sync.dma_start(xt[64 * h:64 * h + 64, :, 3:67], inp["xs"][:, 8 * h:8 * h + 14, :])

        # ---- ssm_ln, ln2, proj, relu
        z1 = ln_ch(ym, "sg_ssm", "sb_ssm", "ssm")
        z2 = ln_ch(z1, "sg_ln2", "sb_ln2", "ln2a")
        pr = pool.tile([128, 14, RW], F32)
        for ci, z_ap in enumerate(chunks2(z2)):
            pp = psT.tile([128, 7 * RW], F32, name=f"pp{ci}", tag="ps1")
            nc.tensor.matmul(pp[:], T["projT"][:], z_ap)
            nc.scalar.activation(pr[:, 7 * ci:7 * ci + 7, :],
                                 pp[:].rearrange("p (a b) -> p a b", a=7),
                                 AF.Relu, bias=T["projb"][:])

        # ---- bn-proj + pool partials, round 0
        prow = pr[:, OWN[0], OWN[1]]
        sqs = pool.tile([128, 8, 64], F32, name="sqs", tag="sqscratch")
        part0 = pool.tile([128, 3], F32)
        nc.vector.tensor_reduce(part0[:, 0:1], prow, mybir.AxisListType.XY, AL.add)
        nc.scalar.activation(sqs[:], prow, AF.Square, accum_out=part0[:, 1:2])
        nc.vector.tensor_copy(part0[:, 2:3], part0[:, 0:1])
        gat0 = allgather(0, part0[:], 3)
        # bn sums over all 8; pool sums over own-b cores
        red0 = pool.tile([128, 4], F32)
        nc.vector.tensor_reduce(red0[:, 0:2], gat0[:, 0:2, :], mybir.AxisListType.X, AL.add)
        pm = pool.tile([128, 16, NCORE], F32, name="pm", tag="pmx")
        nc.vector.tensor_tensor(pm[:, 2:3, :], gat0[:, 2:3, :],
                                bmask_t[:].unsqueeze(1), AL.mult)
        nc.vector.tensor_reduce(red0[:, 2:3], pm[:, 2:3, :], mybir.AxisListType.X, AL.add)
        stat0 = psS.tile([128, 4], F32, name="stat0", tag="lnst")
        nc.tensor.matmul(stat0[:, 0:3], T["selhp"][:], red0[:, 0:3])
        # S = g*rsqrt(v+eps), TT = b - m*S ; pool_bn = poolmean*S + TT
        mS = pool.tile([128, 6], F32)
        nc.vector.tensor_scalar(mS[:, 0:1], stat0[:, 0:1], 1.0 / NPIX, None, AL.mult)
        nc.vector.tensor_tensor(mS[:, 1:2], mS[:, 0:1], mS[:, 0:1], AL.mult)
        nc.vector.scalar_tensor_tensor(mS[:, 1:2], stat0[:, 1:2], 1.0 / NPIX, mS[:, 1:2],
                                       AL.mult, AL.subtract)
        nc.scalar.activation(mS[:, 1:2], mS[:, 1:2], AF.Ln, bias=eps_ap)
        nc.scalar.activation(mS[:, 1:2], mS[:, 1:2], AF.Exp, scale=-0.5)
        nc.vector.tensor_tensor(mS[:, 1:2], mS[:, 1:2], T["bnp_g"][:], AL.mult)  # S
        nc.vector.tensor_tensor(mS[:, 2:3], mS[:, 0:1], mS[:, 1:2], AL.mult)
        nc.vector.scalar_tensor_tensor(mS[:, 2:3], T["bnp_b"][:], 1.0, mS[:, 2:3],
                                       AL.mult, AL.subtract)                      # TT
        nc.vector.tensor_scalar(mS[:, 3:4], stat0[:, 2:3], 1.0 / NPB, None, AL.mult)
        nc.vector.tensor_tensor(mS[:, 3:4], mS[:, 3:4], mS[:, 1:2], AL.mult)
        nc.vector.tensor_tensor(mS[:, 3:4], mS[:, 3:4], mS[:, 2:3], AL.add)       # pool_bn
        # ---- eca
        ecp = psS.tile([128, 1], F32, name="ecp", tag="lnst")
        nc.tensor.matmul(ecp[:], T["ecaT"][:], mS[:, 3:4])
        sg = pool.tile([128, 2], F32)
        nc.scalar.activation(sg[:, 0:1], ecp[:], AF.Exp, scale=-1.0)
        nc.vector.tensor_scalar(sg[:, 0:1], sg[:, 0:1], 1.0, None, AL.add)
        nc.vector.reciprocal(sg[:, 1:2], sg[:, 0:1])
        # ---- x1 = sg * (pr*S + TT) + xt
        x1 = pool.tile([128, 14, RW], F32)
        nc.vector.tensor_scalar(x1[:], pr[:], mS[:, 1:2], mS[:, 2:3], AL.mult, AL.add)
        nc.vector.scalar_tensor_tensor(x1[:], x1[:], sg[:, 1:2], xt[:], AL.mult, AL.add)

        # ---- mlp
        m2 = ln_ch(x1, "sg_ln2", "sb_ln2", "ln2b")
        hm = []
        for t_i in range(4):
            hmt = pool.tile([128, 14, RW], F32, name=f"hm{t_i}")
            for ci, m_ap in enumerate(chunks2(m2)):
                fp = psT.tile([128, 7 * RW], F32, name=f"fp{t_i}{ci}", tag="ps1")
                nc.tensor.matmul(fp[:], T["fc1T"][:, 128 * t_i:128 * t_i + 128], m_ap)
                nc.scalar.activation(hmt[:, 7 * ci:7 * ci + 7, :],
                                     fp[:].rearrange("p (a b) -> p a b", a=7), AF.Relu)
            hm.append(hmt)
        # bn1 partials
        part1 = pool.tile([128, 8], F32)
        for t_i in range(4):
            nc.vector.tensor_reduce(part1[:, 2 * t_i:2 * t_i + 1], hm[t_i][:, OWN[0], OWN[1]],
                                    mybir.AxisListType.XY, AL.add)
            nc.scalar.activation(sqs[:], hm[t_i][:, OWN[0], OWN[1]], AF.Square,
                                 accum_out=part1[:, 2 * t_i + 1:2 * t_i + 2])
        gat1 = allgather(1, part1[:], 8)
        red1 = pool.tile([128, 8], F32)
        nc.vector.tensor_reduce(red1[:], gat1[:, 0:8, :], mybir.AxisListType.X, AL.add)
        stat1 = psS.tile([128, 8], F32, name="stat1", tag="lnst")
        nc.tensor.matmul(stat1[:], T["selhp"][:], red1[:])
        S1 = pool.tile([128, 4], F32)
        T1 = pool.tile([128, 4], F32)
        for t_i in range(4):
            a, bcol = stat1[:, 2 * t_i:2 * t_i + 1], stat1[:, 2 * t_i + 1:2 * t_i + 2]
            mcol = pool.tile([128, 2], F32, name=f"mcol{t_i}", tag="mcol")
            nc.vector.tensor_scalar(mcol[:, 0:1], a, 1.0 / NPIX, None, AL.mult)
            nc.vector.tensor_tensor(mcol[:, 1:2], mcol[:, 0:1], mcol[:, 0:1], AL.mult)
            nc.vector.scalar_tensor_tensor(mcol[:, 1:2], bcol, 1.0 / NPIX, mcol[:, 1:2],
                                           AL.mult, AL.subtract)
            nc.scalar.activation(mcol[:, 1:2], mcol[:, 1:2], AF.Ln, bias=eps_ap)
            nc.scalar.activation(mcol[:, 1:2], mcol[:, 1:2], AF.Exp, scale=-0.5)
            nc.vector.tensor_tensor(S1[:, t_i:t_i + 1], mcol[:, 1:2],
                                    T["bn1_g"][:, t_i:t_i + 1], AL.mult)
            nc.vector.tensor_tensor(mcol[:, 0:1], mcol[:, 0:1], S1[:, t_i:t_i + 1], AL.mult)
            nc.vector.scalar_tensor_tensor(T1[:, t_i:t_i + 1], T["bn1_b"][:, t_i:t_i + 1],
                                           1.0, mcol[:, 0:1], AL.mult, AL.subtract)
        # apply bn1 + mask (valid rows via rowm broadcast, zero the pad cols)
        for t_i in range(4):
            nc.vector.tensor_scalar(hm[t_i][:], hm[t_i][:], S1[:, t_i:t_i + 1],
                                    T1[:, t_i:t_i + 1], AL.mult, AL.add)
            nc.vector.tensor_tensor(hm[t_i][:, :, 3:67], hm[t_i][:, :, 3:67],
                                    rowm[:].unsqueeze(2).broadcast_to([128, 14, 64]),
                                    AL.mult)
            nc.vector.memset(hm[t_i][:, :, 0:3], 0.0)
            nc.vector.memset(hm[t_i][:, :, 67:70], 0.0)

        # ---- depthwise convs + residual (+bias), relu, bn2 partials
        KS = [1, 3, 5, 7]
        part2 = pool.tile([128, 8], F32)
        r2 = []
        for t_i, ks in enumerate(KS):
            pad = ks // 2
            taps = [(dy, dx) for dy in range(-pad, pad + 1) for dx in range(-pad, pad + 1)]
            acc0 = pool.tile([128, 8, 64], F32, name=f"dacc0_{t_i}", tag="dacc0")
            acc1 = pool.tile([128, 8, 64], F32, name=f"dacc1_{t_i}", tag="dacc1")
            accs = [acc0, acc1]
            cur = 0
            for ti2, (dy, dx) in enumerate(taps):
                src = hm[t_i][:, 3 + dy:11 + dy, 3 + dx:67 + dx]
                wap = T["dw_w"][:, t_i, ti2:ti2 + 1]
                if ti2 == 0:
                    nc.vector.scalar_tensor_tensor(accs[0][:], src, wap,
                                                   hm[t_i][:, OWN[0], OWN[1]], AL.mult, AL.add)
                else:
                    nc.vector.scalar_tensor_tensor(accs[1 - cur][:], src, wap, accs[cur][:],
                                             AL.mult, AL.add)
                    cur = 1 - cur
            r2t = pool.tile([128, 8, 64], F32, name=f"r2_{t_i}")
            nc.scalar.activation(r2t[:], accs[cur][:], AF.Relu,
                                 bias=T["dw_b"][:, t_i:t_i + 1],
                                 accum_out=part2[:, 2 * t_i:2 * t_i + 1])
            nc.scalar.activation(sqs[:], r2t[:], AF.Square,
                                 accum_out=part2[:, 2 * t_i + 1:2 * t_i + 2])
            r2.append(r2t)
        gat2 = allgather(2, part2[:], 8)
        red2 = pool.tile([128, 8], F32)
        nc.vector.tensor_reduce(red2[:], gat2[:, 0:8, :], mybir.AxisListType.X, AL.add)
        stat2 = psS.tile([128, 8], F32, name="stat2", tag="lnst")
        nc.tensor.matmul(stat2[:], T["selhp"][:], red2[:])
        S2 = pool.tile([128, 4], F32)
        T2 = pool.tile([128, 4], F32)
        for t_i in range(4):
            a, bcol = stat2[:, 2 * t_i:2 * t_i + 1], stat2[:, 2 * t_i + 1:2 * t_i + 2]
            mcol = pool.tile([128, 2], F32, name=f"m2col{t_i}", tag="mcol")
            nc.vector.tensor_scalar(mcol[:, 0:1], a, 1.0 / NPIX, None, AL.mult)
            nc.vector.tensor_tensor(mcol[:, 1:2], mcol[:, 0:1], mcol[:, 0:1], AL.mult)
            nc.vector.scalar_tensor_tensor(mcol[:, 1:2], bcol, 1.0 / NPIX, mcol[:, 1:2],
                                           AL.mult, AL.subtract)
            nc.scalar.activation(mcol[:, 1:2], mcol[:, 1:2], AF.Ln, bias=eps_ap)
            nc.scalar.activation(mcol[:, 1:2], mcol[:, 1:2], AF.Exp, scale=-0.5)
            nc.vector.tensor_tensor(S2[:, t_i:t_i + 1], mcol[:, 1:2],
                                    T["bn2_g"][:, t_i:t_i + 1], AL.mult)
            nc.vector.tensor_tensor(mcol[:, 0:1], mcol[:, 0:1], S2[:, t_i:t_i + 1], AL.mult)
            nc.vector.scalar_tensor_tensor(T2[:, t_i:t_i + 1], T["bn2_b"][:, t_i:t_i + 1],
                                           1.0, mcol[:, 0:1], AL.mult, AL.subtract)
        # ---- fc2 (accumulate over 4 input tiles) + bn3 + x1
        fo = psT.tile([128, 8, 64], F32, name="fo", tag="dwps")
        for t_i in range(4):
            zt = pool.tile([128, 8, 64], F32, name=f"zt{t_i}", tag="zt", bufs=2)
            nc.vector.tensor_scalar(zt[:], r2[t_i][:], S2[:, t_i:t_i + 1],
                                    T2[:, t_i:t_i + 1], AL.mult, AL.add)
            nc.tensor.matmul(fo[:], T["fc2T"][:, t_i, :], zt[:],
                             start=(t_i == 0), stop=(t_i == 3), skip_group_check=True)
        fo_sb = pool.tile([128, 8, 64], F32)
        part3 = pool.tile([128, 8], F32)
        nc.scalar.activation(fo_sb[:], fo[:], AF.Copy, accum_out=part3[:, 0:1])
        nc.scalar.activation(sqs[:], fo_sb[:], AF.Square, accum_out=part3[:, 1:2])
        gat3 = allgather(3, part3[:, 0:2], 2)
        red3 = pool.tile([128, 2], F32)
        nc.vector.tensor_reduce(red3[:], gat3[:, 0:2, :], mybir.AxisListType.X, AL.add)
        stat3 = psS.tile([128, 2], F32, name="stat3", tag="lnst")
        nc.tensor.matmul(stat3[:], T["selhp"][:], red3[:])
        mS3 = pool.tile([128, 3], F32)
        nc.vector.tensor_scalar(mS3[:, 0:1], stat3[:, 0:1], 1.0 / NPIX, None, AL.mult)
        nc.vector.tensor_tensor(mS3[:, 1:2], mS3[:, 0:1], mS3[:, 0:1], AL.mult)
        nc.vector.scalar_tensor_tensor(mS3[:, 1:2], stat3[:, 1:2], 1.0 / NPIX, mS3[:, 1:2],
                                       AL.mult, AL.subtract)
        nc.scalar.activation(mS3[:, 1:2], mS3[:, 1:2], AF.Ln, bias=eps_ap)
        nc.scalar.activation(mS3[:, 1:2], mS3[:, 1:2], AF.Exp, scale=-0.5)
        nc.vector.tensor_tensor(mS3[:, 1:2], mS3[:, 1:2], T["bn3_g"][:], AL.mult)
        nc.vector.tensor_tensor(mS3[:, 2:3], mS3[:, 0:1], mS3[:, 1:2], AL.mult)
        nc.vector.scalar_tensor_tensor(mS3[:, 2:3], T["bn3_b"][:], 1.0, mS3[:, 2:3],
                                       AL.mult, AL.subtract)
        fin = pool.tile([128, 8, 64], F32)
        nc.vector.tensor_scalar(fin[:], fo_sb[:], mS3[:, 1:2], mS3[:, 2:3], AL.mult, AL.add)
        nc.vector.tensor_tensor(fin[:], fin[:], x1[:, OWN[0], OWN[1]], AL.add)
        for h in (0, 1):
            nc.sync.dma_start(out[:, 8 * h:8 * h + 8, :], fin[64 * h:64 * h + 64])
    _fix_multiwaits(nc)
    return nc

def l3_consts(W_):
    selsum = np.zeros((128, 2), np.float32)
    selhp = np.zeros((128, 128), np.float32)
    sg_ssm = np.zeros((2, 128), np.float32); sb_ssm = np.zeros((128, 1), np.float32)
    sg_ln2 = np.zeros((2, 128), np.float32); sb_ln2 = np.zeros((128, 1), np.float32)
    projT = np.zeros((128, 128), np.float32); projb = np.zeros((128, 1), np.float32)
    bnp_g = np.zeros((128, 1), np.float32); bnp_b = np.zeros((128, 1), np.float32)
    ecaT = np.zeros((128, 128), np.float32)
    fc1T = np.zeros((128, 512), np.float32)
    bn1_g = np.zeros((128, 4), np.float32); bn1_b = np.zeros((128, 4), np.float32)
    dw_w = np.zeros((128, 4, 49), np.float32); dw_b = np.zeros((128, 4), np.float32)
    bn2_g = np.zeros((128, 4), np.float32); bn2_b = np.zeros((128, 4), np.float32)
    fc2T = np.zeros((128, 4, 128), np.float32)
    bn3_g = np.zeros((128, 1), np.float32); bn3_b = np.zeros((128, 1), np.float32)
    cst = np.zeros((128, 2), np.float32); cst[:, 0] = EPS
    dwk = [W_["dw_w1"], W_["dw_w3"], W_["dw_w5"], W_["dw_w7"]]
    dwb = [W_["dw_b1"], W_["dw_b3"], W_["dw_b5"], W_["dw_b7"]]
    for p in range(128):
        h, c = p // 64, p % 64
        selsum[p, h] = 1.0
        sg_ssm[h, p] = W_["ssm_ln_g"][c]; sb_ssm[p, 0] = W_["ssm_ln_b"][c]
        sg_ln2[h, p] = W_["ln2_g"][c]; sb_ln2[p, 0] = W_["ln2_b"][c]
        projb[p, 0] = W_["proj_b"][c]
        bnp_g[p, 0] = W_["proj_bn_g"][c]; bnp_b[p, 0] = W_["proj_bn_b"][c]
        bn3_g[p, 0] = W_["bn3_g"][c]; bn3_b[p, 0] = W_["bn3_b"][c]
        for t in range(4):
            bn1_g[p, t] = W_["bn1_g"][64 * t + c]; bn1_b[p, t] = W_["bn1_b"][64 * t + c]
            bn2_g[p, t] = W_["bn2_g"][64 * t + c]; bn2_b[p, t] = W_["bn2_b"][64 * t + c]
            ks = 2 * t + 1
            kern = dwk[t][c, 0]
            for ti2, (dy, dx) in enumerate([(a, bb) for a in range(-(ks // 2), ks // 2 + 1)
                                            for bb in range(-(ks // 2), ks // 2 + 1)]):
                dw_w[p, t, ti2] = kern[dy + ks // 2, dx + ks // 2]
            dw_b[p, t] = dwb[t][c]
        for p2 in range(128):
            h2, c2 = p2 // 64, p2 % 64
            if c2 == c:
                selhp[p, p2] = 1.0
            if h2 == h:
                projT[p, p2] = W_["proj_w"][c2, c, 0, 0]
                fc2T[p, :, p2] = [W_["fc2_w"][c2, 64 * t + c, 0, 0] for t in range(4)]
                for t in range(4):
                    fc1T[p, 128 * t + p2] = W_["fc1_w"][64 * t + c2, c, 0, 0]
            if h == 0 and abs(c2 - c) <= 1:
                ecaT[p, p2] = W_["eca_w"][c - c2 + 1]
    rowm_all = np.zeros((128, 4, 14), np.float32)
    for q in range(4):
        for p in range(128):
            h = p // 64
            for r in range(14):
                g = 16 * q - 3 + 8 * h + r
                if 0 <= g < H:
                    rowm_all[p, q, r] = 1.0
    bmask_all = np.zeros((128, 2, 8), np.float32)
    for b in range(2):
        bmask_all[:, b, 4 * b:4 * b + 4] = 1.0
    return dict(selsum=selsum, selhp=selhp, sg_ssm=sg_ssm, sb_ssm=sb_ssm,
                sg_ln2=sg_ln2, sb_ln2=sb_ln2, projT=projT, projb=projb,
                bnp_g=bnp_g, bnp_b=bnp_b, ecaT=ecaT, fc1T=fc1T,
                bn1_g=bn1_g, bn1_b=bn1_b, dw_w=dw_w, dw_b=dw_b,
                bn2_g=bn2_g, bn2_b=bn2_b, fc2T=fc2T, bn3_g=bn3_g, bn3_b=bn3_b,
                cst=cst, rowm_all=rowm_all.reshape(128, 56),
                bmask_all=bmask_all.reshape(128, 16))


def l3_inputs(y_by_core, x):
    """y_by_core: list of 8 arrays (32, 4096) from L2 (core -> (b=i//4, k=i%4))."""
    yimg = {}
    for b in range(B):
        for k in range(4):
            yk = y_by_core[4 * b + k]
            if k % 2 == 1:
                yk = yk[:, ::-1]
            inv = INV_R if k < 2 else INV_C
            yimg[(b, k)] = yk[:, inv].reshape(D2, H, W)
    maps = []
    for i in range(NCORE):
        b, q = i // 4, i % 4
        r0 = 16 * q
        lo, hi = r0 - 3, r0 + 19
        slo, shi = max(lo, 0), min(hi, H)
        y4 = np.zeros((4, D2, 22, W), np.float32)
        for k in range(4):
            y4[k, :, slo - lo:shi - lo, :] = yimg[(b, k)][:, slo:shi, :]
        xs = np.zeros((C, 22, W), np.float32)
        xs[:, slo - lo:shi - lo, :] = x[b, :, slo:shi, :]
        maps.append(dict(y4=y4, xs=xs, oh=core_oh(i)))
    return maps


def l3_gather(results):
    o = np.zeros((B, C, H, W), np.float32)
    for i in range(NCORE):
        b, q = i // 4, i % 4
        o[b, :, 16 * q:16 * q + 16, :] = results[i]["out"]
    return o

# ================================================================ FUSED
# One launch: L1 -> AllGather(h, batch group) -> snake -> L2 -> unsnake
# -> AllGather(y, batch group) -> L3. Inputs per core: xs (22-row halo
# window of x) + oh. All weights are NEFF consts.

L3_CSHAPES = [("rowm_all", [128, 56]), ("bmask_all", [128, 16]),
              ("selsum", [128, 2]), ("selhp", [128, 128]),
              ("sg_ssm", [2, 128]), ("sb_ssm", [128, 1]),
              ("sg_ln2", [2, 128]), ("sb_ln2", [128, 1]),
              ("projT", [128, 128]), ("projb", [128, 1]),
              ("bnp_g", [128, 1]), ("bnp_b", [128, 1]),
              ("ecaT", [128, 128]),
              ("fc1T", [128, 512]),
              ("bn1_g", [128, 4]), ("bn1_b", [128, 4]),
              ("dw_w", [128, 4, 49]), ("dw_b", [128, 4]),
              ("bn2_g", [128, 4]), ("bn2_b", [128, 4]),
              ("fc2T", [128, 4, 128]),
              ("bn3_g", [128, 1]), ("bn3_b", [128, 1]),
              ("cst", [128, 2])]


BF16 = mybir.dt.bfloat16


def build_fused(cw1, cw2, cw3):
    nc = _new_bass()
    xs = nc.dram_tensor("xs", [C, 22, W], BF16, kind="ExternalInput")
    oh = nc.dram_tensor("oh", [128, 8], F32, kind="ExternalInput")
    out = nc.dram_tensor("out", [C, 16, W], BF16, kind="ExternalOutput")
    C1 = {nm: _const(nc, f"c1_{nm}", v) for nm, v in cw1.items()}
    C2 = {nm: _const(nc, f"c2_{nm}", v) for nm, v in cw2.items()}
    C3 = {nm: _const(nc, f"c3_{nm}", cw3[nm].reshape(shp)) for nm, shp in L3_CSHAPES}
    hsd = nc.dram_tensor("hsd", [C, 16, W], F32)
    hgat = nc.dram_tensor("hgat", [4 * C, 16, W], F32)
    yod = nc.dram_tensor("yod", [D2, L], F32)
    ygat = nc.dram_tensor("ygat", [4 * D2, L], F32)
    cc_in = [nc.dram_tensor(f"ccin{r}", [128, 16], F32) for r in range(4)]
    cc_out = [nc.dram_tensor(f"ccout{r}", [NCORE * 128, 16], F32) for r in range(4)]
    GRP4 = [[0, 1, 2, 3], [4, 5, 6, 7]]

    with tile.TileContext(nc, linearize=LINEARIZE) as tc, ExitStack() as ctx:
        pp = ctx.enter_context(tc.tile_pool(name="pp", bufs=1))
        oh_t = pp.tile([128, 8], F32)
        nc.sync.dma_start(oh_t[:], oh[:])
        xsb = pp.tile([128, 22, W], BF16, name="xsb")
        for h in (0, 1):
            nc.sync.dma_start(xsb[64 * h:64 * h + 64, :, :], xs[:])
        ut = pp.tile([D2, L], F32, name="ut")

        # ---------------- P1: ln1 + depthwise 3x3 dil-2 conv ----------------
        with tc.tile_pool(name="p1", bufs=1) as pool, \
             tc.tile_pool(name="ps1", bufs=1, space="PSUM") as psum:
            xt = pool.tile([128, 12, 68], F32)
            nc.vector.memset(xt[:], 0.0)
            for h in (0, 1):
                nc.vector.tensor_copy(xt[64 * h:64 * h + 64, :, 2:66],
                                      xsb[64 * h:64 * h + 64, 1 + 8 * h:13 + 8 * h, :])
            selsum_t = pool.tile([128, 2], F32)
            selg_t = pool.tile([2, 128], F32)
            b2_t = pool.tile([128, 1], F32)
            w9_t = pool.tile([128, 9], F32)
            cb_t = pool.tile([128, 1], F32)
            mask_all_t = pool.tile([128, 48], F32)
            cst_t = pool.tile([2, 2], F32)
            nc.sync.dma_start(selsum_t[:], C1["selsum"][:])
            nc.sync.dma_start(selg_t[:], C1["selg"][:])
            nc.sync.dma_start(b2_t[:], C1["b2"][:])
            nc.sync.dma_start(w9_t[:], C1["w9"][:])
            nc.sync.dma_start(cb_t[:], C1["cb"][:])
            nc.sync.dma_start(mask_all_t[:], C1["mask_all"][:])
            nc.sync.dma_start(cst_t[:], C1["cst"][:])
            mask_t = ksel(nc, pool, "mask", mask_all_t[:], 128, 12, 4, oh_t[:])

            XW = xt[:, :, 2:66]
            sq = pool.tile([128, 12, 64], F32)
            nc.scalar.activation(sq[:], XW, AF.Square)
            st_x = psum.tile([2, 768], F32)
            st_xx = psum.tile([2, 768], F32)
            for r0, r1 in ((0, 8), (8, 12)):
                nc.tensor.matmul(st_x[:, r0 * 64:r1 * 64], selsum_t[:], xt[:, r0:r1, 2:66])
                nc.tensor.matmul(st_xx[:, r0 * 64:r1 * 64], selsum_t[:], sq[:, r0:r1, :])
            sm = pool.tile([2, 768], F32)
            nc.vector.tensor_scalar(sm[:], st_x[:], 1.0 / 64, None, AL.mult)
            var = pool.tile([2, 768], F32)
            nc.vector.tensor_tensor(var[:], sm[:], sm[:], AL.mult)
            nc.vector.scalar_tensor_tensor(var[:], st_xx[:], 1.0 / 64, var[:], AL.mult, AL.subtract)
            inv = pool.tile([2, 768], F32)
            nc.scalar.activation(inv[:], var[:], AF.Ln, bias=cst_t[:, 0:1])
            nc.scalar.activation(inv[:], inv[:], AF.Exp, scale=-0.5)
            minv = pool.tile([2, 768], F32)
            nc.vector.tensor_tensor(minv[:], sm[:], inv[:], AL.mult)
            sgb = psum.tile([128, 12, 64], F32)
            msgb = psum.tile([128, 12, 64], F32)
            for r0, r1 in ((0, 8), (8, 12)):
                nc.tensor.matmul(sgb[:, r0:r1, :], selg_t[:], inv[:, r0 * 64:r1 * 64])
                nc.tensor.matmul(msgb[:, r0:r1, :], selg_t[:], minv[:, r0 * 64:r1 * 64])
            xn = pool.tile([128, 12, 64], F32)
            nc.vector.tensor_tensor(xn[:], XW, sgb[:], AL.mult)
            nc.vector.scalar_tensor_tensor(xn[:], xn[:], b2_t[:], msgb[:], AL.add, AL.subtract)
            xmp = pool.tile([128, 12, 68], F32)
            nc.vector.memset(xmp[:], 0.0)
            nc.vector.tensor_tensor(xmp[:, :, 2:66], xn[:],
                                    mask_t[:].unsqueeze(2).broadcast_to([128, 12, 64]), AL.mult)
            acc0 = pool.tile([128, 8, 64], F32)
            acc1 = pool.tile([128, 8, 64], F32)
            acc = [acc0, acc1]
            taps = [(dy, dx) for dy in (-2, 0, 2) for dx in (-2, 0, 2)]
            cur = 0
            for ti, (dy, dx) in enumerate(taps):
                src = xmp[:, 2 + dy:10 + dy, 2 + dx:66 + dx]
                if ti == 0:
                    nc.vector.tensor_scalar(acc[0][:], src, w9_t[:, 0:1], cb_t[:], AL.mult, AL.add)
                else:
                    nc.vector.scalar_tensor_tensor(acc[1 - cur][:], src, w9_t[:, ti:ti + 1],
                                                   acc[cur][:], AL.mult, AL.add)
                    cur = 1 - cur
            for h in (0, 1):
                nc.sync.dma_start(hsd[:, 8 * h:8 * h + 8, :], acc[cur][64 * h:64 * h + 64])

        # ---------------- G1: gather h within batch group ----------------
        nc.gpsimd.collective_compute("AllGather", AL.bypass, replica_groups=GRP4,
                                     ins=[hsd[:]], outs=[hgat[:]])

        # ---------------- P2a: build snake-ordered u for own direction ----
        with tc.tile_pool(name="p2a", bufs=1) as pool:
            himg = pool.tile([C, H, W], F32)
            for j in range(4):
                nc.sync.dma_start(himg[:, 16 * j:16 * j + 16, :],
                                  hgat[64 * j:64 * j + 64, :, :])
            hB = pool.tile([D2, H, W], F32)      # second channel half -> parts 0:32
            nc.sync.dma_start(hB[:], himg[32:64, :, :])
            u0 = pool.tile([D2, H, W], F32)
            nc.vector.tensor_copy(u0[:, 0::2, :], himg[0:32, 0::2, :])
            nc.vector.tensor_copy(u0[:, 1::2, :], himg[0:32, 1::2, ::-1])
            hBT = pool.tile([D2, W, H], F32)
            nc.vector.tensor_copy(hBT[:], hB[:].transpose([0, 2, 1]))
            u2 = pool.tile([D2, W, H], F32)
            nc.vector.tensor_copy(u2[:, 0::2, :], hBT[:, 0::2, :])
            nc.vector.tensor_copy(u2[:, 1::2, :], hBT[:, 1::2, ::-1])
            sa = pool.tile([D2, H, W], F32)
            sb = pool.tile([D2, H, W], F32)
            nc.vector.tensor_scalar(sa[:], u0[:], oh_t[0:D2, 0:1], None, AL.mult)
            nc.vector.scalar_tensor_tensor(sb[:], u0[:, ::-1, ::-1], oh_t[0:D2, 1:2],
                                           sa[:], AL.mult, AL.add)
            nc.vector.scalar_tensor_tensor(sa[:], u2[:], oh_t[0:D2, 2:3],
                                           sb[:], AL.mult, AL.add)
            nc.vector.scalar_tensor_tensor(ut[:].rearrange("p (a b) -> p a b", a=H),
                                           u2[:, ::-1, ::-1], oh_t[0:D2, 3:4],
                                           sa[:], AL.mult, AL.add)

        # ---------------- P2b: selective scan (direction-selected weights) -
        NCH = 8
        CH = L // NCH
        with tc.tile_pool(name="p2b", bufs=1) as pool, \
             tc.tile_pool(name="psA", bufs=3, space="PSUM") as psA, \
             tc.tile_pool(name="psY", bufs=1, space="PSUM") as psY:
            lhsT_u4_t = pool.tile([D2, 128], F32)
            lhsT_y_t = pool.tile([128, D2], F32)
            nc.sync.dma_start(lhsT_u4_t[:], C2["u4"][:])
            nc.sync.dma_start(lhsT_y_t[:], C2["y_lhsT"][:])

            def ksel_dram(nm, hd, P_, F_, fc=None):
                outt = pool.tile([P_, F_], F32, name=f"sel_{nm}")
                fc = fc or F_
                for o in range(0, F_, fc):
                    st = pool.tile([P_, 4 * fc], F32, name=f"st_{nm}_{o}",
                                   tag=f"kst_{P_}x{4 * fc}", bufs=2)
                    for k in range(4):
                        nc.sync.dma_start(st[:, k * fc:(k + 1) * fc],
                                          hd[:, k * F_ + o:k * F_ + o + fc])
                    cur2 = None
                    for k in range(4):
                        if k == 3:
                            dst = outt[:, o:o + fc]
                        else:
                            dst = pool.tile([P_, fc], F32, name=f"ks_{nm}_{o}_{k}",
                                            tag=f"kscr_{P_}x{fc}", bufs=2)[:]
                        if k == 0:
                            nc.vector.tensor_scalar(dst, st[:, 0:fc], oh_t[0:P_, 0:1],
                                                    None, AL.mult)
                        else:
                            nc.vector.scalar_tensor_tensor(dst, st[:, k * fc:(k + 1) * fc],
                                                           oh_t[0:P_, k:k + 1], cur2,
                                                           AL.mult, AL.add)
                        cur2 = dst
                return outt

            lhsT_d_t = ksel_dram("d", C2["d_all"], D2, 128)
            bias4_t = ksel_dram("bias", C2["bias_all"], 128, 1)
            lhsT_B_t = ksel_dram("B", C2["B_all"], D2, 512, 128)
            lhsT_C_t = ksel_dram("C", C2["C_all"], D2, 512, 128)
            A4_t = ksel_dram("A", C2["A_all"], 128, 4)
            lhsT_Ds_t = ksel_dram("Ds", C2["Ds_all"], D2, D2)

            def mm(out_ap, lh, rh, f32r, **kw):
                if f32r:
                    lh, rh = lh.bitcast(F32R), rh.bitcast(F32R)
                nc.tensor.matmul(out_ap, lh, rh, **kw)

            d4 = pool.tile([128, L], F32)
            w4 = pool.tile([128, L], F32)
            for c in range(NCH):
                dp = psA.tile([128, CH], F32, name=f"dp{c}", tag="ps")
                mm(dp[:], lhsT_d_t[:], ut[:, c * CH:(c + 1) * CH], F32R_PROJ)
                nc.scalar.activation(d4[:, c * CH:(c + 1) * CH], dp[:], AF.Exp,
                                     bias=bias4_t[:])
                nc.scalar.activation(d4[:, c * CH:(c + 1) * CH],
                                     d4[:, c * CH:(c + 1) * CH], AF.Ln,
                                     bias=nc.const_aps.tensor(1.0, (128, 1)))
            for c in range(NCH):
                u4p = psA.tile([128, CH], F32, name=f"u4p{c}", tag="ps")
                mm(u4p[:], lhsT_u4_t[:], ut[:, c * CH:(c + 1) * CH], F32R_BC)
                nc.vector.tensor_tensor(w4[:, c * CH:(c + 1) * CH],
                                        d4[:, c * CH:(c + 1) * CH], u4p[:], AL.mult)
            hs = []
            for g in range(4):
                dBu = pool.tile([128, L], F32, name=f"dBu{g}", tag="dBu", bufs=1)
                for c in range(NCH):
                    b4 = psA.tile([128, CH], F32, name=f"b4_{g}_{c}", tag="ps")
                    mm(b4[:], lhsT_B_t[:, g * 128:(g + 1) * 128],
                       ut[:, c * CH:(c + 1) * CH], F32R_BC)
                    nc.vector.tensor_tensor(dBu[:, c * CH:(c + 1) * CH],
                                            w4[:, c * CH:(c + 1) * CH], b4[:], AL.mult)
                dA = pool.tile([128, L], F32, name=f"dA{g}", tag="dA", bufs=2)
                nc.scalar.activation(dA[:], d4[:], AF.Exp, scale=A4_t[:, g:g + 1])
                hsg = pool.tile([128, L], F32, name=f"hs{g}")
                nc.vector.tensor_tensor_scan(hsg[:], dA[:], dBu[:], 0.0, AL.mult, AL.add)
                hs.append(hsg)
            for half in range(2):
                yps = psY.tile([D2, L // 2], F32, name=f"yps{half}", tag="yps")
                for g in range(4):
                    fsb = pool.tile([128, L // 2], F32, name=f"f_{half}_{g}", tag="fsb", bufs=2)
                    for cc in range(NCH // 2):
                        c = half * (NCH // 2) + cc
                        c4 = psA.tile([128, CH], F32, name=f"c4_{g}_{c}", tag="ps")
                        mm(c4[:], lhsT_C_t[:, g * 128:(g + 1) * 128],
                           ut[:, c * CH:(c + 1) * CH], F32R_BC)
                        if g % 2 == 0:
                            nc.vector.tensor_tensor(fsb[:, cc * CH:(cc + 1) * CH],
                                                    hs[g][:, c * CH:(c + 1) * CH], c4[:], AL.mult)
                        else:
                            c4sb = pool.tile([128, CH], F32, name=f"c4sb_{g}_{c}", tag="c4sb", bufs=2)
                            nc.scalar.copy(c4sb[:], c4[:])
                            nc.vector.tensor_tensor(fsb[:, cc * CH:(cc + 1) * CH],
                                                    hs[g][:, c * CH:(c + 1) * CH], c4sb[:], AL.mult)
                    for cc in range(NCH // 2):
                        mm(yps[:, cc * CH:(cc + 1) * CH], lhsT_y_t[:],
                           fsb[:, cc * CH:(cc + 1) * CH], F32R_Y,
                           start=(g == 0), stop=False, skip_group_check=True)
                for cc in range(NCH // 2):
                    c = half * (NCH // 2) + cc
                    mm(yps[:, cc * CH:(cc + 1) * CH], lhsT_Ds_t[:],
                       ut[:, c * CH:(c + 1) * CH], F32R_Y,
                       start=False, stop=True, skip_group_check=True)
                ysb = pool.tile([D2, L // 2], F32, name=f"ysb{half}", tag="ysb", bufs=1)
                nc.scalar.copy(ysb[:], yps[:])
                nc.sync.dma_start(yod[:, half * (L // 2):(half + 1) * (L // 2)], ysb[:])

        # ---------------- G2: gather y within batch group ----------------
        nc.gpsimd.collective_compute("AllGather", AL.bypass, replica_groups=GRP4,
                                     ins=[yod[:]], outs=[ygat[:]])

        # ---------------- P3: merge + LN + proj + eca + MLP ---------------
        RW = 70
        OWN = (slice(3, 11), slice(3, 67))
        with tc.tile_pool(name="p3", bufs=1) as pool, \
             tc.tile_pool(name="psT", bufs=2, space="PSUM") as psT, \
             tc.tile_pool(name="psS", bufs=2, space="PSUM") as psS:
            T = {}
            for nm, shp in L3_CSHAPES:
                T[nm] = pool.tile(shp, F32, name=f"t_{nm}")
                nc.sync.dma_start(T[nm][:], C3[nm][:])
            eps_ap = T["cst"][:, 0:1]
            rowm = ksel(nc, pool, "rowm", T["rowm_all"][:], 128, 14, 4, oh_t[:])
            bmask_t = ksel(nc, pool, "bmask", T["bmask_all"][:], 128, 8, 2, oh_t[:, 4:6])

            # unsnake the 4 gathered direction outputs into padded images
            ygi = pool.tile([128, 70, W], F32)
            nc.vector.memset(ygi[:], 0.0)
            with tc.tile_pool(name="p3u", bufs=1) as pu:
                ysn = pu.tile([128, H, W], F32)
                nc.sync.dma_start(ysn[:], ygat[:])
                wsc = pu.tile([128, W, H], F32)
                # k=0 rows even/odd
                nc.vector.tensor_copy(ygi[0:32, 3:66:2, :], ysn[0:32, 0::2, :])
                nc.vector.tensor_copy(ygi[0:32, 4:67:2, :], ysn[0:32, 1::2, ::-1])
                # k=1: reverse whole seq then row-unsnake
                y1v = ysn[32:64, ::-1, ::-1]
                nc.vector.tensor_copy(ygi[32:64, 3:66:2, :], y1v[:, 0::2, :])
                nc.vector.tensor_copy(ygi[32:64, 4:67:2, :], y1v[:, 1::2, ::-1])
                # k=2: col-grid row-unsnake then transpose
                nc.vector.tensor_copy(wsc[64:96, 0::2, :], ysn[64:96, 0::2, :])
                nc.vector.tensor_copy(wsc[64:96, 1::2, :], ysn[64:96, 1::2, ::-1])
                nc.vector.tensor_copy(ygi[64:96, 3:67, :], wsc[64:96].transpose([0, 2, 1]))
                # k=3: reversed col-grid
                y3v = ysn[96:128, ::-1, ::-1]
                nc.vector.tensor_copy(wsc[96:128, 0::2, :], y3v[:, 0::2, :])
                nc.vector.tensor_copy(wsc[96:128, 1::2, :], y3v[:, 1::2, ::-1])
                nc.vector.tensor_copy(ygi[96:128, 3:67, :], wsc[96:128].transpose([0, 2, 1]))

            # q-variant window select -> ym
            ym = pool.tile([128, 14, RW], F32, name="ym")
            with tc.tile_pool(name="p3v", bufs=1) as pv:
                scur = None
                for qv in range(4):
                    ya = pv.tile([128, 14, RW], F32, name=f"ya{qv}", tag="yaq", bufs=2)
                    yb = pv.tile([128, 14, RW], F32, name=f"yb{qv}", tag="ybq", bufs=2)
                    nc.vector.memset(ya[:], 0.0)
                    nc.vector.memset(yb[:], 0.0)
                    for h in (0, 1):
                        for m, dsttile in ((0, ya), (1, yb), (2, ya), (3, yb)):
                            p0 = 64 * h + 32 * (m // 2)
                            r0 = 16 * qv + 8 * h
                            nc.sync.dma_start(dsttile[p0:p0 + 32, :, 3:67],
                                              ygi[32 * m:32 * m + 32, r0:r0 + 14, :])
                    ymq = pv.tile([128, 14, RW], F32, name=f"ymq{qv}", tag="ymq", bufs=2)
                    nc.vector.tensor_tensor(ymq[:], ya[:], yb[:], AL.add)
                    if qv == 0:
                        s0 = pv.tile([128, 14, RW], F32, name="ysel0", tag="ysel", bufs=2)
                        nc.vector.tensor_scalar(s0[:], ymq[:], oh_t[:, 0:1], None, AL.mult)
                        scur = s0
                    else:
                        dst = ym if qv == 3 else pv.tile([128, 14, RW], F32,
                                                         name=f"ysel{qv}", tag="ysel", bufs=2)
                        nc.vector.scalar_tensor_tensor(dst[:], ymq[:], oh_t[:, qv:qv + 1],
                                                       scur[:], AL.mult, AL.add)
                        scur = dst

            xt = pool.tile([128, 14, RW], F32)
            nc.vector.memset(xt[:], 0.0)
            for h in (0, 1):
                nc.vector.tensor_copy(xt[64 * h:64 * h + 64, :, 3:67],
                                      xsb[64 * h:64 * h + 64, 8 * h:8 * h + 14, :])

            def chunks2(tile3, rows=14):
                return [tile3[:, 0:7, :], tile3[:, 7:14, :]]

            def ln_ch(src, selg_key, b_key, nm):
                sq = pool.tile([128, 14, RW], F32, name=f"sq_{nm}", tag="lnsq")
                nc.scalar.activation(sq[:], src[:], AF.Square)
                inv = pool.tile([2, 14, RW], F32, name=f"inv_{nm}", tag="lninv")
                minv = pool.tile([2, 14, RW], F32, name=f"minv_{nm}", tag="lnminv")
                for ci, (s_ap, q_ap) in enumerate(zip(chunks2(src), chunks2(sq))):
                    px = psS.tile([2, 7 * RW], F32, name=f"px_{nm}{ci}", tag="lnst")
                    pq = psS.tile([2, 7 * RW], F32, name=f"pq_{nm}{ci}", tag="lnst")
                    nc.tensor.matmul(px[:], T["selsum"][:], s_ap)
                    nc.tensor.matmul(pq[:], T["selsum"][:], q_ap)
                    ivc = inv[:, 7 * ci:7 * ci + 7, :]
                    mvc = minv[:, 7 * ci:7 * ci + 7, :]
                    sm = pool.tile([2, 7, RW], F32, name=f"sm_{nm}{ci}", tag="lnsm")
                    nc.vector.tensor_scalar(sm[:], px[:], 1.0 / 64, None, AL.mult)
                    nc.vector.tensor_tensor(ivc, sm[:], sm[:], AL.mult)
                    nc.vector.scalar_tensor_tensor(ivc, pq[:], 1.0 / 64, ivc, AL.mult, AL.subtract)
                    nc.scalar.activation(ivc, ivc, AF.Ln, bias=T["cst"][0:2, 0:1])
                    nc.scalar.activation(ivc, ivc, AF.Exp, scale=-0.5)
                    nc.vector.tensor_tensor(mvc, sm[:], ivc, AL.mult)
                dst = pool.tile([128, 14, RW], F32, name=f"ln_{nm}")
                for ci in range(2):
                    rs = slice(7 * ci, 7 * ci + 7)
                    sgb = psS.tile([128, 7 * RW], F32, name=f"sgb_{nm}{ci}", tag="lnbc")
                    msgb = psS.tile([128, 7 * RW], F32, name=f"msgb_{nm}{ci}", tag="lnbc")
                    nc.tensor.matmul(sgb[:], T[selg_key][:], inv[:, rs, :])
                    nc.tensor.matmul(msgb[:], T[selg_key][:], minv[:, rs, :])
                    nc.vector.tensor_tensor(dst[:, rs, :], src[:, rs, :],
                                            sgb[:].rearrange("p (a b) -> p a b", a=7), AL.mult)
                    nc.vector.scalar_tensor_tensor(dst[:, rs, :], dst[:, rs, :], T[b_key][:],
                                                   msgb[:].rearrange("p (a b) -> p a b", a=7),
                                                   AL.add, AL.subtract)
                return dst

            def allgather(rnd, cols_src_ap, ncols):
                ci = pool.tile([128, 16], F32, name=f"cci_{rnd}", tag="cci")
                nc.vector.memset(ci[:], 0.0)
                nc.vector.tensor_copy(ci[:, 0:ncols], cols_src_ap)
                nc.sync.dma_start(cc_in[rnd][:], ci[:])
                nc.gpsimd.collective_compute(
                    "AllGather", AL.bypass, replica_groups=[list(range(NCORE))],
                    ins=[cc_in[rnd][:]], outs=[cc_out[rnd][:]])
                gat = pool.tile([128, 16, NCORE], F32, name=f"gat_{rnd}", tag="gat")
                src = cc_out[rnd][:].rearrange("(n p) c -> p c n", p=128)
                nc.sync.dma_start(gat[:, 0:16, :], src)
                return gat

            z1 = ln_ch(ym, "sg_ssm", "sb_ssm", "ssm")
            z2 = ln_ch(z1, "sg_ln2", "sb_ln2", "ln2a")
            pr = pool.tile([128, 14, RW], F32)
            for ci, z_ap in enumerate(chunks2(z2)):
                pp2 = psT.tile([128, 7 * RW], F32, name=f"pp{ci}", tag="ps1")
                nc.tensor.matmul(pp2[:], T["projT"][:], z_ap)
                nc.scalar.activation(pr[:, 7 * ci:7 * ci + 7, :],
                                     pp2[:].rearrange("p (a b) -> p a b", a=7),
                                     AF.Relu, bias=T["projb"][:])
            prow = pr[:, OWN[0], OWN[1]]
            sqs = pool.tile([128, 8, 64], F32, name="sqs", tag="sqscratch")
            part0 = pool.tile([128, 3], F32)
            nc.vector.tensor_reduce(part0[:, 0:1], prow, mybir.AxisListType.XY, AL.add)
            nc.scalar.activation(sqs[:], prow, AF.Square, accum_out=part0[:, 1:2])
            nc.vector.tensor_copy(part0[:, 2:3], part0[:, 0:1])
            gat0 = allgather(0, part0[:], 3)
            red0 = pool.tile([128, 4], F32)
            nc.vector.tensor_reduce(red0[:, 0:2], gat0[:, 0:2, :], mybir.AxisListType.X, AL.add)
            pm = pool.tile([128, 16, NCORE], F32, name="pm", tag="pmx")
            nc.vector.tensor_tensor(pm[:, 2:3, :], gat0[:, 2:3, :],
                                    bmask_t[:].unsqueeze(1), AL.mult)
            nc.vector.tensor_reduce(red0[:, 2:3], pm[:, 2:3, :], mybir.AxisListType.X, AL.add)
            stat0 = psS.tile([128, 4], F32, name="stat0", tag="lnst")
            nc.tensor.matmul(stat0[:, 0:3], T["selhp"][:], red0[:, 0:3])
            mS = pool.tile([128, 6], F32)
            nc.vector.tensor_scalar(mS[:, 0:1], stat0[:, 0:1], 1.0 / NPIX, None, AL.mult)
            nc.vector.tensor_tensor(mS[:, 1:2], mS[:, 0:1], mS[:, 0:1], AL.mult)
            nc.vector.scalar_tensor_tensor(mS[:, 1:2], stat0[:, 1:2], 1.0 / NPIX, mS[:, 1:2],
                                           AL.mult, AL.subtract)
            nc.scalar.activation(mS[:, 1:2], mS[:, 1:2], AF.Ln, bias=eps_ap)
            nc.scalar.activation(mS[:, 1:2], mS[:, 1:2], AF.Exp, scale=-0.5)
            nc.vector.tensor_tensor(mS[:, 1:2], mS[:, 1:2], T["bnp_g"][:], AL.mult)
            nc.vector.tensor_tensor(mS[:, 2:3], mS[:, 0:1], mS[:, 1:2], AL.mult)
            nc.vector.scalar_tensor_tensor(mS[:, 2:3], T["bnp_b"][:], 1.0, mS[:, 2:3],
                                           AL.mult, AL.subtract)
            nc.vector.tensor_scalar(mS[:, 3:4], stat0[:, 2:3], 1.0 / NPB, None, AL.mult)
            nc.vector.tensor_tensor(mS[:, 3:4], mS[:, 3:4], mS[:, 1:2], AL.mult)
            nc.vector.tensor_tensor(mS[:, 3:4], mS[:, 3:4], mS[:, 2:3], AL.add)
            ecp = psS.tile([128, 1], F32, name="ecp", tag="lnst")
            nc.tensor.matmul(ecp[:], T["ecaT"][:], mS[:, 3:4])
            sg = pool.tile([128, 2], F32)
            nc.scalar.activation(sg[:, 0:1], ecp[:], AF.Exp, scale=-1.0)
            nc.vector.tensor_scalar(sg[:, 0:1], sg[:, 0:1], 1.0, None, AL.add)
            nc.vector.reciprocal(sg[:, 1:2], sg[:, 0:1])
            x1 = pool.tile([128, 14, RW], F32)
            nc.vector.tensor_scalar(x1[:], pr[:], mS[:, 1:2], mS[:, 2:3], AL.mult, AL.add)
            nc.vector.scalar_tensor_tensor(x1[:], x1[:], sg[:, 1:2], xt[:], AL.mult, AL.add)

            m2 = ln_ch(x1, "sg_ln2", "sb_ln2", "ln2b")
            hm = []
            for t_i in range(4):
                hmt = pool.tile([128, 14, RW], F32, name=f"hm{t_i}")
                for ci, m_ap in enumerate(chunks2(m2)):
                    fp = psT.tile([128, 7 * RW], F32, name=f"fp{t_i}{ci}", tag="ps1")
                    nc.tensor.matmul(fp[:], T["fc1T"][:, 128 * t_i:128 * t_i + 128], m_ap)
                    nc.scalar.activation(hmt[:, 7 * ci:7 * ci + 7, :],
                                         fp[:].rearrange("p (a b) -> p a b", a=7), AF.Relu)
                hm.append(hmt)
            part1 = pool.tile([128, 8], F32)
            for t_i in range(4):
                nc.vector.tensor_reduce(part1[:, 2 * t_i:2 * t_i + 1], hm[t_i][:, OWN[0], OWN[1]],
                                        mybir.AxisListType.XY, AL.add)
                nc.scalar.activation(sqs[:], hm[t_i][:, OWN[0], OWN[1]], AF.Square,
                                     accum_out=part1[:, 2 * t_i + 1:2 * t_i + 2])
            gat1 = allgather(1, part1[:], 8)
            red1 = pool.tile([128, 8], F32)
            nc.vector.tensor_reduce(red1[:], gat1[:, 0:8, :], mybir.AxisListType.X, AL.add)
            stat1 = psS.tile([128, 8], F32, name="stat1", tag="lnst")
            nc.tensor.matmul(stat1[:], T["selhp"][:], red1[:])
            S1 = pool.tile([128, 4], F32)
            T1 = pool.tile([128, 4], F32)
            for t_i in range(4):
                a, bcol = stat1[:, 2 * t_i:2 * t_i + 1], stat1[:, 2 * t_i + 1:2 * t_i + 2]
                mcol = pool.tile([128, 2], F32, name=f"mcol{t_i}", tag="mcol")
                nc.vector.tensor_scalar(mcol[:, 0:1], a, 1.0 / NPIX, None, AL.mult)
                nc.vector.tensor_tensor(mcol[:, 1:2], mcol[:, 0:1], mcol[:, 0:1], AL.mult)
                nc.vector.scalar_tensor_tensor(mcol[:, 1:2], bcol, 1.0 / NPIX, mcol[:, 1:2],
                                               AL.mult, AL.subtract)
                nc.scalar.activation(mcol[:, 1:2], mcol[:, 1:2], AF.Ln, bias=eps_ap)
                nc.scalar.activation(mcol[:, 1:2], mcol[:, 1:2], AF.Exp, scale=-0.5)
                nc.vector.tensor_tensor(S1[:, t_i:t_i + 1], mcol[:, 1:2],
                                        T["bn1_g"][:, t_i:t_i + 1], AL.mult)
                nc.vector.tensor_tensor(mcol[:, 0:1], mcol[:, 0:1], S1[:, t_i:t_i + 1], AL.mult)
                nc.vector.scalar_tensor_tensor(T1[:, t_i:t_i + 1], T["bn1_b"][:, t_i:t_i + 1],
                                               1.0, mcol[:, 0:1], AL.mult, AL.subtract)
            for t_i in range(4):
                nc.vector.tensor_scalar(hm[t_i][:], hm[t_i][:], S1[:, t_i:t_i + 1],
                                        T1[:, t_i:t_i + 1], AL.mult, AL.add)
                nc.vector.tensor_tensor(hm[t_i][:, :, 3:67], hm[t_i][:, :, 3:67],
                                        rowm[:].unsqueeze(2).broadcast_to([128, 14, 64]),
                                        AL.mult)
                nc.vector.memset(hm[t_i][:, :, 0:3], 0.0)
                nc.vector.memset(hm[t_i][:, :, 67:70], 0.0)

            KS = [1, 3, 5, 7]
            part2 = pool.tile([128, 8], F32)
            r2 = []
            for t_i, ks in enumerate(KS):
                pad = ks // 2
                taps = [(dy, dx) for dy in range(-pad, pad + 1) for dx in range(-pad, pad + 1)]
                acc0 = pool.tile([128, 8, 64], F32, name=f"dacc0_{t_i}", tag="dacc0")
                acc1 = pool.tile([128, 8, 64], F32, name=f"dacc1_{t_i}", tag="dacc1")
                accs = [acc0, acc1]
                cur = 0
                for ti2, (dy, dx) in enumerate(taps):
                    src = hm[t_i][:, 3 + dy:11 + dy, 3 + dx:67 + dx]
                    wap = T["dw_w"][:, t_i, ti2:ti2 + 1]
                    if ti2 == 0:
                        nc.vector.scalar_tensor_tensor(accs[0][:], src, wap,
                                                       hm[t_i][:, OWN[0], OWN[1]], AL.mult, AL.add)
                    else:
                        nc.vector.scalar_tensor_tensor(accs[1 - cur][:], src, wap, accs[cur][:],
                                                       AL.mult, AL.add)
                        cur = 1 - cur
                r2t = pool.tile([128, 8, 64], F32, name=f"r2_{t_i}")
                nc.scalar.activation(r2t[:], accs[cur][:], AF.Relu,
                                     bias=T["dw_b"][:, t_i:t_i + 1],
                                     accum_out=part2[:, 2 * t_i:2 * t_i + 1])
                nc.scalar.activation(sqs[:], r2t[:], AF.Square,
                                     accum_out=part2[:, 2 * t_i + 1:2 * t_i + 2])
                r2.append(r2t)
            gat2 = allgather(2, part2[:], 8)
            red2 = pool.tile([128, 8], F32)
            nc.vector.tensor_reduce(red2[:], gat2[:, 0:8, :], mybir.AxisListType.X, AL.add)
            stat2 = psS.tile([128, 8], F32, name="stat2", tag="lnst")
            nc.tensor.matmul(stat2[:], T["selhp"][:], red2[:])
            S2 = pool.tile([128, 4], F32)
            T2 = pool.tile([128, 4], F32)
            for t_i in range(4):
                a, bcol = stat2[:, 2 * t_i:2 * t_i + 1], stat2[:, 2 * t_i + 1:2 * t_i + 2]
                mcol = pool.tile([128, 2], F32, name=f"m2col{t_i}", tag="mcol")
                nc.vector.tensor_scalar(mcol[:, 0:1], a, 1.0 / NPIX, None, AL.mult)
                nc.vector.tensor_tensor(mcol[:, 1:2], mcol[:, 0:1], mcol[:, 0:1], AL.mult)
                nc.vector.scalar_tensor_tensor(mcol[:, 1:2], bcol, 1.0 / NPIX, mcol[:, 1:2],
                                               AL.mult, AL.subtract)
                nc.scalar.activation(mcol[:, 1:2], mcol[:, 1:2], AF.Ln, bias=eps_ap)
                nc.scalar.activation(mcol[:, 1:2], mcol[:, 1:2], AF.Exp, scale=-0.5)
                nc.vector.tensor_tensor(S2[:, t_i:t_i + 1], mcol[:, 1:2],
                                        T["bn2_g"][:, t_i:t_i + 1], AL.mult)
                nc.vector.tensor_tensor(mcol[:, 0:1], mcol[:, 0:1], S2[:, t_i:t_i + 1], AL.mult)
                nc.vector.scalar_tensor_tensor(T2[:, t_i:t_i + 1], T["bn2_b"][:, t_i:t_i + 1],
                                               1.0, mcol[:, 0:1], AL.mult, AL.subtract)
            fo = psT.tile([128, 8, 64], F32, name="fo", tag="dwps")
            for t_i in range(4):
                zt = pool.tile([128, 8, 64], F32, name=f"zt{t_i}", tag="zt", bufs=2)
                nc.vector.tensor_scalar(zt[:], r2[t_i][:], S2[:, t_i:t_i + 1],
                                        T2[:, t_i:t_i + 1], AL.mult, AL.add)
                nc.tensor.matmul(fo[:], T["fc2T"][:, t_i, :], zt[:],
                                 start=(t_i == 0), stop=(t_i == 3), skip_group_check=True)
            fo_sb = pool.tile([128, 8, 64], F32)
            part3 = pool.tile([128, 8], F32)
            nc.scalar.activation(fo_sb[:], fo[:], AF.Copy, accum_out=part3[:, 0:1])
            nc.scalar.activation(sqs[:], fo_sb[:], AF.Square, accum_out=part3[:, 1:2])
            gat3 = allgather(3, part3[:, 0:2], 2)
            red3 = pool.tile([128, 2], F32)
            nc.vector.tensor_reduce(red3[:], gat3[:, 0:2, :], mybir.AxisListType.X, AL.add)
            stat3 = psS.tile([128, 2], F32, name="stat3", tag="lnst")
            nc.tensor.matmul(stat3[:], T["selhp"][:], red3[:])
            mS3 = pool.tile([128, 3], F32)
            nc.vector.tensor_scalar(mS3[:, 0:1], stat3[:, 0:1], 1.0 / NPIX, None, AL.mult)
            nc.vector.tensor_tensor(mS3[:, 1:2], mS3[:, 0:1], mS3[:, 0:1], AL.mult)
            nc.vector.scalar_tensor_tensor(mS3[:, 1:2], stat3[:, 1:2], 1.0 / NPIX, mS3[:, 1:2],
                                           AL.mult, AL.subtract)
            nc.scalar.activation(mS3[:, 1:2], mS3[:, 1:2], AF.Ln, bias=eps_ap)
            nc.scalar.activation(mS3[:, 1:2], mS3[:, 1:2], AF.Exp, scale=-0.5)
            nc.vector.tensor_tensor(mS3[:, 1:2], mS3[:, 1:2], T["bn3_g"][:], AL.mult)
            nc.vector.tensor_tensor(mS3[:, 2:3], mS3[:, 0:1], mS3[:, 1:2], AL.mult)
            nc.vector.scalar_tensor_tensor(mS3[:, 2:3], T["bn3_b"][:], 1.0, mS3[:, 2:3],
                                           AL.mult, AL.subtract)
            fin = pool.tile([128, 8, 64], F32)
            nc.vector.tensor_scalar(fin[:], fo_sb[:], mS3[:, 1:2], mS3[:, 2:3], AL.mult, AL.add)
            nc.vector.tensor_tensor(fin[:], fin[:], x1[:, OWN[0], OWN[1]], AL.add)
            fin16 = pool.tile([128, 8, 64], BF16)
            nc.vector.tensor_copy(fin16[:], fin[:])
            for h in (0, 1):
                nc.sync.dma_start(out[:, 8 * h:8 * h + 8, :], fin16[64 * h:64 * h + 64])
    _fix_multiwaits(nc)
    return nc


def fused_inputs(x):
    import ml_dtypes
    maps = []
    for i in range(NCORE):
        b, q = i // 4, i % 4
        r0 = 16 * q
        lo, hi = r0 - 3, r0 + 19
        slo, shi = max(lo, 0), min(hi, H)
        xarr = np.zeros((C, 22, W), np.float32)
        xarr[:, slo - lo:shi - lo, :] = x[b, :, slo:shi, :]
        maps.append(dict(xs=xarr.astype(ml_dtypes.bfloat16), oh=core_oh(i)))
    return maps


# ================================================================ kernel()
_PROGS = {}
TRACE = False            # set True to collect per-launch NTFF exec times
LAST_EXEC_NS = []        # filled per kernel() call when TRACE

WEIGHT_NAMES = [
    "ln1_g", "ln1_b", "ln2_g", "ln2_b", "con1_w", "con1_b",
    "xproj_w", "dtproj_w", "dtproj_b", "A_log", "Ds", "ssm_ln_g", "ssm_ln_b",
    "proj_w", "proj_b", "proj_bn_g", "proj_bn_b", "eca_w",
    "fc1_w", "bn1_g", "bn1_b",
    "dw_w1", "dw_b1", "dw_w3", "dw_b3", "dw_w5", "dw_b5", "dw_w7", "dw_b7",
    "bn2_g", "bn2_b", "fc2_w", "bn3_g", "bn3_b"]


FUSED = not os.environ.get("KERNEL_3L")


def _programs(W_):
    import hashlib
    hsh = hashlib.sha1()
    for nm in WEIGHT_NAMES:
        hsh.update(_f(W_[nm]).tobytes())
    key = hsh.hexdigest()
    if key not in _PROGS:
        _PROGS.clear()
        cw1 = l1_consts(W_["ln1_g"], W_["ln1_b"], W_["con1_w"], W_["con1_b"])
        cw2 = l2_consts(W_["xproj_w"], W_["dtproj_w"], W_["dtproj_b"],
                        W_["A_log"], W_["Ds"])
        cw3 = l3_consts(W_)
        if FUSED:
            _PROGS[key] = dict(fused=build_fused(cw1, cw2, cw3))
        else:
            _PROGS[key] = dict(l1=build_l1(cw1), l2=build_l2(cw2), l3=build_l3(cw3))
    return _PROGS[key]

_RUNNERS = {}


def _fast_runner(nc, n_cores=NCORE):
    """Cached jit + device-resident output operand buffers for one program.
    run_bass_via_pjrt rebuilds its jit closure (full retrace) and re-ships
    donated zero output buffers on every call; this does both once."""
    rs = _RUNNERS.get(id(nc))
    if rs is not None:
        return rs
    import jax
    import concourse.mybir as _mb
    from jax.experimental.shard_map import shard_map
    from jax.sharding import Mesh, PartitionSpec
    from concourse import bass2jax
    bass2jax.install_neuronx_cc_hook()
    assert nc.dbg_addr is None
    part_name = nc.partition_id_tensor.name if nc.partition_id_tensor else None
    in_names, out_names, out_avals = [], [], []
    for alloc in nc.m.functions[0].allocations:
        if not isinstance(alloc, _mb.MemoryLocationSet):
            continue
        name = alloc.memorylocations[0].name if alloc.memorylocations else None
        if alloc.kind == "ExternalInput":
            if name != part_name:
                in_names.append(name)
        elif alloc.kind == "ExternalOutput":
            out_names.append(name)
            out_avals.append(jax.core.ShapedArray(tuple(alloc.tensor_shape),
                                                  _mb.dt.np(alloc.dtype)))
    n_params = len(in_names)
    all_names = list(in_names) + list(out_names)
    if part_name is not None:
        all_names.append(part_name)
    all_names = tuple(all_names)

    def _body(*args):
        operands = list(args)
        if part_name is not None:
            operands.append(bass2jax.partition_id_tensor())
        outs = bass2jax._bass_exec_p.bind(
            *operands, out_avals=tuple(out_avals), in_names=all_names,
            out_names=tuple(out_names), lowering_input_output_aliases=(),
            sim_require_finite=True, sim_require_nnan=True, nc=nc)
        return tuple(outs)

    devices = jax.devices()[:n_cores]
    mesh = Mesh(np.asarray(devices), ("core",))
    spec = (PartitionSpec("core"),)
    fn = jax.jit(shard_map(_body, mesh=mesh,
                           in_specs=spec * (n_params + len(out_names)),
                           out_specs=spec * len(out_names), check_rep=False),
                 keep_unused=True)
    # outputs are fully written by the kernels, so skip donation and keep the
    # operand buffers device-resident across calls (no re-transfer).
    out_bufs = [
        jax.device_put(
            np.zeros((n_cores * av.shape[0], *av.shape[1:]), av.dtype),
            jax.sharding.NamedSharding(mesh, PartitionSpec("core")))
        for av in out_avals]
    rs = dict(fn=fn, in_names=in_names, out_names=out_names,
              out_avals=out_avals, out_bufs=out_bufs, n_cores=n_cores)
    _RUNNERS[id(nc)] = rs
    return rs


def _precat(nc, maps):
    """Host-side input marshaling (outside the timed launch)."""
    rs = _fast_runner(nc)
    return [np.concatenate([np.asarray(m[nm]) for m in maps], axis=0)
            for nm in rs["in_names"]]


def _run_fast(nc, maps, concat_in=None):
    rs = _fast_runner(nc)
    n = rs["n_cores"]
    if concat_in is None:
        concat_in = [np.concatenate([np.asarray(m[nm]) for m in maps], axis=0)
                     for nm in rs["in_names"]]
    outs = rs["fn"](*concat_in, *rs["out_bufs"])
    res = []
    for c in range(n):
        res.append({nm: np.asarray(outs[i]).reshape(n, *rs["out_avals"][i].shape)[c]
                    for i, nm in enumerate(rs["out_names"])})
    return res


def _run(nc, maps, cores):
    import time as _t
    from concourse.bass_utils import run_bass_kernel_spmd
    try:
        concat_in = _precat(nc, maps)
        t0 = _t.perf_counter()
        res = _run_fast(nc, maps, concat_in)
        if TRACE:
            LAST_EXEC_NS.append(int((_t.perf_counter() - t0) * 1e9))
        return res
    except Exception:
        import traceback
        traceback.print_exc()
        _RUNNERS.pop(id(nc), None)
    t0 = _t.perf_counter()
    r = run_bass_kernel_spmd(nc, maps, cores)
    if TRACE:
        LAST_EXEC_NS.append(int((_t.perf_counter() - t0) * 1e9))
    return r.results


def kernel(**inputs):
    W_ = {k: np.asarray(v) for k, v in inputs.items()}
    x = np.asarray(W_["x"], np.float32)
    P = _programs(W_)
    cores = list(range(NCORE))
    LAST_EXEC_NS.clear()

    if FUSED:
        res = _run(P["fused"], fused_inputs(x), cores)
        return l3_gather(res)

    m1 = l1_inputs(x)
    h = l1_gather(_run(P["l1"], m1, cores))

    m2 = l2_inputs(h)
    r2 = _run(P["l2"], m2, cores)
    y_by_core = [r2[i]["y"] for i in range(NCORE)]

    m3 = l3_inputs(y_by_core, x)
    return l3_gather(_run(P["l3"], m3, cores))

